# revision 16
# baseline (speedup 1.0000x reference)
"""Fused Trainium2 kernel for nn_MultiHeadRelationalModule.

Data-parallel over 8 NeuronCores (8 samples each). The whole per-sample
pipeline (conv1 -> conv2 -> +coords -> K/Q/V proj -> LayerNorm ->
relational attention (4 heads, 596x596) -> softmax -> weighted sum ->
lin1 -> LN -> maxpool -> lin2 -> elu) runs on-chip; the big attention
maps never touch HBM.

v2: all large matmuls run in bf16 (4x faster per PE row than fp32 on
TRN2; fp32 needs 4 cycles/row, bf16 needs 1). PSUM accumulation stays
fp32. Q+K projections merged into one 128-partition matmul per head;
V projections merged across heads. Elementwise work balanced across
Act/DVE/Pool engines.

Key identities used:
  elu(x) + 1 == max(x + 1, min(exp(x), 1))        (exact)
  A' = elu(z)+1 fed to matmul with alin_w: subtract colsum(alin_w) in the
       following bias to undo the +1 (softmax bias becomes
       alin_b - alin_w.sum(0)).
  softmax over c2 with A2^T layout (c2 on partitions): exp on chip,
       denominator via an appended ones-column on V in the E matmul.
  LN(x) = (x - mu) * rsqrt(var + eps); affine params in this model are
       identity (ones/zeros), verified at runtime.
  max-pool commutes with the final LN (monotone affine map).
"""

import numpy as np
from contextlib import ExitStack

import concourse.bacc as bacc
import concourse.bass as bass
import concourse.mybir as mybir
import concourse.tile as tile
from concourse.bass_utils import run_bass_kernel_spmd

F32 = mybir.dt.float32
BF16 = mybir.dt.bfloat16
FP8 = mybir.dt.float8e4
ALSC = 16.0  # alin pre-scale into fp8e4m3 normal range; undone in exp2 scale
AF = mybir.ActivationFunctionType
ALU = mybir.AluOpType

N_CORES = 8
SPB = 8               # samples per core
N_PIX = 596
HEADS = 4
D = 64
CH = [(0, 128), (128, 256), (256, 384), (384, 512), (512, 596)]
FH = [(0, 512), (512, 596)]
SHIFTS = [(0, 0), (0, 1), (1, 0), (1, 1)]
LN_N = float(HEADS * N_PIX * D)       # 152576
LN2_N = float(N_PIX * D)              # 38144
EPS = 1e-5

_cache = {}


def _prep_consts(inp):
    """Host-side preprocessing of weights into kernel-friendly layouts."""
    f = np.float32
    c = {}
    conv1_w = np.asarray(inp["conv1_w"], f)
    c["w1s"] = np.ascontiguousarray(
        np.concatenate([conv1_w[:, :, di, dj].T for (di, dj) in SHIFTS], axis=1)
    )  # (4, 64)
    c["b1"] = np.ascontiguousarray(np.asarray(inp["conv1_b"], f)[:, None])  # (16,1)
    conv2_w = np.asarray(inp["conv2_w"], f)
    c["w2s"] = np.ascontiguousarray(
        np.concatenate([conv2_w[:, :, di, dj].T for (di, dj) in SHIFTS], axis=1)
    )  # (16, 128)
    c["b2"] = np.ascontiguousarray(np.asarray(inp["conv2_b"], f)[:, None])  # (32,1)

    p = np.arange(N_PIX)
    c["coords"] = np.ascontiguousarray(
        np.stack([(p % 4) / 4.0, (p // 4) / 149.0]).astype(f)
    )  # (2, 596)

    # Q/K projection merged per head: cols h*128:h*128+64 = Q (stacked rows
    # 0:64), cols h*128+64:h*128+128 = K (stacked rows 64:128).
    qp_w = np.asarray(inp["qp_w"], f)
    kp_w = np.asarray(inp["kp_w"], f)
    kqw2 = np.zeros((34, 512), f)
    qkb2 = np.zeros((128, HEADS), f)
    for h in range(HEADS):
        kqw2[:, h * 128:h * 128 + 64] = qp_w[:, h * 64:(h + 1) * 64]
        kqw2[:, h * 128 + 64:h * 128 + 128] = kp_w[:, h * 64:(h + 1) * 64]
        qkb2[0:64, h] = np.asarray(inp["qp_b"], f)[h * 64:(h + 1) * 64]
        qkb2[64:128, h] = np.asarray(inp["kp_b"], f)[h * 64:(h + 1) * 64]
    c["kqw2"] = kqw2
    c["qkb2"] = qkb2

    c["vw"] = np.ascontiguousarray(np.asarray(inp["vp_w"], f))  # (34, 256)
    vbb2 = np.zeros((128, 512), f)   # per head: [V bias (64) | 0 (ones blk)]
    for h in range(HEADS):
        vbb2[:, h * 128:h * 128 + 64] = np.asarray(inp["vp_b"], f)[None,
                                                                   h * 64:(h + 1) * 64]
    c["vbb2"] = vbb2

    c["qklin"] = np.ascontiguousarray(
        np.concatenate([np.asarray(inp["qlin_w"], f),
                        np.asarray(inp["klin_w"], f)], axis=0)
    )  # (128, 596): rows 0:64 qlin (Q), 64:128 klin (K)

    qkbias = np.zeros((128, 10), f)
    qkl_b = np.asarray(inp["qlin_b"], f) + np.asarray(inp["klin_b"], f)
    for ci, (c0, c1) in enumerate(CH):
        qkbias[0:c1 - c0, ci] = qkl_b[c0:c1]
        qkbias[0:c1 - c0, 5 + ci] = qkl_b[c0:c1] + 1.0
    c["qkbias"] = qkbias

    c["alin"] = np.ascontiguousarray(np.asarray(inp["alin_w"], f))  # (596, 596)

    expb = np.zeros((128, 5), f)
    eb = np.asarray(inp["alin_b"], f) - np.asarray(inp["alin_w"], f).sum(axis=0)
    for ci, (c0, c1) in enumerate(CH):
        expb[0:c1 - c0, ci] = eb[c0:c1]
    c["expb"] = expb

    l1 = np.zeros((128, 128), f)
    lin1_w = np.asarray(inp["lin1_w"], f)
    l1[:, 0:64] = lin1_w[0:128]
    l1[:, 64:128] = lin1_w[128:256]
    c["lin1w"] = l1
    c["bl1"] = np.ascontiguousarray(np.asarray(inp["lin1_b"], f)[:, None])  # (64,1)
    c["lin2w"] = np.ascontiguousarray(np.asarray(inp["lin2_w"], f))  # (64,10)
    bl2 = np.zeros((10, 2), f)
    bl2[:, 0] = np.asarray(inp["lin2_b"], f)
    bl2[:, 1] = np.asarray(inp["lin2_b"], f) + 1.0
    c["bl2"] = bl2
    c["ones_r"] = np.ones((1, 128), f)
    c["ones_c"] = np.ones((128, 1), f)
    c["epsc"] = np.full((1, 1), EPS, f)
    c["id34"] = np.eye(34, dtype=f)
    # LN-stat helper constants: per tensor T in (Q, K, V) with weights W_T
    # (34, 256) and bias b_T: sum(T) = s^T W_T 1 + 596*sum(b),
    # ssq(T) = sum_k w_k^T G w_k + 2 s^T (W_T b_T) + 596*||b_T||^2.
    wsum3 = np.zeros((34, 3), f)
    wb3 = np.zeros((34, 3), f)
    c3k = np.zeros((1, 3), f)
    cs3k = np.zeros((1, 3), f)
    for i, (wn, bn) in enumerate((("qp_w", "qp_b"), ("kp_w", "kp_b"),
                                  ("vp_w", "vp_b"))):
        W = np.asarray(inp[wn], np.float64)
        b = np.asarray(inp[bn], np.float64)
        wsum3[:, i] = W.sum(axis=1).astype(f)
        wb3[:, i] = (2.0 * (W @ b)).astype(f)
        c3k[0, i] = np.float32(596.0 * float(b @ b) / LN_N)
        cs3k[0, i] = np.float32(596.0 * float(b.sum()) / LN_N)
    c["wsum3"] = wsum3
    c["wb3"] = wb3
    c["c3k"] = c3k
    c["cs3k"] = cs3k
    return c


CONST_SHAPES = {
    "w1s": (4, 64), "b1": (16, 1), "w2s": (16, 128), "b2": (32, 1),
    "coords": (2, N_PIX), "kqw2": (34, 512), "qkb2": (128, HEADS),
    "vw": (34, 256), "vbb2": (128, 512),
    "qklin": (128, N_PIX), "qkbias": (128, 10), "alin": (N_PIX, N_PIX),
    "expb": (128, 5), "lin1w": (128, 128), "bl1": (64, 1), "lin2w": (64, 10),
    "bl2": (10, 2), "ones_r": (1, 128), "ones_c": (128, 1), "epsc": (1, 1),
    "id34": (34, 34), "wsum3": (34, 3), "wb3": (34, 3), "c3k": (1, 3),
    "cs3k": (1, 3),
}


def build_nc(spb=SPB):
    """Build the Bass program (same program runs SPMD on each core)."""
    nc = bacc.Bacc("TRN2", target_bir_lowering=False, debug=False)

    x_dram = nc.dram_tensor("x", [spb, 4, 151, 6], F32, kind="ExternalInput").ap()
    out_dram = nc.dram_tensor("out", [spb, 10], F32, kind="ExternalOutput").ap()
    cdram = {
        k: nc.dram_tensor(k, list(v), F32, kind="ExternalInput").ap()
        for k, v in CONST_SHAPES.items()
    }

    with tile.TileContext(nc) as tc, ExitStack() as ctx:
        pc = ctx.enter_context(tc.tile_pool(name="consts", bufs=1))
        # SBUF pools
        px = ctx.enter_context(tc.tile_pool(name="px", bufs=2))
        ph1 = ctx.enter_context(tc.tile_pool(name="ph1", bufs=2))
        pfeat = ctx.enter_context(tc.tile_pool(name="pfeat", bufs=2))
        pqk = ctx.enter_context(tc.tile_pool(name="pqk", bufs=8))
        pqkb = ctx.enter_context(tc.tile_pool(name="pqkb", bufs=8))
        pv = ctx.enter_context(tc.tile_pool(name="pv", bufs=12))
        pat = ctx.enter_context(tc.tile_pool(name="pat", bufs=10))
        pet = ctx.enter_context(tc.tile_pool(name="pet", bufs=3))
        pext = ctx.enter_context(tc.tile_pool(name="pext", bufs=7))
        psq = ctx.enter_context(tc.tile_pool(name="psq", bufs=2))
        pst = ctx.enter_context(tc.tile_pool(name="pst", bufs=3))
        peall = ctx.enter_context(tc.tile_pool(name="peall", bufs=4))
        pfix = ctx.enter_context(tc.tile_pool(name="pfix", bufs=1))
        # PSUM pools (8 banks total: 2+2+2+2), phase-separated so sample
        # s+1's front-end never waits on sample s's tail.
        PS = bass.MemorySpace.PSUM
        ps_front = ctx.enter_context(tc.tile_pool(name="ps_front", bufs=1, space=PS))
        ps_at = ctx.enter_context(tc.tile_pool(name="ps_at", bufs=4, space=PS))
        ps_a2 = ctx.enter_context(tc.tile_pool(name="ps_a2", bufs=2, space=PS))
        ps_e = ctx.enter_context(tc.tile_pool(name="ps_e", bufs=1, space=PS))

        # ---- prefetch sample 0's input before the const DMAs ----
        x_t0 = px.tile([4, 151, 6], F32, name="x_t", tag="x")
        nc.sync.dma_start(out=x_t0[:, :, :], in_=x_dram[0])

        # ---- load constants (fp32) ----
        csb = {}
        for k, shp in CONST_SHAPES.items():
            if k == "alin":
                continue
            t = pc.tile(list(shp), F32, name=f"c_{k}")
            nc.sync.dma_start(out=t[:, :], in_=cdram[k][:, :])
            csb[k] = t
        alin_f32 = []
        for ci, (c0, c1) in enumerate(CH):
            t = pc.tile([c1 - c0, N_PIX], F32, name=f"c_alin{ci}")
            nc.sync.dma_start(out=t[:, :], in_=cdram["alin"][c0:c1, :])
            alin_f32.append(t)

        # ---- one-time bf16 conversions of matmul operands ----
        def to_bf(name, src, shp):
            t = pc.tile(list(shp), BF16, name=name)
            nc.vector.tensor_copy(t[:, :], src[:, :])
            return t

        w1s_bf = to_bf("w1s_bf", csb["w1s"], (4, 64))
        w2s_bf = to_bf("w2s_bf", csb["w2s"], (16, 128))
        coords_bf = to_bf("coords_bf", csb["coords"], (2, N_PIX))
        kqw2_bf = to_bf("kqw2_bf", csb["kqw2"], (34, 512))
        vw_bf = to_bf("vw_bf", csb["vw"], (34, 256))
        qklin_bf = to_bf("qklin_bf", csb["qklin"], (128, N_PIX))
        lin1w_bf = to_bf("lin1w_bf", csb["lin1w"], (128, 128))
        alin_bf = [to_bf(f"alin_bf{ci}", alin_f32[ci], (c1 - c0, N_PIX))
                   for ci, (c0, c1) in enumerate(CH)]
        id34_bf = to_bf("id34_bf", csb["id34"], (34, 34))
        ones_bf = pc.tile([128, 1], BF16, name="ones_bf")
        nc.vector.memset(ones_bf[:, :], 1.0)
        emax_all = pfix.tile([64, spb], F32, name="emax_all")
        emax_raw = pfix.tile([64, spb], F32, name="emax_raw")
        stats2_all = pfix.tile([1, 2 * spb], F32, name="stats2_all")

        # ================= pipelined per-sample stages =================
        W84 = 84 * HEADS
        vbb3c = csb["vbb2"].rearrange("p (h c) -> p h c", c=128)

        def front_a(s):
            """x load/cast + conv1 + conv2 + coords -> feats."""
            S = {"s": s}
            if s == 0:
                x_t = x_t0
            else:
                x_t = px.tile([4, 151, 6], F32, name="x_t", tag="x")
                nc.sync.dma_start(out=x_t[:, :, :], in_=x_dram[s])
            x_bf = px.tile([4, 151, 6], BF16, name="x_bf", tag="xbf")
            nc.gpsimd.tensor_copy(x_bf[:, :, :], x_t[:, :, :])

            h1 = ph1.tile([16, 750], BF16, name="h1", tag="h1")
            h1v = h1.rearrange("c (h w) -> c h w", w=5)
            for (r0, nr, dst0) in ((0, 102, 0), (102, 48, 510)):
                cps = ps_front.tile([16, nr * 5], F32, name="c1ps", tag="fr")
                for si, (di, dj) in enumerate(SHIFTS):
                    nc.tensor.matmul(
                        cps[:, :],
                        w1s_bf[:, si * 16:(si + 1) * 16],
                        x_bf[:, di + r0:di + r0 + nr, dj:dj + 5],
                        start=(si == 0), stop=(si == 3),
                    )
                nc.scalar.activation(h1[:, dst0:dst0 + nr * 5], cps[:, :],
                                     AF.Relu, bias=csb["b1"][:, 0:1])

            feats = pfeat.tile([34, N_PIX], BF16, name="feats", tag="feats")
            nc.gpsimd.tensor_copy(feats[32:34, :], coords_bf[:, :])
            for (r0, nr, dst0) in ((0, 128, 0), (128, 21, 512)):
                cps = ps_front.tile([32, nr * 4], F32, name="c2ps", tag="fr")
                for si, (di, dj) in enumerate(SHIFTS):
                    nc.tensor.matmul(
                        cps[:, :],
                        w2s_bf[:, si * 32:(si + 1) * 32],
                        h1v[:, di + r0:di + r0 + nr, dj:dj + 4],
                        start=(si == 0), stop=(si == 3),
                    )
                nc.scalar.activation(feats[0:32, dst0:dst0 + nr * 4], cps[:, :],
                                     AF.Relu, bias=csb["b2"][:, 0:1])
            S["feats"] = feats
            return S

        def front_b(S):
            """LN stats from s/G on the PE, then K/Q/V projections."""
            feats = S["feats"]
            # s = sum_f feats[:, f]; G = feats @ feats^T (via PE transposes)
            s_sb = pst.tile([34, 1], F32, name="s_sb", tag="s_sb")
            nc.vector.tensor_reduce(s_sb[:, :], feats[:, :],
                                    axis=mybir.AxisListType.X, op=ALU.add)
            g_ps = ps_front.tile([34, 34], F32, name="g_ps", tag="fr")
            for ci, (c0, c1) in enumerate(CH):
                csz = c1 - c0
                ft_ps = ps_a2.tile([128, 34], BF16, name="ft_ps", tag="a2")
                nc.tensor.transpose(ft_ps[0:csz, :], feats[:, c0:c1],
                                    id34_bf[:, :])
                ft_sb = pst.tile([128, 34], BF16, name="ft_sb", tag="ft")
                nc.vector.tensor_copy(ft_sb[0:csz, :], ft_ps[0:csz, :])
                nc.tensor.matmul(g_ps[:, :], ft_sb[0:csz, :],
                                 ft_sb[0:csz, :],
                                 start=(ci == 0), stop=(ci == 4))
            g_sb = pst.tile([34, 34], BF16, name="g_sb", tag="g_sb")
            nc.vector.tensor_copy(g_sb[:, :], g_ps[:, :])
            gw2_ps = ps_front.tile([34, 512], F32, name="gw2_ps", tag="fr")
            nc.tensor.matmul(gw2_ps[:, :], g_sb[:, :], kqw2_bf[:, :],
                             start=True, stop=True)
            d2 = psq.tile([34, 768], BF16, name="d2", tag="d2")
            nc.vector.tensor_tensor(d2[:, 0:512], csb["kqw2"][:, :],
                                    gw2_ps[:, :], op=ALU.mult)
            gwv_ps = ps_front.tile([34, 256], F32, name="gwv_ps", tag="fr")
            nc.tensor.matmul(gwv_ps[:, :], g_sb[:, :], vw_bf[:, :],
                             start=True, stop=True)
            nc.vector.tensor_tensor(d2[:, 512:768], csb["vw"][:, :],
                                    gwv_ps[:, :], op=ALU.mult)
            cs2_ps = ps_front.tile([1, 512], F32, name="cs2_ps", tag="fr")
            nc.tensor.matmul(cs2_ps[:, :], ones_bf[0:34, 0:1], d2[:, 0:512],
                             start=True, stop=True)
            csv_ps = ps_front.tile([1, 256], F32, name="csv_ps", tag="fr")
            nc.tensor.matmul(csv_ps[:, :], ones_bf[0:34, 0:1], d2[:, 512:768],
                             start=True, stop=True)
            # per-(h, qk) partial ssq, then fold heads
            r1 = pst.tile([1, 8], F32, name="r1", tag="r1")
            nc.vector.tensor_reduce(
                r1[:, :].rearrange("p (h t u) -> p h t u", t=2, u=1),
                cs2_ps[:, :].rearrange("p (h t d) -> p h t d", t=2, d=64),
                axis=mybir.AxisListType.X, op=ALU.add)
            ssqr = pst.tile([1, 3], F32, name="ssqr", tag="ssqr")
            nc.vector.tensor_reduce(
                ssqr[:, 0:2].rearrange("p (t u) -> p t u", u=1),
                r1[:, :].rearrange("p (h t) -> p t h", t=2),
                axis=mybir.AxisListType.X, op=ALU.add)
            nc.vector.tensor_reduce(ssqr[:, 2:3], csv_ps[:, :],
                                    axis=mybir.AxisListType.X, op=ALU.add)
            stats_ps = ps_front.tile([1, 6], F32, name="stats_ps", tag="fr")
            nc.tensor.matmul(stats_ps[0:1, 0:3], s_sb[:, :],
                             csb["wsum3"][:, :], start=True, stop=True)
            nc.tensor.matmul(stats_ps[0:1, 3:6], s_sb[:, :],
                             csb["wb3"][:, :], start=True, stop=True)
            mu3 = pst.tile([1, 3], F32, name="mu3", tag="mu3")
            nc.vector.scalar_tensor_tensor(mu3[:, :], stats_ps[0:1, 0:3],
                                           1.0 / LN_N, csb["cs3k"][0:1, :],
                                           op0=ALU.mult, op1=ALU.add)
            tsq = pst.tile([1, 3], F32, name="tsq", tag="tsq")
            nc.vector.tensor_tensor(tsq[:, :], ssqr[:, :], stats_ps[0:1, 3:6],
                                    op=ALU.add)
            msq3 = pst.tile([1, 3], F32, name="msq3", tag="msq3")
            nc.vector.scalar_tensor_tensor(msq3[:, :], tsq[:, :], 1.0 / LN_N,
                                           csb["c3k"][0:1, :],
                                           op0=ALU.mult, op1=ALU.add)
            S["mu3"] = mu3
            S["msq3"] = msq3

            # projections (plain copies; no accumulation needed)
            stacked = []
            for h in range(HEADS):
                st_t = pqk.tile([128, N_PIX], BF16, name="st_t", tag="qk")
                stacked.append(st_t)
                pps = ps_front.tile([128, 512], F32, name="pps", tag="fr")
                pps2 = ps_front.tile([128, 84], F32, name="pps2", tag="fr")
                nc.tensor.matmul(pps[:, :], kqw2_bf[:, h * 128:(h + 1) * 128],
                                 feats[:, 0:512], start=True, stop=True)
                nc.tensor.matmul(pps2[:, :], kqw2_bf[:, h * 128:(h + 1) * 128],
                                 feats[:, 512:596], start=True, stop=True)
                nc.vector.tensor_scalar_add(st_t[:, 0:512], pps[:, :],
                                            csb["qkb2"][:, h:h + 1])
                nc.vector.tensor_scalar_add(st_t[:, 512:596], pps2[:, :],
                                            csb["qkb2"][:, h:h + 1])

            vtiles = []
            for ci, (c0, c1) in enumerate(CH):
                csz = c1 - c0
                vps = ps_front.tile([128, 256], F32, name="vps", tag="fr")
                nc.tensor.matmul(vps[0:csz, :], feats[:, c0:c1],
                                 vw_bf[:, :], start=True, stop=True)
                vt = pv.tile([128, 512], BF16, name="vt", tag="v")
                vt3 = vt.rearrange("p (h c) -> p h c", c=128)
                vps3 = vps.rearrange("p (h c) -> p h c", c=64)
                nc.vector.memset(vt3[0:csz, :, 64:128], 1.0)
                nc.vector.scalar_tensor_tensor(
                    vt3[0:csz, :, 0:64], vps3[0:csz, :, :], 1.0,
                    vbb3c[0:csz, :, 0:64],
                    op0=ALU.mult, op1=ALU.add)
                vtiles.append(vt)
            S["stacked"] = stacked
            S["vtiles"] = vtiles
            return S

        def front_c(S):
            """LN scalar pipeline + LN apply (fp32 -> bf16)."""
            mu3, msq3 = S["mu3"], S["msq3"]
            nmu2 = pst.tile([1, 3], F32, name="nmu2", tag="nmu2")
            nc.vector.scalar_tensor_tensor(nmu2[:, :], mu3[:, :], -1.0,
                                           mu3[:, :],
                                           op0=ALU.mult, op1=ALU.mult)
            var3 = pst.tile([1, 3], F32, name="var3", tag="var3")
            nc.vector.tensor_tensor(var3[:, :], msq3[:, :], nmu2[:, :],
                                    op=ALU.add)
            std3 = pst.tile([1, 3], F32, name="std3", tag="std3")
            nc.scalar.activation(std3[:, :], var3[:, :], AF.Sqrt,
                                 bias=csb["epsc"][0:1, 0:1])
            rsnmr = pst.tile([1, 6], F32, name="rsnmr", tag="rsnmr")
            rsv = rsnmr.rearrange("p (a b) -> p a b", b=2)
            nc.vector.reciprocal(rsv[:, :, 0:1], std3[:, :])
            nc.vector.scalar_tensor_tensor(rsv[:, :, 1:2], mu3[:, :], -1.0,
                                           rsv[:, :, 0:1],
                                           op0=ALU.mult, op1=ALU.mult)
            bc_ps = ps_a2.tile([128, 6], F32, name="bc_ps", tag="a2")
            nc.tensor.matmul(bc_ps[:, :], csb["ones_r"][0:1, :], rsnmr[:, :],
                             start=True, stop=True)
            bc = pst.tile([128, 6], F32, name="bc", tag="bc")
            nc.vector.tensor_copy(bc[:, :], bc_ps[:, :])
            # bc cols: [rsQ, nmrQ, rsK, nmrK, rsV, nmrV]

            stacked_bf = []
            for h in range(HEADS):
                sb = pqkb.tile([128, N_PIX], BF16, name="st_bf", tag="qkb")
                stacked_bf.append(sb)
                nc.vector.tensor_scalar(sb[0:64, :], S["stacked"][h][0:64, :],
                                        bc[0:64, 0:1], bc[0:64, 1:2],
                                        op0=ALU.mult, op1=ALU.add)
                nc.vector.tensor_scalar(sb[64:128, :],
                                        S["stacked"][h][64:128, :],
                                        bc[0:64, 2:3], bc[0:64, 3:4],
                                        op0=ALU.mult, op1=ALU.add)
            for ci, (c0, c1) in enumerate(CH):
                csz = c1 - c0
                vt3 = S["vtiles"][ci].rearrange("p (h c) -> p h c", c=128)
                nc.vector.tensor_scalar(vt3[0:csz, :, 0:64],
                                        vt3[0:csz, :, 0:64],
                                        bc[0:csz, 4:5], bc[0:csz, 5:6],
                                        op0=ALU.mult, op1=ALU.add)
            S["stacked_bf"] = stacked_bf
            S["eall"] = [peall.tile([128, N_PIX], BF16, name=f"eall{i}",
                                    tag="eall") for i in range(2)]
            return S

        # ---- attention stages (pipeline carried across samples) ----
        def at_chunk(S, p, ci, dest):
            c0, c1 = CH[ci]
            csz = c1 - c0
            w = 512 if not p["merged"] else W84
            aps = ps_at.tile([128, 512], F32, name="aps", tag="at")
            if p["merged"]:
                for h in range(HEADS):
                    nc.tensor.matmul(aps[0:csz, h * 84:(h + 1) * 84],
                                     qklin_bf[:, c0:c1],
                                     S["stacked_bf"][h][:, 512:596],
                                     start=True, stop=True)
            else:
                nc.tensor.matmul(aps[0:csz, 0:512],
                                 qklin_bf[:, c0:c1],
                                 S["stacked_bf"][p["h"]][:, 0:512],
                                 start=True, stop=True)
            et = pet.tile([128, 512], F32, name="et", tag="et")
            nc.scalar.activation(et[0:csz, 0:w], aps[0:csz, 0:w],
                                 AF.Exp,
                                 bias=csb["qkbias"][0:csz, ci:ci + 1])
            nc.gpsimd.tensor_scalar_min(et[0:csz, 0:w],
                                        et[0:csz, 0:w], 1.0)
            nc.vector.scalar_tensor_tensor(
                dest[0:csz, 0:w], aps[0:csz, 0:w],
                csb["qkbias"][0:csz, 5 + ci:6 + ci],
                et[0:csz, 0:w], op0=ALU.add, op1=ALU.max)

        def e_c2(st, c2i):
            S, p, tiles = st["S"], st["p"], st["tiles"]
            c20, c21 = CH[c2i]
            c2sz = c21 - c20
            w = 512 if not p["merged"] else W84
            if c2i == 0:
                st["eps"] = ps_e.tile([128, 512], F32, name="eps_t", tag="e")
            eps_t = st["eps"]
            a2ps = ps_a2.tile([128, 512], F32, name="a2ps", tag="a2")
            for ci, (c0, c1) in enumerate(CH):
                csz = c1 - c0
                nc.tensor.matmul(a2ps[0:c2sz, 0:w],
                                 alin_bf[ci][:, c20:c21],
                                 tiles[ci][0:csz, 0:w],
                                 start=(ci == 0), stop=(ci == 4))
            ext = pext.tile([128, 512], BF16, name="ext", tag="ext")
            nc.scalar.activation(ext[0:c2sz, 0:w],
                                 a2ps[0:c2sz, 0:w], AF.Exp,
                                 bias=csb["expb"][0:c2sz, c2i:c2i + 1])
            if p["merged"]:
                # PSUM accumulation groups must not interleave within a
                # bank's 2KB zero region: buffer the ext tiles and run the
                # four per-head accumulations sequentially in e_tail.
                st.setdefault("exts", []).append(ext)
            else:
                nc.tensor.matmul(eps_t[:, 0:512],
                                 S["vtiles"][c2i][0:c2sz,
                                                  p["h"] * 128:
                                                  (p["h"] + 1) * 128],
                                 ext[0:c2sz, 0:512],
                                 start=(c2i == 0), stop=(c2i == 4))

        def e_tail(st):
            S, p, eps_t = st["S"], st["p"], st["eps"]
            w = 512 if not p["merged"] else W84
            eall = S["eall"]
            if p["merged"]:
                for h in range(HEADS):
                    for c2i, (c20, c21) in enumerate(CH):
                        c2sz = c21 - c20
                        nc.tensor.matmul(
                            eps_t[:, h * 84:(h + 1) * 84],
                            S["vtiles"][c2i][0:c2sz, h * 128:(h + 1) * 128],
                            st["exts"][c2i][0:c2sz, h * 84:(h + 1) * 84],
                            start=(c2i == 0), stop=(c2i == 4))
            recip64 = pst.tile([64, 512], F32, name="recip64", tag="recip")
            nc.vector.reciprocal(recip64[:, 0:w], eps_t[64:128, 0:w])
            if p["merged"]:
                for h in range(HEADS):
                    nc.vector.tensor_tensor(
                        eall[h // 2][(h % 2) * 64:(h % 2) * 64 + 64, 512:596],
                        eps_t[0:64, h * 84:(h + 1) * 84],
                        recip64[:, h * 84:(h + 1) * 84], op=ALU.mult)
            else:
                h = p["h"]
                nc.vector.tensor_tensor(
                    eall[h // 2][(h % 2) * 64:(h % 2) * 64 + 64, 0:512],
                    eps_t[0:64, 0:512], recip64[:, 0:512], op=ALU.mult)

        pending = [None]

        def do_pass(S, p):
            tiles = [pat.tile([128, 512], BF16, name=f"att{i}", tag="atile")
                     for i in range(5)]
            for ci in range(4):
                at_chunk(S, p, ci, tiles[ci][:, :])
            prev = pending[0]
            if prev is None:
                at_chunk(S, p, 4, tiles[4][:, :])
            else:
                e_c2(prev, 0)
                e_c2(prev, 1)
                e_c2(prev, 2)
                at_chunk(S, p, 4, tiles[4][:, :])
                e_c2(prev, 3)
                e_c2(prev, 4)
                e_tail(prev)
            pending[0] = {"S": S, "p": p, "tiles": tiles}

        def flush_pipe():
            prev = pending[0]
            for c2i in range(5):
                e_c2(prev, c2i)
            e_tail(prev)
            pending[0] = None

        def tail(S):
            """lin1 + LN2 raw stats (scalar pipeline batched at the end)."""
            s, eall = S["s"], S["eall"]
            e2 = psq.tile([64, N_PIX], F32, name="e2", tag="e2")
            ls2 = pst.tile([64, 2], F32, name="ls2", tag="ls2")
            lpart = pst.tile([64, 2], F32, name="lpart", tag="lpart")
            for (f0, f1) in FH:
                fsz = f1 - f0
                lps = ps_e.tile([64, 512], F32, name="lps", tag="e")
                for ck in range(2):
                    nc.tensor.matmul(lps[:, 0:fsz],
                                     lin1w_bf[:, ck * 64:(ck + 1) * 64],
                                     eall[ck][:, f0:f1],
                                     start=(ck == 0), stop=(ck == 1))
                nc.scalar.activation(e2[:, f0:f1], lps[:, 0:fsz], AF.Relu,
                                     bias=csb["bl1"][:, 0:1],
                                     accum_out=lpart[:, (0 if f0 == 0 else 1):
                                                     (1 if f0 == 0 else 2)])
            nc.vector.tensor_reduce(ls2[:, 0:1], lpart[:, :],
                                    axis=mybir.AxisListType.X, op=ALU.add)
            sqe = psq.tile([64, N_PIX], F32, name="sqe", tag="sqe")
            nc.scalar.activation(sqe[:, :], e2[:, :], AF.Square,
                                 accum_out=ls2[:, 1:2])
            nc.vector.tensor_reduce(emax_raw[:, s:s + 1], e2[:, :],
                                    axis=mybir.AxisListType.X, op=ALU.max)
            st2 = ps_at.tile([1, 2], F32, name="st2", tag="at")
            nc.tensor.matmul(st2[0:1, :], csb["ones_c"][0:64, 0:1], ls2[:, :],
                             start=True, stop=True)
            nc.vector.tensor_copy(stats2_all[:, 2 * s:2 * s + 2], st2[0:1, :])

        # ---- pipelined schedule: sample s+1's front-end is emitted between
        # sample s's attention passes; the at/e pass pipeline is carried
        # across the sample boundary.
        S = front_a(0)
        front_b(S)
        front_c(S)
        states = {0: S}
        for s in range(spb):
            S = states[s]
            plist = ([dict(h=h, merged=False) for h in range(HEADS)]
                     + [dict(h=None, merged=True)])
            do_pass(S, plist[0])
            if s > 0:
                tail(states.pop(s - 1))
            do_pass(S, plist[1])
            if s + 1 < spb:
                Sn = front_a(s + 1)
            do_pass(S, plist[2])
            if s + 1 < spb:
                front_b(Sn)
            do_pass(S, plist[3])
            if s + 1 < spb:
                front_c(Sn)
                states[s + 1] = Sn
            do_pass(S, plist[4])
        flush_pipe()
        tail(states.pop(spb - 1))

        # ---------------- batched LN2 scalar pipeline (all samples) --------
        m2a = pst.tile([1, 2 * spb], F32, name="m2a", tag="m2a")
        m2av = m2a.rearrange("p (a b) -> p a b", b=2)
        nc.vector.tensor_scalar_mul(m2a[:, :], stats2_all[:, :], 1.0 / LN2_N)
        nmu2a = pst.tile([1, spb], F32, name="nmu2a", tag="nmu2a")
        nc.vector.scalar_tensor_tensor(nmu2a[:, :],
                                       m2av[:, :, 0:1], -1.0, m2av[:, :, 0:1],
                                       op0=ALU.mult, op1=ALU.mult)
        var2a = pst.tile([1, spb], F32, name="var2a", tag="var2a")
        nc.vector.tensor_tensor(var2a[:, :], m2av[:, :, 1:2], nmu2a[:, :],
                                op=ALU.add)
        std2a = pst.tile([1, spb], F32, name="std2a", tag="std2a")
        nc.scalar.activation(std2a[:, :], var2a[:, :], AF.Sqrt,
                             bias=csb["epsc"][0:1, 0:1])
        rs2a = pst.tile([1, 2 * spb], F32, name="rs2a", tag="rs2a")
        rs2av = rs2a.rearrange("p (a b) -> p a b", b=2)
        nc.vector.reciprocal(rs2av[:, :, 0:1], std2a[:, :])
        nc.vector.scalar_tensor_tensor(rs2av[:, :, 1:2],
                                       m2av[:, :, 0:1], -1.0,
                                       rs2av[:, :, 0:1],
                                       op0=ALU.mult, op1=ALU.mult)
        bc2p = ps_at.tile([64, 2 * spb], F32, name="bc2p", tag="at")
        nc.tensor.matmul(bc2p[:, :], csb["ones_r"][0:1, 0:64], rs2a[:, :],
                         start=True, stop=True)
        bc2 = pst.tile([64, 2 * spb], F32, name="bc2", tag="bc2")
        nc.vector.tensor_copy(bc2[:, :], bc2p[:, :])
        for s in range(spb):
            nc.vector.tensor_scalar(emax_all[:, s:s + 1], emax_raw[:, s:s + 1],
                                    bc2[:, 2 * s:2 * s + 1],
                                    bc2[:, 2 * s + 1:2 * s + 2],
                                    op0=ALU.mult, op1=ALU.add)

        # ---------------- lin2 + final elu ----------------
        l2ps = ps_at.tile([10, spb], F32, name="l2ps", tag="at")
        nc.tensor.matmul(l2ps[:, :], csb["lin2w"][:, :], emax_all[:, :],
                         start=True, stop=True)
        fe = pst.tile([10, spb], F32, name="fe", tag="fe")
        nc.scalar.activation(fe[:, :], l2ps[:, :], AF.Exp,
                             bias=csb["bl2"][:, 0:1])
        nc.vector.tensor_scalar(fe[:, :], fe[:, :], 1.0, -1.0,
                                op0=ALU.min, op1=ALU.add)
        out_sb = pst.tile([10, spb], F32, name="out_sb", tag="out_sb")
        nc.vector.scalar_tensor_tensor(out_sb[:, :], l2ps[:, :],
                                       csb["bl2"][:, 0:1], fe[:, :],
                                       op0=ALU.add, op1=ALU.max)
        nc.sync.dma_start(out=out_dram.rearrange("s t -> t s"), in_=out_sb[:, :])

    return nc


def _reference_numpy(inp):
    """Pure-numpy fallback (only used if LN affine params are nontrivial)."""
    def ln(x, g=None, b=None):
        axes = tuple(range(1, x.ndim))
        mu = x.mean(axis=axes, keepdims=True)
        var = x.var(axis=axes, keepdims=True)
        y = (x - mu) / np.sqrt(var + EPS)
        return y * g + b if g is not None else y

    def elu(x):
        return np.where(x > 0, x, np.expm1(np.minimum(x, 0)))

    x = np.asarray(inp["x"], np.float64)
    N = x.shape[0]
    w1, b1 = np.asarray(inp["conv1_w"], np.float64), np.asarray(inp["conv1_b"], np.float64)
    h = np.zeros((N, 16, 150, 5))
    for di in range(2):
        for dj in range(2):
            h += np.einsum("oc,nchw->nohw", w1[:, :, di, dj],
                           x[:, :, di:di + 150, dj:dj + 5])
    h = np.maximum(h + b1[None, :, None, None], 0)
    w2, b2 = np.asarray(inp["conv2_w"], np.float64), np.asarray(inp["conv2_b"], np.float64)
    h2 = np.zeros((N, 32, 149, 4))
    for di in range(2):
        for dj in range(2):
            h2 += np.einsum("oc,nchw->nohw", w2[:, :, di, dj],
                            h[:, :, di:di + 149, dj:dj + 4])
    h2 = np.maximum(h2 + b2[None, :, None, None], 0)
    p = np.arange(N_PIX)
    xc, yc = (p % 4) / 4.0, (p // 4) / 149.0
    feats = np.concatenate(
        [h2.transpose(0, 2, 3, 1).reshape(N, N_PIX, 32),
         np.broadcast_to(np.stack([xc, yc], 1)[None], (N, N_PIX, 2))], axis=2)

    def proj(wn, bn, gn, bn2):
        P = (feats @ np.asarray(inp[wn], np.float64) + np.asarray(inp[bn], np.float64))
        P = P.reshape(N, N_PIX, HEADS, D).transpose(0, 2, 1, 3)
        return ln(P, np.asarray(inp[gn], np.float64), np.asarray(inp[bn2], np.float64))

    K = proj("kp_w", "kp_b", "knorm_g", "knorm_b")
    Q = proj("qp_w", "qp_b", "qnorm_g", "qnorm_b")
    V = proj("vp_w", "vp_b", "vnorm_g", "vnorm_b")
    A = elu(Q @ np.asarray(inp["qlin_w"], np.float64) + np.asarray(inp["qlin_b"], np.float64)
            + K @ np.asarray(inp["klin_w"], np.float64) + np.asarray(inp["klin_b"], np.float64))
    A = A @ np.asarray(inp["alin_w"], np.float64) + np.asarray(inp["alin_b"], np.float64)
    A = A - A.max(axis=-1, keepdims=True)
    A = np.exp(A)
    A = A / A.sum(axis=-1, keepdims=True)
    E = np.einsum("bhfc,bhcd->bhfd", A, V)
    E = E.transpose(0, 2, 1, 3).reshape(N, N_PIX, HEADS * D)
    E = np.maximum(E @ np.asarray(inp["lin1_w"], np.float64)
                   + np.asarray(inp["lin1_b"], np.float64), 0)
    E = ln(E)
    E = E.max(axis=1)
    out = E @ np.asarray(inp["lin2_w"], np.float64) + np.asarray(inp["lin2_b"], np.float64)
    return elu(out).astype(np.float32)


def kernel(**inputs):
    trivial = (np.all(np.asarray(inputs["knorm_g"]) == 1.0)
               and np.all(np.asarray(inputs["knorm_b"]) == 0.0)
               and np.all(np.asarray(inputs["qnorm_g"]) == 1.0)
               and np.all(np.asarray(inputs["qnorm_b"]) == 0.0)
               and np.all(np.asarray(inputs["vnorm_g"]) == 1.0)
               and np.all(np.asarray(inputs["vnorm_b"]) == 0.0))
    if not trivial:
        return _reference_numpy(inputs)

    x = np.ascontiguousarray(np.asarray(inputs["x"], np.float32))
    n = x.shape[0]
    assert n == N_CORES * SPB, f"expected batch {N_CORES * SPB}, got {n}"
    consts = _prep_consts(inputs)

    if "nc" not in _cache:
        nc = build_nc(SPB)
        nc.compile()
        _cache["nc"] = nc
    nc = _cache["nc"]

    in_maps = []
    for c in range(N_CORES):
        m = dict(consts)
        m["x"] = np.ascontiguousarray(x[c * SPB:(c + 1) * SPB])
        in_maps.append(m)

    import os
    trace = bool(int(os.environ.get("KERNEL_TRACE", "0")))
    res = run_bass_kernel_spmd(nc, in_maps, list(range(N_CORES)), trace=trace)
    kernel._last_results = res
    out = np.concatenate([np.asarray(r["out"]) for r in res.results], axis=0)
    return out.astype(np.float32)


kernel._last_results = None


# revision 22
# speedup vs baseline: 1.0129x; 1.0129x over previous
"""Fused Trainium2 kernel for nn_MultiHeadRelationalModule.

Data-parallel over 8 NeuronCores (8 samples each). The whole per-sample
pipeline (conv1 -> conv2 -> +coords -> K/Q/V proj -> LayerNorm ->
relational attention (4 heads, 596x596) -> softmax -> weighted sum ->
lin1 -> LN -> maxpool -> lin2 -> elu) runs on-chip; the big attention
maps never touch HBM.

v2: all large matmuls run in bf16 (4x faster per PE row than fp32 on
TRN2; fp32 needs 4 cycles/row, bf16 needs 1). PSUM accumulation stays
fp32. Q+K projections merged into one 128-partition matmul per head;
V projections merged across heads. Elementwise work balanced across
Act/DVE/Pool engines.

Key identities used:
  elu(x) + 1 == max(x + 1, min(exp(x), 1))        (exact)
  A' = elu(z)+1 fed to matmul with alin_w: subtract colsum(alin_w) in the
       following bias to undo the +1 (softmax bias becomes
       alin_b - alin_w.sum(0)).
  softmax over c2 with A2^T layout (c2 on partitions): exp on chip,
       denominator via an appended ones-column on V in the E matmul.
  LN(x) = (x - mu) * rsqrt(var + eps); affine params in this model are
       identity (ones/zeros), verified at runtime.
  max-pool commutes with the final LN (monotone affine map).
"""

import numpy as np
from contextlib import ExitStack

import concourse.bacc as bacc
import concourse.bass as bass
import concourse.mybir as mybir
import concourse.tile as tile
from concourse.bass_utils import run_bass_kernel_spmd

F32 = mybir.dt.float32
BF16 = mybir.dt.bfloat16
FP8 = mybir.dt.float8e4
ALSC = 16.0  # alin pre-scale into fp8e4m3 normal range; undone in exp2 scale
AF = mybir.ActivationFunctionType
ALU = mybir.AluOpType

N_CORES = 8
SPB = 8               # samples per core
N_PIX = 596
HEADS = 4
D = 64
CH = [(0, 128), (128, 256), (256, 384), (384, 512), (512, 596)]
FH = [(0, 512), (512, 596)]
SHIFTS = [(0, 0), (0, 1), (1, 0), (1, 1)]
LN_N = float(HEADS * N_PIX * D)       # 152576
LN2_N = float(N_PIX * D)              # 38144
EPS = 1e-5

_cache = {}


def _prep_consts(inp):
    """Host-side preprocessing of weights into kernel-friendly layouts."""
    f = np.float32
    c = {}
    conv1_w = np.asarray(inp["conv1_w"], f)
    c["w1s"] = np.ascontiguousarray(
        np.concatenate([conv1_w[:, :, di, dj].T for (di, dj) in SHIFTS], axis=1)
    )  # (4, 64)
    c["b1"] = np.ascontiguousarray(np.asarray(inp["conv1_b"], f)[:, None])  # (16,1)
    conv2_w = np.asarray(inp["conv2_w"], f)
    c["w2s"] = np.ascontiguousarray(
        np.concatenate([conv2_w[:, :, di, dj].T for (di, dj) in SHIFTS], axis=1)
    )  # (16, 128)
    c["b2"] = np.ascontiguousarray(np.asarray(inp["conv2_b"], f)[:, None])  # (32,1)

    p = np.arange(N_PIX)
    c["coords"] = np.ascontiguousarray(
        np.stack([(p % 4) / 4.0, (p // 4) / 149.0]).astype(f)
    )  # (2, 596)

    # Q/K projection merged per head: cols h*128:h*128+64 = Q (stacked rows
    # 0:64), cols h*128+64:h*128+128 = K (stacked rows 64:128).
    qp_w = np.asarray(inp["qp_w"], f)
    kp_w = np.asarray(inp["kp_w"], f)
    kqw2 = np.zeros((34, 512), f)
    qkb2 = np.zeros((128, HEADS), f)
    for h in range(HEADS):
        kqw2[:, h * 128:h * 128 + 64] = qp_w[:, h * 64:(h + 1) * 64]
        kqw2[:, h * 128 + 64:h * 128 + 128] = kp_w[:, h * 64:(h + 1) * 64]
        qkb2[0:64, h] = np.asarray(inp["qp_b"], f)[h * 64:(h + 1) * 64]
        qkb2[64:128, h] = np.asarray(inp["kp_b"], f)[h * 64:(h + 1) * 64]
    c["kqw2"] = kqw2
    c["qkb2"] = qkb2

    c["vw"] = np.ascontiguousarray(np.asarray(inp["vp_w"], f))  # (34, 256)
    vbb2 = np.zeros((128, 512), f)   # per head: [V bias (64) | 0 (ones blk)]
    for h in range(HEADS):
        vbb2[:, h * 128:h * 128 + 64] = np.asarray(inp["vp_b"], f)[None,
                                                                   h * 64:(h + 1) * 64]
    c["vbb2"] = vbb2

    c["qklin"] = np.ascontiguousarray(
        np.concatenate([np.asarray(inp["qlin_w"], f),
                        np.asarray(inp["klin_w"], f)], axis=0)
    )  # (128, 596): rows 0:64 qlin (Q), 64:128 klin (K)

    qkbias = np.zeros((128, 10), f)
    qkl_b = np.asarray(inp["qlin_b"], f) + np.asarray(inp["klin_b"], f)
    for ci, (c0, c1) in enumerate(CH):
        qkbias[0:c1 - c0, ci] = qkl_b[c0:c1]
        qkbias[0:c1 - c0, 5 + ci] = qkl_b[c0:c1] + 1.0
    c["qkbias"] = qkbias

    c["alin"] = np.ascontiguousarray(np.asarray(inp["alin_w"], f))  # (596, 596)

    expb = np.zeros((128, 5), f)
    eb = np.asarray(inp["alin_b"], f) - np.asarray(inp["alin_w"], f).sum(axis=0)
    for ci, (c0, c1) in enumerate(CH):
        expb[0:c1 - c0, ci] = eb[c0:c1]
    c["expb"] = expb

    l1 = np.zeros((128, 128), f)
    lin1_w = np.asarray(inp["lin1_w"], f)
    l1[:, 0:64] = lin1_w[0:128]
    l1[:, 64:128] = lin1_w[128:256]
    c["lin1w"] = l1
    c["bl1"] = np.ascontiguousarray(np.asarray(inp["lin1_b"], f)[:, None])  # (64,1)
    c["lin2w"] = np.ascontiguousarray(np.asarray(inp["lin2_w"], f))  # (64,10)
    bl2 = np.zeros((10, 2), f)
    bl2[:, 0] = np.asarray(inp["lin2_b"], f)
    bl2[:, 1] = np.asarray(inp["lin2_b"], f) + 1.0
    c["bl2"] = bl2
    c["ones_r"] = np.ones((1, 128), f)
    c["ones_c"] = np.ones((128, 1), f)
    c["epsc"] = np.full((1, 1), EPS, f)
    c["id34"] = np.eye(34, dtype=f)
    # LN-stat helper constants: per tensor T in (Q, K, V) with weights W_T
    # (34, 256) and bias b_T: sum(T) = s^T W_T 1 + 596*sum(b),
    # ssq(T) = sum_k w_k^T G w_k + 2 s^T (W_T b_T) + 596*||b_T||^2.
    wsum3 = np.zeros((34, 3), f)
    wb3 = np.zeros((34, 3), f)
    c3k = np.zeros((1, 3), f)
    cs3k = np.zeros((1, 3), f)
    for i, (wn, bn) in enumerate((("qp_w", "qp_b"), ("kp_w", "kp_b"),
                                  ("vp_w", "vp_b"))):
        W = np.asarray(inp[wn], np.float64)
        b = np.asarray(inp[bn], np.float64)
        wsum3[:, i] = W.sum(axis=1).astype(f)
        wb3[:, i] = (2.0 * (W @ b)).astype(f)
        c3k[0, i] = np.float32(596.0 * float(b @ b) / LN_N)
        cs3k[0, i] = np.float32(596.0 * float(b.sum()) / LN_N)
    c["wsum3"] = wsum3
    c["wb3"] = wb3
    c["c3k"] = c3k
    c["cs3k"] = cs3k
    return c


CONST_SHAPES = {
    "w1s": (4, 64), "b1": (16, 1), "w2s": (16, 128), "b2": (32, 1),
    "coords": (2, N_PIX), "kqw2": (34, 512), "qkb2": (128, HEADS),
    "vw": (34, 256), "vbb2": (128, 512),
    "qklin": (128, N_PIX), "qkbias": (128, 10), "alin": (N_PIX, N_PIX),
    "expb": (128, 5), "lin1w": (128, 128), "bl1": (64, 1), "lin2w": (64, 10),
    "bl2": (10, 2), "ones_r": (1, 128), "ones_c": (128, 1), "epsc": (1, 1),
    "id34": (34, 34), "wsum3": (34, 3), "wb3": (34, 3), "c3k": (1, 3),
    "cs3k": (1, 3),
}


def build_nc(spb=SPB):
    """Build the Bass program (same program runs SPMD on each core)."""
    nc = bacc.Bacc("TRN2", target_bir_lowering=False, debug=False)

    x_dram = nc.dram_tensor("x", [spb, 4, 151, 6], F32, kind="ExternalInput").ap()
    out_dram = nc.dram_tensor("out", [spb, 10], F32, kind="ExternalOutput").ap()
    cdram = {
        k: nc.dram_tensor(k, list(v), F32, kind="ExternalInput").ap()
        for k, v in CONST_SHAPES.items()
    }

    with tile.TileContext(nc) as tc, ExitStack() as ctx:
        pc = ctx.enter_context(tc.tile_pool(name="consts", bufs=1))
        # SBUF pools
        px = ctx.enter_context(tc.tile_pool(name="px", bufs=2))
        ph1 = ctx.enter_context(tc.tile_pool(name="ph1", bufs=2))
        pfeat = ctx.enter_context(tc.tile_pool(name="pfeat", bufs=2))
        pqk = ctx.enter_context(tc.tile_pool(name="pqk", bufs=8))
        pqkb = ctx.enter_context(tc.tile_pool(name="pqkb", bufs=8))
        pv = ctx.enter_context(tc.tile_pool(name="pv", bufs=12))
        pat = ctx.enter_context(tc.tile_pool(name="pat", bufs=10))
        pet = ctx.enter_context(tc.tile_pool(name="pet", bufs=3))
        pext = ctx.enter_context(tc.tile_pool(name="pext", bufs=7))
        psq = ctx.enter_context(tc.tile_pool(name="psq", bufs=2))
        pst = ctx.enter_context(tc.tile_pool(name="pst", bufs=3))
        peall = ctx.enter_context(tc.tile_pool(name="peall", bufs=4))
        pfix = ctx.enter_context(tc.tile_pool(name="pfix", bufs=1))
        # PSUM pools (8 banks total: 2+2+2+2), phase-separated so sample
        # s+1's front-end never waits on sample s's tail.
        PS = bass.MemorySpace.PSUM
        ps_front = ctx.enter_context(tc.tile_pool(name="ps_front", bufs=1, space=PS))
        ps_at = ctx.enter_context(tc.tile_pool(name="ps_at", bufs=3, space=PS))
        ps_a2 = ctx.enter_context(tc.tile_pool(name="ps_a2", bufs=3, space=PS))
        ps_e = ctx.enter_context(tc.tile_pool(name="ps_e", bufs=1, space=PS))

        # ---- prefetch sample 0's input before the const DMAs ----
        x_t0 = px.tile([4, 151, 6], F32, name="x_t", tag="x")
        nc.sync.dma_start(out=x_t0[:, :, :], in_=x_dram[0])

        # ---- load constants (fp32) ----
        csb = {}
        for k, shp in CONST_SHAPES.items():
            if k == "alin":
                continue
            t = pc.tile(list(shp), F32, name=f"c_{k}")
            nc.sync.dma_start(out=t[:, :], in_=cdram[k][:, :])
            csb[k] = t
        alin_f32 = []
        for ci, (c0, c1) in enumerate(CH):
            t = pc.tile([c1 - c0, N_PIX], F32, name=f"c_alin{ci}")
            nc.sync.dma_start(out=t[:, :], in_=cdram["alin"][c0:c1, :])
            alin_f32.append(t)

        # ---- one-time bf16 conversions of matmul operands ----
        def to_bf(name, src, shp):
            t = pc.tile(list(shp), BF16, name=name)
            nc.vector.tensor_copy(t[:, :], src[:, :])
            return t

        w1s_bf = to_bf("w1s_bf", csb["w1s"], (4, 64))
        w2s_bf = to_bf("w2s_bf", csb["w2s"], (16, 128))
        coords_bf = to_bf("coords_bf", csb["coords"], (2, N_PIX))
        kqw2_bf = to_bf("kqw2_bf", csb["kqw2"], (34, 512))
        vw_bf = to_bf("vw_bf", csb["vw"], (34, 256))
        qklin_bf = to_bf("qklin_bf", csb["qklin"], (128, N_PIX))
        lin1w_bf = to_bf("lin1w_bf", csb["lin1w"], (128, 128))
        alin_bf = [to_bf(f"alin_bf{ci}", alin_f32[ci], (c1 - c0, N_PIX))
                   for ci, (c0, c1) in enumerate(CH)]
        id34_bf = to_bf("id34_bf", csb["id34"], (34, 34))
        ones_bf = pc.tile([128, 1], BF16, name="ones_bf")
        nc.vector.memset(ones_bf[:, :], 1.0)
        emax_all = pfix.tile([64, spb], F32, name="emax_all")
        emax_raw = pfix.tile([64, spb], F32, name="emax_raw")
        stats2_all = pfix.tile([1, 2 * spb], F32, name="stats2_all")

        # ================= pipelined per-sample stages =================
        W84 = 84 * HEADS
        vbb3c = csb["vbb2"].rearrange("p (h c) -> p h c", c=128)

        def front_a(s):
            """x load/cast + conv1 + conv2 + coords -> feats."""
            S = {"s": s}
            if s == 0:
                x_t = x_t0
            else:
                x_t = px.tile([4, 151, 6], F32, name="x_t", tag="x")
                nc.sync.dma_start(out=x_t[:, :, :], in_=x_dram[s])
            x_bf = px.tile([4, 151, 6], BF16, name="x_bf", tag="xbf")
            nc.gpsimd.tensor_copy(x_bf[:, :, :], x_t[:, :, :])

            h1 = ph1.tile([16, 750], BF16, name="h1", tag="h1")
            h1v = h1.rearrange("c (h w) -> c h w", w=5)
            for (r0, nr, dst0) in ((0, 102, 0), (102, 48, 510)):
                cps = ps_front.tile([16, nr * 5], F32, name="c1ps", tag="fr")
                for si, (di, dj) in enumerate(SHIFTS):
                    nc.tensor.matmul(
                        cps[:, :],
                        w1s_bf[:, si * 16:(si + 1) * 16],
                        x_bf[:, di + r0:di + r0 + nr, dj:dj + 5],
                        start=(si == 0), stop=(si == 3),
                    )
                nc.scalar.activation(h1[:, dst0:dst0 + nr * 5], cps[:, :],
                                     AF.Relu, bias=csb["b1"][:, 0:1])

            feats = pfeat.tile([34, N_PIX], BF16, name="feats", tag="feats")
            nc.gpsimd.tensor_copy(feats[32:34, :], coords_bf[:, :])
            for (r0, nr, dst0) in ((0, 128, 0), (128, 21, 512)):
                cps = ps_front.tile([32, nr * 4], F32, name="c2ps", tag="fr")
                for si, (di, dj) in enumerate(SHIFTS):
                    nc.tensor.matmul(
                        cps[:, :],
                        w2s_bf[:, si * 32:(si + 1) * 32],
                        h1v[:, di + r0:di + r0 + nr, dj:dj + 4],
                        start=(si == 0), stop=(si == 3),
                    )
                nc.scalar.activation(feats[0:32, dst0:dst0 + nr * 4], cps[:, :],
                                     AF.Relu, bias=csb["b2"][:, 0:1])
            S["feats"] = feats
            return S

        def front_b(S):
            """LN stats from s/G on the PE, then K/Q/V projections."""
            feats = S["feats"]
            # s = sum_f feats[:, f]; G = feats @ feats^T (via PE transposes)
            s_sb = pst.tile([34, 1], F32, name="s_sb", tag="s_sb")
            nc.vector.tensor_reduce(s_sb[:, :], feats[:, :],
                                    axis=mybir.AxisListType.X, op=ALU.add)
            g_ps = ps_front.tile([34, 34], F32, name="g_ps", tag="fr")
            for ci, (c0, c1) in enumerate(CH):
                csz = c1 - c0
                ft_ps = ps_a2.tile([128, 34], BF16, name="ft_ps", tag="a2")
                nc.tensor.transpose(ft_ps[0:csz, :], feats[:, c0:c1],
                                    id34_bf[:, :])
                ft_sb = pst.tile([128, 34], BF16, name="ft_sb", tag="ft")
                nc.vector.tensor_copy(ft_sb[0:csz, :], ft_ps[0:csz, :])
                nc.tensor.matmul(g_ps[:, :], ft_sb[0:csz, :],
                                 ft_sb[0:csz, :],
                                 start=(ci == 0), stop=(ci == 4))
            g_sb = pst.tile([34, 34], BF16, name="g_sb", tag="g_sb")
            nc.vector.tensor_copy(g_sb[:, :], g_ps[:, :])
            gw2_ps = ps_front.tile([34, 512], F32, name="gw2_ps", tag="fr")
            nc.tensor.matmul(gw2_ps[:, :], g_sb[:, :], kqw2_bf[:, :],
                             start=True, stop=True)
            d2 = psq.tile([34, 768], BF16, name="d2", tag="d2")
            nc.vector.tensor_tensor(d2[:, 0:512], csb["kqw2"][:, :],
                                    gw2_ps[:, :], op=ALU.mult)
            gwv_ps = ps_front.tile([34, 256], F32, name="gwv_ps", tag="fr")
            nc.tensor.matmul(gwv_ps[:, :], g_sb[:, :], vw_bf[:, :],
                             start=True, stop=True)
            nc.vector.tensor_tensor(d2[:, 512:768], csb["vw"][:, :],
                                    gwv_ps[:, :], op=ALU.mult)
            cs2_ps = ps_front.tile([1, 512], F32, name="cs2_ps", tag="fr")
            nc.tensor.matmul(cs2_ps[:, :], ones_bf[0:34, 0:1], d2[:, 0:512],
                             start=True, stop=True)
            csv_ps = ps_front.tile([1, 256], F32, name="csv_ps", tag="fr")
            nc.tensor.matmul(csv_ps[:, :], ones_bf[0:34, 0:1], d2[:, 512:768],
                             start=True, stop=True)
            # per-(h, qk) partial ssq, then fold heads
            r1 = pst.tile([1, 8], F32, name="r1", tag="r1")
            nc.vector.tensor_reduce(
                r1[:, :].rearrange("p (h t u) -> p h t u", t=2, u=1),
                cs2_ps[:, :].rearrange("p (h t d) -> p h t d", t=2, d=64),
                axis=mybir.AxisListType.X, op=ALU.add)
            ssqr = pst.tile([1, 3], F32, name="ssqr", tag="ssqr")
            nc.vector.tensor_reduce(
                ssqr[:, 0:2].rearrange("p (t u) -> p t u", u=1),
                r1[:, :].rearrange("p (h t) -> p t h", t=2),
                axis=mybir.AxisListType.X, op=ALU.add)
            nc.vector.tensor_reduce(ssqr[:, 2:3], csv_ps[:, :],
                                    axis=mybir.AxisListType.X, op=ALU.add)
            stats_ps = ps_front.tile([1, 6], F32, name="stats_ps", tag="fr")
            nc.tensor.matmul(stats_ps[0:1, 0:3], s_sb[:, :],
                             csb["wsum3"][:, :], start=True, stop=True)
            nc.tensor.matmul(stats_ps[0:1, 3:6], s_sb[:, :],
                             csb["wb3"][:, :], start=True, stop=True)
            mu3 = pst.tile([1, 3], F32, name="mu3", tag="mu3")
            nc.vector.scalar_tensor_tensor(mu3[:, :], stats_ps[0:1, 0:3],
                                           1.0 / LN_N, csb["cs3k"][0:1, :],
                                           op0=ALU.mult, op1=ALU.add)
            tsq = pst.tile([1, 3], F32, name="tsq", tag="tsq")
            nc.vector.tensor_tensor(tsq[:, :], ssqr[:, :], stats_ps[0:1, 3:6],
                                    op=ALU.add)
            msq3 = pst.tile([1, 3], F32, name="msq3", tag="msq3")
            nc.vector.scalar_tensor_tensor(msq3[:, :], tsq[:, :], 1.0 / LN_N,
                                           csb["c3k"][0:1, :],
                                           op0=ALU.mult, op1=ALU.add)
            S["mu3"] = mu3
            S["msq3"] = msq3

            # projections (plain copies; no accumulation needed)
            stacked = []
            for h in range(HEADS):
                st_t = pqk.tile([128, N_PIX], BF16, name="st_t", tag="qk")
                stacked.append(st_t)
                pps = ps_front.tile([128, 512], F32, name="pps", tag="fr")
                pps2 = ps_front.tile([128, 84], F32, name="pps2", tag="fr")
                nc.tensor.matmul(pps[:, :], kqw2_bf[:, h * 128:(h + 1) * 128],
                                 feats[:, 0:512], start=True, stop=True)
                nc.tensor.matmul(pps2[:, :], kqw2_bf[:, h * 128:(h + 1) * 128],
                                 feats[:, 512:596], start=True, stop=True)
                nc.vector.tensor_scalar_add(st_t[:, 0:512], pps[:, :],
                                            csb["qkb2"][:, h:h + 1])
                nc.vector.tensor_scalar_add(st_t[:, 512:596], pps2[:, :],
                                            csb["qkb2"][:, h:h + 1])

            vtiles = []
            for ci, (c0, c1) in enumerate(CH):
                csz = c1 - c0
                vps = ps_front.tile([128, 256], F32, name="vps", tag="fr")
                nc.tensor.matmul(vps[0:csz, :], feats[:, c0:c1],
                                 vw_bf[:, :], start=True, stop=True)
                vt = pv.tile([128, 512], BF16, name="vt", tag="v")
                vt3 = vt.rearrange("p (h c) -> p h c", c=128)
                vps3 = vps.rearrange("p (h c) -> p h c", c=64)
                nc.vector.memset(vt3[0:csz, :, 64:128], 1.0)
                nc.vector.scalar_tensor_tensor(
                    vt3[0:csz, :, 0:64], vps3[0:csz, :, :], 1.0,
                    vbb3c[0:csz, :, 0:64],
                    op0=ALU.mult, op1=ALU.add)
                vtiles.append(vt)
            S["stacked"] = stacked
            S["vtiles"] = vtiles
            return S

        def front_c(S):
            """LN scalar pipeline + LN apply (fp32 -> bf16)."""
            mu3, msq3 = S["mu3"], S["msq3"]
            nmu2 = pst.tile([1, 3], F32, name="nmu2", tag="nmu2")
            nc.vector.scalar_tensor_tensor(nmu2[:, :], mu3[:, :], -1.0,
                                           mu3[:, :],
                                           op0=ALU.mult, op1=ALU.mult)
            var3 = pst.tile([1, 3], F32, name="var3", tag="var3")
            nc.vector.tensor_tensor(var3[:, :], msq3[:, :], nmu2[:, :],
                                    op=ALU.add)
            std3 = pst.tile([1, 3], F32, name="std3", tag="std3")
            nc.scalar.activation(std3[:, :], var3[:, :], AF.Sqrt,
                                 bias=csb["epsc"][0:1, 0:1])
            rsnmr = pst.tile([1, 6], F32, name="rsnmr", tag="rsnmr")
            rsv = rsnmr.rearrange("p (a b) -> p a b", b=2)
            nc.vector.reciprocal(rsv[:, :, 0:1], std3[:, :])
            nc.vector.scalar_tensor_tensor(rsv[:, :, 1:2], mu3[:, :], -1.0,
                                           rsv[:, :, 0:1],
                                           op0=ALU.mult, op1=ALU.mult)
            bc_ps = ps_a2.tile([128, 6], F32, name="bc_ps", tag="a2")
            nc.tensor.matmul(bc_ps[:, :], csb["ones_r"][0:1, :], rsnmr[:, :],
                             start=True, stop=True)
            bc = pst.tile([128, 6], F32, name="bc", tag="bc")
            nc.vector.tensor_copy(bc[:, :], bc_ps[:, :])
            # bc cols: [rsQ, nmrQ, rsK, nmrK, rsV, nmrV]

            stacked_bf = []
            for h in range(HEADS):
                sb = pqkb.tile([128, N_PIX], BF16, name="st_bf", tag="qkb")
                stacked_bf.append(sb)
                nc.vector.tensor_scalar(sb[0:64, :], S["stacked"][h][0:64, :],
                                        bc[0:64, 0:1], bc[0:64, 1:2],
                                        op0=ALU.mult, op1=ALU.add)
                nc.vector.tensor_scalar(sb[64:128, :],
                                        S["stacked"][h][64:128, :],
                                        bc[0:64, 2:3], bc[0:64, 3:4],
                                        op0=ALU.mult, op1=ALU.add)
            for ci, (c0, c1) in enumerate(CH):
                csz = c1 - c0
                vt3 = S["vtiles"][ci].rearrange("p (h c) -> p h c", c=128)
                nc.vector.tensor_scalar(vt3[0:csz, :, 0:64],
                                        vt3[0:csz, :, 0:64],
                                        bc[0:csz, 4:5], bc[0:csz, 5:6],
                                        op0=ALU.mult, op1=ALU.add)
            S["stacked_bf"] = stacked_bf
            S["eall"] = [peall.tile([128, N_PIX], BF16, name=f"eall{i}",
                                    tag="eall") for i in range(2)]
            return S

        # ---- attention stages (pipeline carried across samples) ----
        def at_chunk(S, p, ci, dest):
            c0, c1 = CH[ci]
            csz = c1 - c0
            w = 512 if not p["merged"] else W84
            aps = ps_at.tile([128, 512], F32, name="aps", tag="at")
            if p["merged"]:
                for h in range(HEADS):
                    nc.tensor.matmul(aps[0:csz, h * 84:(h + 1) * 84],
                                     qklin_bf[:, c0:c1],
                                     S["stacked_bf"][h][:, 512:596],
                                     start=True, stop=True)
            else:
                nc.tensor.matmul(aps[0:csz, 0:512],
                                 qklin_bf[:, c0:c1],
                                 S["stacked_bf"][p["h"]][:, 0:512],
                                 start=True, stop=True)
            et = pet.tile([128, 512], F32, name="et", tag="et")
            nc.scalar.activation(et[0:csz, 0:w], aps[0:csz, 0:w],
                                 AF.Exp,
                                 bias=csb["qkbias"][0:csz, ci:ci + 1])
            nc.gpsimd.tensor_scalar_min(et[0:csz, 0:w],
                                        et[0:csz, 0:w], 1.0)
            nc.vector.scalar_tensor_tensor(
                dest[0:csz, 0:w], aps[0:csz, 0:w],
                csb["qkbias"][0:csz, 5 + ci:6 + ci],
                et[0:csz, 0:w], op0=ALU.add, op1=ALU.max)

        def e_c2(st, c2i):
            S, p, tiles = st["S"], st["p"], st["tiles"]
            c20, c21 = CH[c2i]
            c2sz = c21 - c20
            w = 512 if not p["merged"] else W84
            if c2i == 0:
                st["eps"] = ps_e.tile([128, 512], F32, name="eps_t", tag="e")
            eps_t = st["eps"]
            a2ps = ps_a2.tile([128, 512], F32, name="a2ps", tag="a2")
            for ci, (c0, c1) in enumerate(CH):
                csz = c1 - c0
                nc.tensor.matmul(a2ps[0:c2sz, 0:w],
                                 alin_bf[ci][:, c20:c21],
                                 tiles[ci][0:csz, 0:w],
                                 start=(ci == 0), stop=(ci == 4))
            ext = pext.tile([128, 512], BF16, name="ext", tag="ext")
            nc.scalar.activation(ext[0:c2sz, 0:w],
                                 a2ps[0:c2sz, 0:w], AF.Exp,
                                 bias=csb["expb"][0:c2sz, c2i:c2i + 1])
            if p["merged"]:
                # PSUM accumulation groups must not interleave within a
                # bank's 2KB zero region: buffer the ext tiles and run the
                # four per-head accumulations sequentially in e_tail.
                st.setdefault("exts", []).append(ext)
            else:
                nc.tensor.matmul(eps_t[:, 0:512],
                                 S["vtiles"][c2i][0:c2sz,
                                                  p["h"] * 128:
                                                  (p["h"] + 1) * 128],
                                 ext[0:c2sz, 0:512],
                                 start=(c2i == 0), stop=(c2i == 4))

        def e_tail(st):
            S, p, eps_t = st["S"], st["p"], st["eps"]
            w = 512 if not p["merged"] else W84
            eall = S["eall"]
            if p["merged"]:
                for h in range(HEADS):
                    for c2i, (c20, c21) in enumerate(CH):
                        c2sz = c21 - c20
                        nc.tensor.matmul(
                            eps_t[:, h * 84:(h + 1) * 84],
                            S["vtiles"][c2i][0:c2sz, h * 128:(h + 1) * 128],
                            st["exts"][c2i][0:c2sz, h * 84:(h + 1) * 84],
                            start=(c2i == 0), stop=(c2i == 4))
            recip64 = pst.tile([64, 512], F32, name="recip64", tag="recip")
            nc.vector.reciprocal(recip64[:, 0:w], eps_t[64:128, 0:w])
            if p["merged"]:
                for h in range(HEADS):
                    nc.vector.tensor_tensor(
                        eall[h // 2][(h % 2) * 64:(h % 2) * 64 + 64, 512:596],
                        eps_t[0:64, h * 84:(h + 1) * 84],
                        recip64[:, h * 84:(h + 1) * 84], op=ALU.mult)
            else:
                h = p["h"]
                nc.vector.tensor_tensor(
                    eall[h // 2][(h % 2) * 64:(h % 2) * 64 + 64, 0:512],
                    eps_t[0:64, 0:512], recip64[:, 0:512], op=ALU.mult)

        pending = [None]

        def do_pass(S, p):
            tiles = [pat.tile([128, 512], BF16, name=f"att{i}", tag="atile")
                     for i in range(5)]
            for ci in range(4):
                at_chunk(S, p, ci, tiles[ci][:, :])
            prev = pending[0]
            if prev is None:
                at_chunk(S, p, 4, tiles[4][:, :])
            else:
                e_c2(prev, 0)
                e_c2(prev, 1)
                e_c2(prev, 2)
                at_chunk(S, p, 4, tiles[4][:, :])
                e_c2(prev, 3)
                e_c2(prev, 4)
                e_tail(prev)
            pending[0] = {"S": S, "p": p, "tiles": tiles}

        def flush_pipe():
            prev = pending[0]
            for c2i in range(5):
                e_c2(prev, c2i)
            e_tail(prev)
            pending[0] = None

        def tail(S):
            """lin1 + LN2 raw stats (scalar pipeline batched at the end)."""
            s, eall = S["s"], S["eall"]
            e2 = psq.tile([64, N_PIX], F32, name="e2", tag="e2")
            ls2 = pst.tile([64, 2], F32, name="ls2", tag="ls2")
            lpart = pst.tile([64, 2], F32, name="lpart", tag="lpart")
            for (f0, f1) in FH:
                fsz = f1 - f0
                lps = ps_e.tile([64, 512], F32, name="lps", tag="e")
                for ck in range(2):
                    nc.tensor.matmul(lps[:, 0:fsz],
                                     lin1w_bf[:, ck * 64:(ck + 1) * 64],
                                     eall[ck][:, f0:f1],
                                     start=(ck == 0), stop=(ck == 1))
                nc.scalar.activation(e2[:, f0:f1], lps[:, 0:fsz], AF.Relu,
                                     bias=csb["bl1"][:, 0:1],
                                     accum_out=lpart[:, (0 if f0 == 0 else 1):
                                                     (1 if f0 == 0 else 2)])
            nc.vector.tensor_reduce(ls2[:, 0:1], lpart[:, :],
                                    axis=mybir.AxisListType.X, op=ALU.add)
            sqe = psq.tile([64, N_PIX], F32, name="sqe", tag="sqe")
            nc.scalar.activation(sqe[:, :], e2[:, :], AF.Square,
                                 accum_out=ls2[:, 1:2])
            nc.vector.tensor_reduce(emax_raw[:, s:s + 1], e2[:, :],
                                    axis=mybir.AxisListType.X, op=ALU.max)
            st2 = ps_at.tile([1, 2], F32, name="st2", tag="at")
            nc.tensor.matmul(st2[0:1, :], csb["ones_c"][0:64, 0:1], ls2[:, :],
                             start=True, stop=True)
            nc.vector.tensor_copy(stats2_all[:, 2 * s:2 * s + 2], st2[0:1, :])

        # ---- pipelined schedule: sample s+1's front-end is emitted between
        # sample s's attention passes; the at/e pass pipeline is carried
        # across the sample boundary.
        S = front_a(0)
        front_b(S)
        front_c(S)
        states = {0: S}
        for s in range(spb):
            S = states[s]
            plist = ([dict(h=h, merged=False) for h in range(HEADS)]
                     + [dict(h=None, merged=True)])
            do_pass(S, plist[0])
            if s > 0:
                tail(states.pop(s - 1))
            do_pass(S, plist[1])
            if s + 1 < spb:
                Sn = front_a(s + 1)
            do_pass(S, plist[2])
            if s + 1 < spb:
                front_b(Sn)
            do_pass(S, plist[3])
            if s + 1 < spb:
                front_c(Sn)
                states[s + 1] = Sn
            do_pass(S, plist[4])
        flush_pipe()
        tail(states.pop(spb - 1))

        # ---------------- batched LN2 scalar pipeline (all samples) --------
        m2a = pst.tile([1, 2 * spb], F32, name="m2a", tag="m2a")
        m2av = m2a.rearrange("p (a b) -> p a b", b=2)
        nc.vector.tensor_scalar_mul(m2a[:, :], stats2_all[:, :], 1.0 / LN2_N)
        nmu2a = pst.tile([1, spb], F32, name="nmu2a", tag="nmu2a")
        nc.vector.scalar_tensor_tensor(nmu2a[:, :],
                                       m2av[:, :, 0:1], -1.0, m2av[:, :, 0:1],
                                       op0=ALU.mult, op1=ALU.mult)
        var2a = pst.tile([1, spb], F32, name="var2a", tag="var2a")
        nc.vector.tensor_tensor(var2a[:, :], m2av[:, :, 1:2], nmu2a[:, :],
                                op=ALU.add)
        std2a = pst.tile([1, spb], F32, name="std2a", tag="std2a")
        nc.scalar.activation(std2a[:, :], var2a[:, :], AF.Sqrt,
                             bias=csb["epsc"][0:1, 0:1])
        rs2a = pst.tile([1, 2 * spb], F32, name="rs2a", tag="rs2a")
        rs2av = rs2a.rearrange("p (a b) -> p a b", b=2)
        nc.vector.reciprocal(rs2av[:, :, 0:1], std2a[:, :])
        nc.vector.scalar_tensor_tensor(rs2av[:, :, 1:2],
                                       m2av[:, :, 0:1], -1.0,
                                       rs2av[:, :, 0:1],
                                       op0=ALU.mult, op1=ALU.mult)
        bc2p = ps_at.tile([64, 2 * spb], F32, name="bc2p", tag="at")
        nc.tensor.matmul(bc2p[:, :], csb["ones_r"][0:1, 0:64], rs2a[:, :],
                         start=True, stop=True)
        bc2 = pst.tile([64, 2 * spb], F32, name="bc2", tag="bc2")
        nc.vector.tensor_copy(bc2[:, :], bc2p[:, :])
        for s in range(spb):
            nc.vector.tensor_scalar(emax_all[:, s:s + 1], emax_raw[:, s:s + 1],
                                    bc2[:, 2 * s:2 * s + 1],
                                    bc2[:, 2 * s + 1:2 * s + 2],
                                    op0=ALU.mult, op1=ALU.add)

        # ---------------- lin2 + final elu ----------------
        l2ps = ps_at.tile([10, spb], F32, name="l2ps", tag="at")
        nc.tensor.matmul(l2ps[:, :], csb["lin2w"][:, :], emax_all[:, :],
                         start=True, stop=True)
        fe = pst.tile([10, spb], F32, name="fe", tag="fe")
        nc.scalar.activation(fe[:, :], l2ps[:, :], AF.Exp,
                             bias=csb["bl2"][:, 0:1])
        nc.vector.tensor_scalar(fe[:, :], fe[:, :], 1.0, -1.0,
                                op0=ALU.min, op1=ALU.add)
        out_sb = pst.tile([10, spb], F32, name="out_sb", tag="out_sb")
        nc.vector.scalar_tensor_tensor(out_sb[:, :], l2ps[:, :],
                                       csb["bl2"][:, 0:1], fe[:, :],
                                       op0=ALU.add, op1=ALU.max)
        nc.sync.dma_start(out=out_dram.rearrange("s t -> t s"), in_=out_sb[:, :])

    return nc


def _reference_numpy(inp):
    """Pure-numpy fallback (only used if LN affine params are nontrivial)."""
    def ln(x, g=None, b=None):
        axes = tuple(range(1, x.ndim))
        mu = x.mean(axis=axes, keepdims=True)
        var = x.var(axis=axes, keepdims=True)
        y = (x - mu) / np.sqrt(var + EPS)
        return y * g + b if g is not None else y

    def elu(x):
        return np.where(x > 0, x, np.expm1(np.minimum(x, 0)))

    x = np.asarray(inp["x"], np.float64)
    N = x.shape[0]
    w1, b1 = np.asarray(inp["conv1_w"], np.float64), np.asarray(inp["conv1_b"], np.float64)
    h = np.zeros((N, 16, 150, 5))
    for di in range(2):
        for dj in range(2):
            h += np.einsum("oc,nchw->nohw", w1[:, :, di, dj],
                           x[:, :, di:di + 150, dj:dj + 5])
    h = np.maximum(h + b1[None, :, None, None], 0)
    w2, b2 = np.asarray(inp["conv2_w"], np.float64), np.asarray(inp["conv2_b"], np.float64)
    h2 = np.zeros((N, 32, 149, 4))
    for di in range(2):
        for dj in range(2):
            h2 += np.einsum("oc,nchw->nohw", w2[:, :, di, dj],
                            h[:, :, di:di + 149, dj:dj + 4])
    h2 = np.maximum(h2 + b2[None, :, None, None], 0)
    p = np.arange(N_PIX)
    xc, yc = (p % 4) / 4.0, (p // 4) / 149.0
    feats = np.concatenate(
        [h2.transpose(0, 2, 3, 1).reshape(N, N_PIX, 32),
         np.broadcast_to(np.stack([xc, yc], 1)[None], (N, N_PIX, 2))], axis=2)

    def proj(wn, bn, gn, bn2):
        P = (feats @ np.asarray(inp[wn], np.float64) + np.asarray(inp[bn], np.float64))
        P = P.reshape(N, N_PIX, HEADS, D).transpose(0, 2, 1, 3)
        return ln(P, np.asarray(inp[gn], np.float64), np.asarray(inp[bn2], np.float64))

    K = proj("kp_w", "kp_b", "knorm_g", "knorm_b")
    Q = proj("qp_w", "qp_b", "qnorm_g", "qnorm_b")
    V = proj("vp_w", "vp_b", "vnorm_g", "vnorm_b")
    A = elu(Q @ np.asarray(inp["qlin_w"], np.float64) + np.asarray(inp["qlin_b"], np.float64)
            + K @ np.asarray(inp["klin_w"], np.float64) + np.asarray(inp["klin_b"], np.float64))
    A = A @ np.asarray(inp["alin_w"], np.float64) + np.asarray(inp["alin_b"], np.float64)
    A = A - A.max(axis=-1, keepdims=True)
    A = np.exp(A)
    A = A / A.sum(axis=-1, keepdims=True)
    E = np.einsum("bhfc,bhcd->bhfd", A, V)
    E = E.transpose(0, 2, 1, 3).reshape(N, N_PIX, HEADS * D)
    E = np.maximum(E @ np.asarray(inp["lin1_w"], np.float64)
                   + np.asarray(inp["lin1_b"], np.float64), 0)
    E = ln(E)
    E = E.max(axis=1)
    out = E @ np.asarray(inp["lin2_w"], np.float64) + np.asarray(inp["lin2_b"], np.float64)
    return elu(out).astype(np.float32)


def kernel(**inputs):
    trivial = (np.all(np.asarray(inputs["knorm_g"]) == 1.0)
               and np.all(np.asarray(inputs["knorm_b"]) == 0.0)
               and np.all(np.asarray(inputs["qnorm_g"]) == 1.0)
               and np.all(np.asarray(inputs["qnorm_b"]) == 0.0)
               and np.all(np.asarray(inputs["vnorm_g"]) == 1.0)
               and np.all(np.asarray(inputs["vnorm_b"]) == 0.0))
    if not trivial:
        return _reference_numpy(inputs)

    x = np.ascontiguousarray(np.asarray(inputs["x"], np.float32))
    n = x.shape[0]
    assert n == N_CORES * SPB, f"expected batch {N_CORES * SPB}, got {n}"
    consts = _prep_consts(inputs)

    if "nc" not in _cache:
        nc = build_nc(SPB)
        nc.compile()
        _cache["nc"] = nc
    nc = _cache["nc"]

    in_maps = []
    for c in range(N_CORES):
        m = dict(consts)
        m["x"] = np.ascontiguousarray(x[c * SPB:(c + 1) * SPB])
        in_maps.append(m)

    import os
    trace = bool(int(os.environ.get("KERNEL_TRACE", "0")))
    res = run_bass_kernel_spmd(nc, in_maps, list(range(N_CORES)), trace=trace)
    kernel._last_results = res
    out = np.concatenate([np.asarray(r["out"]) for r in res.results], axis=0)
    return out.astype(np.float32)


kernel._last_results = None


# revision 24
# speedup vs baseline: 1.0847x; 1.0709x over previous
"""Fused Trainium2 kernel for nn_MultiHeadRelationalModule.

Data-parallel over 8 NeuronCores (8 samples each). The whole per-sample
pipeline (conv1 -> conv2 -> +coords -> K/Q/V proj -> LayerNorm ->
relational attention (4 heads, 596x596) -> softmax -> weighted sum ->
lin1 -> LN -> maxpool -> lin2 -> elu) runs on-chip; the big attention
maps never touch HBM.

v2: all large matmuls run in bf16 (4x faster per PE row than fp32 on
TRN2; fp32 needs 4 cycles/row, bf16 needs 1). PSUM accumulation stays
fp32. Q+K projections merged into one 128-partition matmul per head;
V projections merged across heads. Elementwise work balanced across
Act/DVE/Pool engines.

Key identities used:
  elu(x) + 1 == max(x + 1, min(exp(x), 1))        (exact)
  A' = elu(z)+1 fed to matmul with alin_w: subtract colsum(alin_w) in the
       following bias to undo the +1 (softmax bias becomes
       alin_b - alin_w.sum(0)).
  softmax over c2 with A2^T layout (c2 on partitions): exp on chip,
       denominator via an appended ones-column on V in the E matmul.
  LN(x) = (x - mu) * rsqrt(var + eps); affine params in this model are
       identity (ones/zeros), verified at runtime.
  max-pool commutes with the final LN (monotone affine map).
"""

import numpy as np
from contextlib import ExitStack

import concourse.bacc as bacc
import concourse.bass as bass
import concourse.mybir as mybir
import concourse.tile as tile
from concourse.bass_utils import run_bass_kernel_spmd

F32 = mybir.dt.float32
BF16 = mybir.dt.bfloat16
FP8 = mybir.dt.float8e4
ALSC = 16.0  # alin pre-scale into fp8e4m3 normal range; undone in exp2 scale
AF = mybir.ActivationFunctionType
ALU = mybir.AluOpType

N_CORES = 8
SPB = 8               # samples per core
N_PIX = 596
HEADS = 4
D = 64
CH = [(0, 128), (128, 256), (256, 384), (384, 512), (512, 596)]
FH = [(0, 512), (512, 596)]
SHIFTS = [(0, 0), (0, 1), (1, 0), (1, 1)]
LN_N = float(HEADS * N_PIX * D)       # 152576
LN2_N = float(N_PIX * D)              # 38144
EPS = 1e-5

_cache = {}


def _prep_consts(inp):
    """Host-side preprocessing of weights into kernel-friendly layouts."""
    f = np.float32
    c = {}
    conv1_w = np.asarray(inp["conv1_w"], f)
    c["w1s"] = np.ascontiguousarray(
        np.concatenate([conv1_w[:, :, di, dj].T for (di, dj) in SHIFTS], axis=1)
    )  # (4, 64)
    c["b1"] = np.ascontiguousarray(np.asarray(inp["conv1_b"], f)[:, None])  # (16,1)
    conv2_w = np.asarray(inp["conv2_w"], f)
    c["w2s"] = np.ascontiguousarray(
        np.concatenate([conv2_w[:, :, di, dj].T for (di, dj) in SHIFTS], axis=1)
    )  # (16, 128)
    c["b2"] = np.ascontiguousarray(np.asarray(inp["conv2_b"], f)[:, None])  # (32,1)

    p = np.arange(N_PIX)
    c["coords"] = np.ascontiguousarray(
        np.stack([(p % 4) / 4.0, (p // 4) / 149.0]).astype(f)
    )  # (2, 596)

    # Q/K projection merged per head: cols h*128:h*128+64 = Q (stacked rows
    # 0:64), cols h*128+64:h*128+128 = K (stacked rows 64:128).
    qp_w = np.asarray(inp["qp_w"], f)
    kp_w = np.asarray(inp["kp_w"], f)
    kqw2 = np.zeros((34, 512), f)
    qkb2 = np.zeros((128, HEADS), f)
    for h in range(HEADS):
        kqw2[:, h * 128:h * 128 + 64] = qp_w[:, h * 64:(h + 1) * 64]
        kqw2[:, h * 128 + 64:h * 128 + 128] = kp_w[:, h * 64:(h + 1) * 64]
        qkb2[0:64, h] = np.asarray(inp["qp_b"], f)[h * 64:(h + 1) * 64]
        qkb2[64:128, h] = np.asarray(inp["kp_b"], f)[h * 64:(h + 1) * 64]
    c["kqw2"] = kqw2
    c["qkb2"] = qkb2

    c["vw"] = np.ascontiguousarray(np.asarray(inp["vp_w"], f))  # (34, 256)
    vbb2 = np.zeros((128, 512), f)   # per head: [V bias (64) | 0 (ones blk)]
    for h in range(HEADS):
        vbb2[:, h * 128:h * 128 + 64] = np.asarray(inp["vp_b"], f)[None,
                                                                   h * 64:(h + 1) * 64]
    c["vbb2"] = vbb2

    c["qklin"] = np.ascontiguousarray(
        np.concatenate([np.asarray(inp["qlin_w"], f),
                        np.asarray(inp["klin_w"], f)], axis=0)
    )  # (128, 596): rows 0:64 qlin (Q), 64:128 klin (K)

    qkbias = np.zeros((128, 10), f)
    qkl_b = np.asarray(inp["qlin_b"], f) + np.asarray(inp["klin_b"], f)
    for ci, (c0, c1) in enumerate(CH):
        qkbias[0:c1 - c0, ci] = qkl_b[c0:c1]
        qkbias[0:c1 - c0, 5 + ci] = qkl_b[c0:c1] + 1.0
    c["qkbias"] = qkbias

    c["alin"] = np.ascontiguousarray(np.asarray(inp["alin_w"], f))  # (596, 596)
    # fp8e4m3 DoubleRowSwInterleave weight pairs for alin rows 0:512 (x16 so
    # the ~0.05-scale entries sit in e4m3's normal range; undone in exp2's
    # scale).  Per matmul slice: cols [A[m], B[m]] pairs, m descending.
    import ml_dtypes
    alin16 = np.pad(np.asarray(inp["alin_w"], f) * ALSC, ((0, 0), (0, 44)))
    for j in range(2):
        A = alin16[256 * j:256 * j + 128]
        B = alin16[256 * j + 128:256 * j + 256]
        buf = np.zeros((128, 1280), f)
        for ci in range(5):
            c0 = 128 * ci
            blk = np.empty((128, 256), f)
            blk[:, 0::2] = A[:, c0:c0 + 128][:, ::-1]
            blk[:, 1::2] = B[:, c0:c0 + 128][:, ::-1]
            buf[:, 2 * c0:2 * c0 + 256] = blk
        c[f"alin_i8_{j}"] = np.ascontiguousarray(
            buf.astype(ml_dtypes.float8_e4m3))

    expb = np.zeros((128, 5), f)
    eb = np.asarray(inp["alin_b"], f) - np.asarray(inp["alin_w"], f).sum(axis=0)
    for ci, (c0, c1) in enumerate(CH):
        expb[0:c1 - c0, ci] = eb[c0:c1]
    c["expb"] = expb

    l1 = np.zeros((128, 128), f)
    lin1_w = np.asarray(inp["lin1_w"], f)
    l1[:, 0:64] = lin1_w[0:128]
    l1[:, 64:128] = lin1_w[128:256]
    c["lin1w"] = l1
    c["bl1"] = np.ascontiguousarray(np.asarray(inp["lin1_b"], f)[:, None])  # (64,1)
    c["lin2w"] = np.ascontiguousarray(np.asarray(inp["lin2_w"], f))  # (64,10)
    bl2 = np.zeros((10, 2), f)
    bl2[:, 0] = np.asarray(inp["lin2_b"], f)
    bl2[:, 1] = np.asarray(inp["lin2_b"], f) + 1.0
    c["bl2"] = bl2
    c["ones_r"] = np.ones((1, 128), f)
    c["ones_c"] = np.ones((128, 1), f)
    c["epsc"] = np.full((1, 1), EPS, f)
    c["id34"] = np.eye(34, dtype=f)
    # LN-stat helper constants: per tensor T in (Q, K, V) with weights W_T
    # (34, 256) and bias b_T: sum(T) = s^T W_T 1 + 596*sum(b),
    # ssq(T) = sum_k w_k^T G w_k + 2 s^T (W_T b_T) + 596*||b_T||^2.
    wsum3 = np.zeros((34, 3), f)
    wb3 = np.zeros((34, 3), f)
    c3k = np.zeros((1, 3), f)
    cs3k = np.zeros((1, 3), f)
    for i, (wn, bn) in enumerate((("qp_w", "qp_b"), ("kp_w", "kp_b"),
                                  ("vp_w", "vp_b"))):
        W = np.asarray(inp[wn], np.float64)
        b = np.asarray(inp[bn], np.float64)
        wsum3[:, i] = W.sum(axis=1).astype(f)
        wb3[:, i] = (2.0 * (W @ b)).astype(f)
        c3k[0, i] = np.float32(596.0 * float(b @ b) / LN_N)
        cs3k[0, i] = np.float32(596.0 * float(b.sum()) / LN_N)
    c["wsum3"] = wsum3
    c["wb3"] = wb3
    c["c3k"] = c3k
    c["cs3k"] = cs3k
    return c


CONST_SHAPES = {
    "w1s": (4, 64), "b1": (16, 1), "w2s": (16, 128), "b2": (32, 1),
    "coords": (2, N_PIX), "kqw2": (34, 512), "qkb2": (128, HEADS),
    "vw": (34, 256), "vbb2": (128, 512),
    "qklin": (128, N_PIX), "qkbias": (128, 10), "alin": (N_PIX, N_PIX),
    "expb": (128, 5), "lin1w": (128, 128), "bl1": (64, 1), "lin2w": (64, 10),
    "bl2": (10, 2), "ones_r": (1, 128), "ones_c": (128, 1), "epsc": (1, 1),
    "id34": (34, 34), "wsum3": (34, 3), "wb3": (34, 3), "c3k": (1, 3),
    "cs3k": (1, 3),
}
CONST_FP8 = {"alin_i8_0": (128, 1280), "alin_i8_1": (128, 1280)}


def build_nc(spb=SPB):
    """Build the Bass program (same program runs SPMD on each core)."""
    nc = bacc.Bacc("TRN2", target_bir_lowering=False, debug=False)

    x_dram = nc.dram_tensor("x", [spb, 4, 151, 6], F32, kind="ExternalInput").ap()
    out_dram = nc.dram_tensor("out", [spb, 10], F32, kind="ExternalOutput").ap()
    cdram = {
        k: nc.dram_tensor(k, list(v), F32, kind="ExternalInput").ap()
        for k, v in CONST_SHAPES.items()
    }
    for k, v in CONST_FP8.items():
        cdram[k] = nc.dram_tensor(k, list(v), FP8, kind="ExternalInput").ap()

    with tile.TileContext(nc) as tc, ExitStack() as ctx:
        pc = ctx.enter_context(tc.tile_pool(name="consts", bufs=1))
        # SBUF pools
        px = ctx.enter_context(tc.tile_pool(name="px", bufs=2))
        ph1 = ctx.enter_context(tc.tile_pool(name="ph1", bufs=2))
        pfeat = ctx.enter_context(tc.tile_pool(name="pfeat", bufs=2))
        pqk = ctx.enter_context(tc.tile_pool(name="pqk", bufs=8))
        pqkb = ctx.enter_context(tc.tile_pool(name="pqkb", bufs=8))
        pv = ctx.enter_context(tc.tile_pool(name="pv", bufs=12))
        pat = ctx.enter_context(tc.tile_pool(name="pat", bufs=10))
        pet = ctx.enter_context(tc.tile_pool(name="pet", bufs=3))
        pext = ctx.enter_context(tc.tile_pool(name="pext", bufs=7))
        psq = ctx.enter_context(tc.tile_pool(name="psq", bufs=2))
        pst = ctx.enter_context(tc.tile_pool(name="pst", bufs=3))
        peall = ctx.enter_context(tc.tile_pool(name="peall", bufs=4))
        pfix = ctx.enter_context(tc.tile_pool(name="pfix", bufs=1))
        # PSUM pools (8 banks total: 2+2+2+2), phase-separated so sample
        # s+1's front-end never waits on sample s's tail.
        PS = bass.MemorySpace.PSUM
        ps_front = ctx.enter_context(tc.tile_pool(name="ps_front", bufs=1, space=PS))
        ps_at = ctx.enter_context(tc.tile_pool(name="ps_at", bufs=3, space=PS))
        ps_a2 = ctx.enter_context(tc.tile_pool(name="ps_a2", bufs=3, space=PS))
        ps_e = ctx.enter_context(tc.tile_pool(name="ps_e", bufs=1, space=PS))

        # ---- prefetch sample 0's input before the const DMAs ----
        x_t0 = px.tile([4, 151, 6], F32, name="x_t", tag="x")
        nc.sync.dma_start(out=x_t0[:, :, :], in_=x_dram[0])

        # ---- load constants (fp32) ----
        csb = {}
        for k, shp in CONST_SHAPES.items():
            if k == "alin":
                continue
            t = pc.tile(list(shp), F32, name=f"c_{k}")
            nc.sync.dma_start(out=t[:, :], in_=cdram[k][:, :])
            csb[k] = t
        alin4_f32 = pc.tile([84, N_PIX], F32, name="c_alin4")
        nc.sync.dma_start(out=alin4_f32[:, :], in_=cdram["alin"][512:596, :])
        alin_i8 = []
        for j in range(2):
            t = pc.tile([128, 1280], FP8, name=f"alin_i8_{j}")
            nc.sync.dma_start(out=t[:, :], in_=cdram[f"alin_i8_{j}"][:, :])
            alin_i8.append(t)

        # ---- one-time bf16 conversions of matmul operands ----
        def to_bf(name, src, shp):
            t = pc.tile(list(shp), BF16, name=name)
            nc.vector.tensor_copy(t[:, :], src[:, :])
            return t

        w1s_bf = to_bf("w1s_bf", csb["w1s"], (4, 64))
        w2s_bf = to_bf("w2s_bf", csb["w2s"], (16, 128))
        coords_bf = to_bf("coords_bf", csb["coords"], (2, N_PIX))
        kqw2_bf = to_bf("kqw2_bf", csb["kqw2"], (34, 512))
        vw_bf = to_bf("vw_bf", csb["vw"], (34, 256))
        qklin_bf = to_bf("qklin_bf", csb["qklin"], (128, N_PIX))
        lin1w_bf = to_bf("lin1w_bf", csb["lin1w"], (128, 128))
        alin_bf4 = pc.tile([84, 640], BF16, name="alin_bf4")
        nc.vector.memset(alin_bf4[:, 596:640], 0.0)
        nc.vector.tensor_scalar_mul(alin_bf4[:, 0:N_PIX], alin4_f32[:, :],
                                    ALSC)
        id34_bf = to_bf("id34_bf", csb["id34"], (34, 34))
        ones_bf = pc.tile([128, 1], BF16, name="ones_bf")
        nc.vector.memset(ones_bf[:, :], 1.0)
        emax_all = pfix.tile([64, spb], F32, name="emax_all")
        emax_raw = pfix.tile([64, spb], F32, name="emax_raw")
        stats2_all = pfix.tile([1, 2 * spb], F32, name="stats2_all")

        # ================= pipelined per-sample stages =================
        W84 = 84 * HEADS
        vbb3c = csb["vbb2"].rearrange("p (h c) -> p h c", c=128)

        def front_a(s):
            """x load/cast + conv1 + conv2 + coords -> feats."""
            S = {"s": s}
            if s == 0:
                x_t = x_t0
            else:
                x_t = px.tile([4, 151, 6], F32, name="x_t", tag="x")
                nc.sync.dma_start(out=x_t[:, :, :], in_=x_dram[s])
            x_bf = px.tile([4, 151, 6], BF16, name="x_bf", tag="xbf")
            nc.gpsimd.tensor_copy(x_bf[:, :, :], x_t[:, :, :])

            h1 = ph1.tile([16, 750], BF16, name="h1", tag="h1")
            h1v = h1.rearrange("c (h w) -> c h w", w=5)
            for (r0, nr, dst0) in ((0, 102, 0), (102, 48, 510)):
                cps = ps_front.tile([16, nr * 5], F32, name="c1ps", tag="fr")
                for si, (di, dj) in enumerate(SHIFTS):
                    nc.tensor.matmul(
                        cps[:, :],
                        w1s_bf[:, si * 16:(si + 1) * 16],
                        x_bf[:, di + r0:di + r0 + nr, dj:dj + 5],
                        start=(si == 0), stop=(si == 3),
                    )
                nc.scalar.activation(h1[:, dst0:dst0 + nr * 5], cps[:, :],
                                     AF.Relu, bias=csb["b1"][:, 0:1])

            feats = pfeat.tile([34, N_PIX], BF16, name="feats", tag="feats")
            nc.gpsimd.tensor_copy(feats[32:34, :], coords_bf[:, :])
            for (r0, nr, dst0) in ((0, 128, 0), (128, 21, 512)):
                cps = ps_front.tile([32, nr * 4], F32, name="c2ps", tag="fr")
                for si, (di, dj) in enumerate(SHIFTS):
                    nc.tensor.matmul(
                        cps[:, :],
                        w2s_bf[:, si * 32:(si + 1) * 32],
                        h1v[:, di + r0:di + r0 + nr, dj:dj + 4],
                        start=(si == 0), stop=(si == 3),
                    )
                nc.scalar.activation(feats[0:32, dst0:dst0 + nr * 4], cps[:, :],
                                     AF.Relu, bias=csb["b2"][:, 0:1])
            S["feats"] = feats
            return S

        def front_b(S):
            """LN stats from s/G on the PE, then K/Q/V projections."""
            feats = S["feats"]
            # s = sum_f feats[:, f]; G = feats @ feats^T (via PE transposes)
            s_sb = pst.tile([34, 1], F32, name="s_sb", tag="s_sb")
            nc.vector.tensor_reduce(s_sb[:, :], feats[:, :],
                                    axis=mybir.AxisListType.X, op=ALU.add)
            g_ps = ps_front.tile([34, 34], F32, name="g_ps", tag="fr")
            for ci, (c0, c1) in enumerate(CH):
                csz = c1 - c0
                ft_ps = ps_a2.tile([128, 34], BF16, name="ft_ps", tag="a2")
                nc.tensor.transpose(ft_ps[0:csz, :], feats[:, c0:c1],
                                    id34_bf[:, :])
                ft_sb = pst.tile([128, 34], BF16, name="ft_sb", tag="ft")
                nc.vector.tensor_copy(ft_sb[0:csz, :], ft_ps[0:csz, :])
                nc.tensor.matmul(g_ps[:, :], ft_sb[0:csz, :],
                                 ft_sb[0:csz, :],
                                 start=(ci == 0), stop=(ci == 4))
            g_sb = pst.tile([34, 34], BF16, name="g_sb", tag="g_sb")
            nc.vector.tensor_copy(g_sb[:, :], g_ps[:, :])
            gw2_ps = ps_front.tile([34, 512], F32, name="gw2_ps", tag="fr")
            nc.tensor.matmul(gw2_ps[:, :], g_sb[:, :], kqw2_bf[:, :],
                             start=True, stop=True)
            d2 = psq.tile([34, 768], BF16, name="d2", tag="d2")
            nc.vector.tensor_tensor(d2[:, 0:512], csb["kqw2"][:, :],
                                    gw2_ps[:, :], op=ALU.mult)
            gwv_ps = ps_front.tile([34, 256], F32, name="gwv_ps", tag="fr")
            nc.tensor.matmul(gwv_ps[:, :], g_sb[:, :], vw_bf[:, :],
                             start=True, stop=True)
            nc.vector.tensor_tensor(d2[:, 512:768], csb["vw"][:, :],
                                    gwv_ps[:, :], op=ALU.mult)
            cs2_ps = ps_front.tile([1, 512], F32, name="cs2_ps", tag="fr")
            nc.tensor.matmul(cs2_ps[:, :], ones_bf[0:34, 0:1], d2[:, 0:512],
                             start=True, stop=True)
            csv_ps = ps_front.tile([1, 256], F32, name="csv_ps", tag="fr")
            nc.tensor.matmul(csv_ps[:, :], ones_bf[0:34, 0:1], d2[:, 512:768],
                             start=True, stop=True)
            # per-(h, qk) partial ssq, then fold heads
            r1 = pst.tile([1, 8], F32, name="r1", tag="r1")
            nc.vector.tensor_reduce(
                r1[:, :].rearrange("p (h t u) -> p h t u", t=2, u=1),
                cs2_ps[:, :].rearrange("p (h t d) -> p h t d", t=2, d=64),
                axis=mybir.AxisListType.X, op=ALU.add)
            ssqr = pst.tile([1, 3], F32, name="ssqr", tag="ssqr")
            nc.vector.tensor_reduce(
                ssqr[:, 0:2].rearrange("p (t u) -> p t u", u=1),
                r1[:, :].rearrange("p (h t) -> p t h", t=2),
                axis=mybir.AxisListType.X, op=ALU.add)
            nc.vector.tensor_reduce(ssqr[:, 2:3], csv_ps[:, :],
                                    axis=mybir.AxisListType.X, op=ALU.add)
            stats_ps = ps_front.tile([1, 6], F32, name="stats_ps", tag="fr")
            nc.tensor.matmul(stats_ps[0:1, 0:3], s_sb[:, :],
                             csb["wsum3"][:, :], start=True, stop=True)
            nc.tensor.matmul(stats_ps[0:1, 3:6], s_sb[:, :],
                             csb["wb3"][:, :], start=True, stop=True)
            mu3 = pst.tile([1, 3], F32, name="mu3", tag="mu3")
            nc.vector.scalar_tensor_tensor(mu3[:, :], stats_ps[0:1, 0:3],
                                           1.0 / LN_N, csb["cs3k"][0:1, :],
                                           op0=ALU.mult, op1=ALU.add)
            tsq = pst.tile([1, 3], F32, name="tsq", tag="tsq")
            nc.vector.tensor_tensor(tsq[:, :], ssqr[:, :], stats_ps[0:1, 3:6],
                                    op=ALU.add)
            msq3 = pst.tile([1, 3], F32, name="msq3", tag="msq3")
            nc.vector.scalar_tensor_tensor(msq3[:, :], tsq[:, :], 1.0 / LN_N,
                                           csb["c3k"][0:1, :],
                                           op0=ALU.mult, op1=ALU.add)
            S["mu3"] = mu3
            S["msq3"] = msq3

            # projections (plain copies; no accumulation needed)
            stacked = []
            for h in range(HEADS):
                st_t = pqk.tile([128, N_PIX], BF16, name="st_t", tag="qk")
                stacked.append(st_t)
                pps = ps_front.tile([128, 512], F32, name="pps", tag="fr")
                pps2 = ps_front.tile([128, 84], F32, name="pps2", tag="fr")
                nc.tensor.matmul(pps[:, :], kqw2_bf[:, h * 128:(h + 1) * 128],
                                 feats[:, 0:512], start=True, stop=True)
                nc.tensor.matmul(pps2[:, :], kqw2_bf[:, h * 128:(h + 1) * 128],
                                 feats[:, 512:596], start=True, stop=True)
                nc.vector.tensor_scalar_add(st_t[:, 0:512], pps[:, :],
                                            csb["qkb2"][:, h:h + 1])
                nc.vector.tensor_scalar_add(st_t[:, 512:596], pps2[:, :],
                                            csb["qkb2"][:, h:h + 1])

            vtiles = []
            for ci, (c0, c1) in enumerate(CH):
                csz = c1 - c0
                vps = ps_front.tile([128, 256], F32, name="vps", tag="fr")
                nc.tensor.matmul(vps[0:csz, :], feats[:, c0:c1],
                                 vw_bf[:, :], start=True, stop=True)
                vt = pv.tile([128, 512], BF16, name="vt", tag="v")
                vt3 = vt.rearrange("p (h c) -> p h c", c=128)
                vps3 = vps.rearrange("p (h c) -> p h c", c=64)
                nc.vector.memset(vt3[0:csz, :, 64:128], 1.0)
                nc.vector.scalar_tensor_tensor(
                    vt3[0:csz, :, 0:64], vps3[0:csz, :, :], 1.0,
                    vbb3c[0:csz, :, 0:64],
                    op0=ALU.mult, op1=ALU.add)
                vtiles.append(vt)
            S["stacked"] = stacked
            S["vtiles"] = vtiles
            return S

        def front_c(S):
            """LN scalar pipeline + LN apply (fp32 -> bf16)."""
            mu3, msq3 = S["mu3"], S["msq3"]
            nmu2 = pst.tile([1, 3], F32, name="nmu2", tag="nmu2")
            nc.vector.scalar_tensor_tensor(nmu2[:, :], mu3[:, :], -1.0,
                                           mu3[:, :],
                                           op0=ALU.mult, op1=ALU.mult)
            var3 = pst.tile([1, 3], F32, name="var3", tag="var3")
            nc.vector.tensor_tensor(var3[:, :], msq3[:, :], nmu2[:, :],
                                    op=ALU.add)
            std3 = pst.tile([1, 3], F32, name="std3", tag="std3")
            nc.scalar.activation(std3[:, :], var3[:, :], AF.Sqrt,
                                 bias=csb["epsc"][0:1, 0:1])
            rsnmr = pst.tile([1, 6], F32, name="rsnmr", tag="rsnmr")
            rsv = rsnmr.rearrange("p (a b) -> p a b", b=2)
            nc.vector.reciprocal(rsv[:, :, 0:1], std3[:, :])
            nc.vector.scalar_tensor_tensor(rsv[:, :, 1:2], mu3[:, :], -1.0,
                                           rsv[:, :, 0:1],
                                           op0=ALU.mult, op1=ALU.mult)
            bc_ps = ps_a2.tile([128, 6], F32, name="bc_ps", tag="a2")
            nc.tensor.matmul(bc_ps[:, :], csb["ones_r"][0:1, :], rsnmr[:, :],
                             start=True, stop=True)
            bc = pst.tile([128, 6], F32, name="bc", tag="bc")
            nc.vector.tensor_copy(bc[:, :], bc_ps[:, :])
            # bc cols: [rsQ, nmrQ, rsK, nmrK, rsV, nmrV]

            stacked_bf = []
            for h in range(HEADS):
                sb = pqkb.tile([128, N_PIX], BF16, name="st_bf", tag="qkb")
                stacked_bf.append(sb)
                nc.vector.tensor_scalar(sb[0:64, :], S["stacked"][h][0:64, :],
                                        bc[0:64, 0:1], bc[0:64, 1:2],
                                        op0=ALU.mult, op1=ALU.add)
                nc.vector.tensor_scalar(sb[64:128, :],
                                        S["stacked"][h][64:128, :],
                                        bc[0:64, 2:3], bc[0:64, 3:4],
                                        op0=ALU.mult, op1=ALU.add)
            for ci, (c0, c1) in enumerate(CH):
                csz = c1 - c0
                vt3 = S["vtiles"][ci].rearrange("p (h c) -> p h c", c=128)
                nc.vector.tensor_scalar(vt3[0:csz, :, 0:64],
                                        vt3[0:csz, :, 0:64],
                                        bc[0:csz, 4:5], bc[0:csz, 5:6],
                                        op0=ALU.mult, op1=ALU.add)
            S["stacked_bf"] = stacked_bf
            S["eall"] = [peall.tile([128, N_PIX], BF16, name=f"eall{i}",
                                    tag="eall") for i in range(2)]
            return S

        # ---- attention stages (pipeline carried across samples) ----
        def at_chunk(S, p, ci, dest):
            c0, c1 = CH[ci]
            csz = c1 - c0
            w = 512 if not p["merged"] else W84
            aps = ps_at.tile([128, 512], F32, name="aps", tag="at")
            if p["merged"]:
                for h in range(HEADS):
                    nc.tensor.matmul(aps[0:csz, h * 84:(h + 1) * 84],
                                     qklin_bf[:, c0:c1],
                                     S["stacked_bf"][h][:, 512:596],
                                     start=True, stop=True)
            else:
                nc.tensor.matmul(aps[0:csz, 0:512],
                                 qklin_bf[:, c0:c1],
                                 S["stacked_bf"][p["h"]][:, 0:512],
                                 start=True, stop=True)
            et = pet.tile([128, 512], F32, name="et", tag="et")
            nc.scalar.activation(et[0:csz, 0:w], aps[0:csz, 0:w],
                                 AF.Exp,
                                 bias=csb["qkbias"][0:csz, ci:ci + 1])
            nc.gpsimd.tensor_scalar_min(et[0:csz, 0:w],
                                        et[0:csz, 0:w], 1.0)
            nc.vector.scalar_tensor_tensor(
                dest[0:csz, 0:w], aps[0:csz, 0:w],
                csb["qkbias"][0:csz, 5 + ci:6 + ci],
                et[0:csz, 0:w], op0=ALU.add, op1=ALU.max)

        def e_c2(st, c2i):
            S, p, tiles = st["S"], st["p"], st["tiles"]
            c20, c21 = CH[c2i]
            c2sz = c21 - c20
            w = 512 if not p["merged"] else W84
            if c2i == 0:
                st["eps"] = ps_e.tile([128, 512], F32, name="eps_t", tag="e")
            eps_t = st["eps"]
            a2ps = ps_a2.tile([128, 512], F32, name="a2ps", tag="a2")
            for j in range(2):
                nc.tensor.matmul(
                    a2ps[0:128, 0:w],
                    alin_i8[j][:, 256 * c2i:256 * c2i + 256],
                    tiles[j][:, :, 0:w],
                    start=(j == 0), stop=False,
                    perf_mode=mybir.MatmulPerfMode.DoubleRowSwInterleave)
            nc.tensor.matmul(a2ps[0:128, 0:w],
                             alin_bf4[:, 128 * c2i:128 * c2i + 128],
                             tiles[2][0:84, 0:w],
                             start=False, stop=True)
            ext = pext.tile([128, 512], BF16, name="ext", tag="ext")
            nc.scalar.activation(ext[0:c2sz, 0:w],
                                 a2ps[0:c2sz, 0:w], AF.Exp,
                                 bias=csb["expb"][0:c2sz, c2i:c2i + 1],
                                 scale=1.0 / ALSC)
            if p["merged"]:
                # PSUM accumulation groups must not interleave within a
                # bank's 2KB zero region: buffer the ext tiles and run the
                # four per-head accumulations sequentially in e_tail.
                st.setdefault("exts", []).append(ext)
            else:
                nc.tensor.matmul(eps_t[:, 0:512],
                                 S["vtiles"][c2i][0:c2sz,
                                                  p["h"] * 128:
                                                  (p["h"] + 1) * 128],
                                 ext[0:c2sz, 0:512],
                                 start=(c2i == 0), stop=(c2i == 4))

        def e_tail(st):
            S, p, eps_t = st["S"], st["p"], st["eps"]
            w = 512 if not p["merged"] else W84
            eall = S["eall"]
            if p["merged"]:
                for h in range(HEADS):
                    for c2i, (c20, c21) in enumerate(CH):
                        c2sz = c21 - c20
                        nc.tensor.matmul(
                            eps_t[:, h * 84:(h + 1) * 84],
                            S["vtiles"][c2i][0:c2sz, h * 128:(h + 1) * 128],
                            st["exts"][c2i][0:c2sz, h * 84:(h + 1) * 84],
                            start=(c2i == 0), stop=(c2i == 4))
            recip64 = pst.tile([64, 512], F32, name="recip64", tag="recip")
            nc.vector.reciprocal(recip64[:, 0:w], eps_t[64:128, 0:w])
            if p["merged"]:
                for h in range(HEADS):
                    nc.vector.tensor_tensor(
                        eall[h // 2][(h % 2) * 64:(h % 2) * 64 + 64, 512:596],
                        eps_t[0:64, h * 84:(h + 1) * 84],
                        recip64[:, h * 84:(h + 1) * 84], op=ALU.mult)
            else:
                h = p["h"]
                nc.vector.tensor_tensor(
                    eall[h // 2][(h % 2) * 64:(h % 2) * 64 + 64, 0:512],
                    eps_t[0:64, 0:512], recip64[:, 0:512], op=ALU.mult)

        pending = [None]

        def do_pass(S, p):
            pair0 = pat.tile([128, 2, 512], FP8, name="atp0", tag="atile")
            pair1 = pat.tile([128, 2, 512], FP8, name="atp1", tag="atile")
            at4 = pat.tile([128, 512], BF16, name="at4", tag="a4")
            tiles = [pair0, pair1, at4]
            at_chunk(S, p, 0, pair0[:, 0, :])
            at_chunk(S, p, 1, pair0[:, 1, :])
            at_chunk(S, p, 2, pair1[:, 0, :])
            at_chunk(S, p, 3, pair1[:, 1, :])
            prev = pending[0]
            if prev is None:
                at_chunk(S, p, 4, at4[:, :])
            else:
                e_c2(prev, 0)
                e_c2(prev, 1)
                e_c2(prev, 2)
                at_chunk(S, p, 4, at4[:, :])
                e_c2(prev, 3)
                e_c2(prev, 4)
                e_tail(prev)
            pending[0] = {"S": S, "p": p, "tiles": tiles}

        def flush_pipe():
            prev = pending[0]
            for c2i in range(5):
                e_c2(prev, c2i)
            e_tail(prev)
            pending[0] = None

        def tail(S):
            """lin1 + LN2 raw stats (scalar pipeline batched at the end)."""
            s, eall = S["s"], S["eall"]
            e2 = psq.tile([64, N_PIX], F32, name="e2", tag="e2")
            ls2 = pst.tile([64, 2], F32, name="ls2", tag="ls2")
            lpart = pst.tile([64, 2], F32, name="lpart", tag="lpart")
            for (f0, f1) in FH:
                fsz = f1 - f0
                lps = ps_e.tile([64, 512], F32, name="lps", tag="e")
                for ck in range(2):
                    nc.tensor.matmul(lps[:, 0:fsz],
                                     lin1w_bf[:, ck * 64:(ck + 1) * 64],
                                     eall[ck][:, f0:f1],
                                     start=(ck == 0), stop=(ck == 1))
                nc.scalar.activation(e2[:, f0:f1], lps[:, 0:fsz], AF.Relu,
                                     bias=csb["bl1"][:, 0:1],
                                     accum_out=lpart[:, (0 if f0 == 0 else 1):
                                                     (1 if f0 == 0 else 2)])
            nc.vector.tensor_reduce(ls2[:, 0:1], lpart[:, :],
                                    axis=mybir.AxisListType.X, op=ALU.add)
            sqe = psq.tile([64, N_PIX], F32, name="sqe", tag="sqe")
            nc.scalar.activation(sqe[:, :], e2[:, :], AF.Square,
                                 accum_out=ls2[:, 1:2])
            nc.vector.tensor_reduce(emax_raw[:, s:s + 1], e2[:, :],
                                    axis=mybir.AxisListType.X, op=ALU.max)
            st2 = ps_at.tile([1, 2], F32, name="st2", tag="at")
            nc.tensor.matmul(st2[0:1, :], csb["ones_c"][0:64, 0:1], ls2[:, :],
                             start=True, stop=True)
            nc.vector.tensor_copy(stats2_all[:, 2 * s:2 * s + 2], st2[0:1, :])

        # ---- pipelined schedule: sample s+1's front-end is emitted between
        # sample s's attention passes; the at/e pass pipeline is carried
        # across the sample boundary.
        S = front_a(0)
        front_b(S)
        front_c(S)
        states = {0: S}
        for s in range(spb):
            S = states[s]
            plist = ([dict(h=h, merged=False) for h in range(HEADS)]
                     + [dict(h=None, merged=True)])
            do_pass(S, plist[0])
            if s > 0:
                tail(states.pop(s - 1))
            do_pass(S, plist[1])
            if s + 1 < spb:
                Sn = front_a(s + 1)
            do_pass(S, plist[2])
            if s + 1 < spb:
                front_b(Sn)
            do_pass(S, plist[3])
            if s + 1 < spb:
                front_c(Sn)
                states[s + 1] = Sn
            do_pass(S, plist[4])
        flush_pipe()
        tail(states.pop(spb - 1))

        # ---------------- batched LN2 scalar pipeline (all samples) --------
        m2a = pst.tile([1, 2 * spb], F32, name="m2a", tag="m2a")
        m2av = m2a.rearrange("p (a b) -> p a b", b=2)
        nc.vector.tensor_scalar_mul(m2a[:, :], stats2_all[:, :], 1.0 / LN2_N)
        nmu2a = pst.tile([1, spb], F32, name="nmu2a", tag="nmu2a")
        nc.vector.scalar_tensor_tensor(nmu2a[:, :],
                                       m2av[:, :, 0:1], -1.0, m2av[:, :, 0:1],
                                       op0=ALU.mult, op1=ALU.mult)
        var2a = pst.tile([1, spb], F32, name="var2a", tag="var2a")
        nc.vector.tensor_tensor(var2a[:, :], m2av[:, :, 1:2], nmu2a[:, :],
                                op=ALU.add)
        std2a = pst.tile([1, spb], F32, name="std2a", tag="std2a")
        nc.scalar.activation(std2a[:, :], var2a[:, :], AF.Sqrt,
                             bias=csb["epsc"][0:1, 0:1])
        rs2a = pst.tile([1, 2 * spb], F32, name="rs2a", tag="rs2a")
        rs2av = rs2a.rearrange("p (a b) -> p a b", b=2)
        nc.vector.reciprocal(rs2av[:, :, 0:1], std2a[:, :])
        nc.vector.scalar_tensor_tensor(rs2av[:, :, 1:2],
                                       m2av[:, :, 0:1], -1.0,
                                       rs2av[:, :, 0:1],
                                       op0=ALU.mult, op1=ALU.mult)
        bc2p = ps_at.tile([64, 2 * spb], F32, name="bc2p", tag="at")
        nc.tensor.matmul(bc2p[:, :], csb["ones_r"][0:1, 0:64], rs2a[:, :],
                         start=True, stop=True)
        bc2 = pst.tile([64, 2 * spb], F32, name="bc2", tag="bc2")
        nc.vector.tensor_copy(bc2[:, :], bc2p[:, :])
        for s in range(spb):
            nc.vector.tensor_scalar(emax_all[:, s:s + 1], emax_raw[:, s:s + 1],
                                    bc2[:, 2 * s:2 * s + 1],
                                    bc2[:, 2 * s + 1:2 * s + 2],
                                    op0=ALU.mult, op1=ALU.add)

        # ---------------- lin2 + final elu ----------------
        l2ps = ps_at.tile([10, spb], F32, name="l2ps", tag="at")
        nc.tensor.matmul(l2ps[:, :], csb["lin2w"][:, :], emax_all[:, :],
                         start=True, stop=True)
        fe = pst.tile([10, spb], F32, name="fe", tag="fe")
        nc.scalar.activation(fe[:, :], l2ps[:, :], AF.Exp,
                             bias=csb["bl2"][:, 0:1])
        nc.vector.tensor_scalar(fe[:, :], fe[:, :], 1.0, -1.0,
                                op0=ALU.min, op1=ALU.add)
        out_sb = pst.tile([10, spb], F32, name="out_sb", tag="out_sb")
        nc.vector.scalar_tensor_tensor(out_sb[:, :], l2ps[:, :],
                                       csb["bl2"][:, 0:1], fe[:, :],
                                       op0=ALU.add, op1=ALU.max)
        nc.sync.dma_start(out=out_dram.rearrange("s t -> t s"), in_=out_sb[:, :])

    return nc


def _reference_numpy(inp):
    """Pure-numpy fallback (only used if LN affine params are nontrivial)."""
    def ln(x, g=None, b=None):
        axes = tuple(range(1, x.ndim))
        mu = x.mean(axis=axes, keepdims=True)
        var = x.var(axis=axes, keepdims=True)
        y = (x - mu) / np.sqrt(var + EPS)
        return y * g + b if g is not None else y

    def elu(x):
        return np.where(x > 0, x, np.expm1(np.minimum(x, 0)))

    x = np.asarray(inp["x"], np.float64)
    N = x.shape[0]
    w1, b1 = np.asarray(inp["conv1_w"], np.float64), np.asarray(inp["conv1_b"], np.float64)
    h = np.zeros((N, 16, 150, 5))
    for di in range(2):
        for dj in range(2):
            h += np.einsum("oc,nchw->nohw", w1[:, :, di, dj],
                           x[:, :, di:di + 150, dj:dj + 5])
    h = np.maximum(h + b1[None, :, None, None], 0)
    w2, b2 = np.asarray(inp["conv2_w"], np.float64), np.asarray(inp["conv2_b"], np.float64)
    h2 = np.zeros((N, 32, 149, 4))
    for di in range(2):
        for dj in range(2):
            h2 += np.einsum("oc,nchw->nohw", w2[:, :, di, dj],
                            h[:, :, di:di + 149, dj:dj + 4])
    h2 = np.maximum(h2 + b2[None, :, None, None], 0)
    p = np.arange(N_PIX)
    xc, yc = (p % 4) / 4.0, (p // 4) / 149.0
    feats = np.concatenate(
        [h2.transpose(0, 2, 3, 1).reshape(N, N_PIX, 32),
         np.broadcast_to(np.stack([xc, yc], 1)[None], (N, N_PIX, 2))], axis=2)

    def proj(wn, bn, gn, bn2):
        P = (feats @ np.asarray(inp[wn], np.float64) + np.asarray(inp[bn], np.float64))
        P = P.reshape(N, N_PIX, HEADS, D).transpose(0, 2, 1, 3)
        return ln(P, np.asarray(inp[gn], np.float64), np.asarray(inp[bn2], np.float64))

    K = proj("kp_w", "kp_b", "knorm_g", "knorm_b")
    Q = proj("qp_w", "qp_b", "qnorm_g", "qnorm_b")
    V = proj("vp_w", "vp_b", "vnorm_g", "vnorm_b")
    A = elu(Q @ np.asarray(inp["qlin_w"], np.float64) + np.asarray(inp["qlin_b"], np.float64)
            + K @ np.asarray(inp["klin_w"], np.float64) + np.asarray(inp["klin_b"], np.float64))
    A = A @ np.asarray(inp["alin_w"], np.float64) + np.asarray(inp["alin_b"], np.float64)
    A = A - A.max(axis=-1, keepdims=True)
    A = np.exp(A)
    A = A / A.sum(axis=-1, keepdims=True)
    E = np.einsum("bhfc,bhcd->bhfd", A, V)
    E = E.transpose(0, 2, 1, 3).reshape(N, N_PIX, HEADS * D)
    E = np.maximum(E @ np.asarray(inp["lin1_w"], np.float64)
                   + np.asarray(inp["lin1_b"], np.float64), 0)
    E = ln(E)
    E = E.max(axis=1)
    out = E @ np.asarray(inp["lin2_w"], np.float64) + np.asarray(inp["lin2_b"], np.float64)
    return elu(out).astype(np.float32)


def kernel(**inputs):
    trivial = (np.all(np.asarray(inputs["knorm_g"]) == 1.0)
               and np.all(np.asarray(inputs["knorm_b"]) == 0.0)
               and np.all(np.asarray(inputs["qnorm_g"]) == 1.0)
               and np.all(np.asarray(inputs["qnorm_b"]) == 0.0)
               and np.all(np.asarray(inputs["vnorm_g"]) == 1.0)
               and np.all(np.asarray(inputs["vnorm_b"]) == 0.0))
    if not trivial:
        return _reference_numpy(inputs)

    x = np.ascontiguousarray(np.asarray(inputs["x"], np.float32))
    n = x.shape[0]
    assert n == N_CORES * SPB, f"expected batch {N_CORES * SPB}, got {n}"
    consts = _prep_consts(inputs)

    if "nc" not in _cache:
        nc = build_nc(SPB)
        nc.compile()
        _cache["nc"] = nc
    nc = _cache["nc"]

    in_maps = []
    for c in range(N_CORES):
        m = dict(consts)
        m["x"] = np.ascontiguousarray(x[c * SPB:(c + 1) * SPB])
        in_maps.append(m)

    import os
    trace = bool(int(os.environ.get("KERNEL_TRACE", "0")))
    res = run_bass_kernel_spmd(nc, in_maps, list(range(N_CORES)), trace=trace)
    kernel._last_results = res
    out = np.concatenate([np.asarray(r["out"]) for r in res.results], axis=0)
    return out.astype(np.float32)


kernel._last_results = None


# revision 33
# speedup vs baseline: 1.0914x; 1.0062x over previous
"""Fused Trainium2 kernel for nn_MultiHeadRelationalModule.

Data-parallel over 8 NeuronCores (8 samples each). The whole per-sample
pipeline (conv1 -> conv2 -> +coords -> K/Q/V proj -> LayerNorm ->
relational attention (4 heads, 596x596) -> softmax -> weighted sum ->
lin1 -> LN -> maxpool -> lin2 -> elu) runs on-chip; the big attention
maps never touch HBM.

v2: all large matmuls run in bf16 (4x faster per PE row than fp32 on
TRN2; fp32 needs 4 cycles/row, bf16 needs 1). PSUM accumulation stays
fp32. Q+K projections merged into one 128-partition matmul per head;
V projections merged across heads. Elementwise work balanced across
Act/DVE/Pool engines.

Key identities used:
  elu(x) + 1 == max(x + 1, min(exp(x), 1))        (exact)
  A' = elu(z)+1 fed to matmul with alin_w: subtract colsum(alin_w) in the
       following bias to undo the +1 (softmax bias becomes
       alin_b - alin_w.sum(0)).
  softmax over c2 with A2^T layout (c2 on partitions): exp on chip,
       denominator via an appended ones-column on V in the E matmul.
  LN(x) = (x - mu) * rsqrt(var + eps); affine params in this model are
       identity (ones/zeros), verified at runtime.
  max-pool commutes with the final LN (monotone affine map).
"""

import numpy as np
from contextlib import ExitStack

import concourse.bacc as bacc
import concourse.bass as bass
import concourse.mybir as mybir
import concourse.tile as tile
from concourse.bass_utils import run_bass_kernel_spmd

F32 = mybir.dt.float32
BF16 = mybir.dt.bfloat16
FP8 = mybir.dt.float8e4
ALSC = 16.0  # alin pre-scale into fp8e4m3 normal range; undone in exp2 scale
AF = mybir.ActivationFunctionType
ALU = mybir.AluOpType

N_CORES = 8
SPB = 8               # samples per core
N_PIX = 596
HEADS = 4
D = 64
CH = [(0, 128), (128, 256), (256, 384), (384, 512), (512, 596)]
FH = [(0, 512), (512, 596)]
SHIFTS = [(0, 0), (0, 1), (1, 0), (1, 1)]
LN_N = float(HEADS * N_PIX * D)       # 152576
LN2_N = float(N_PIX * D)              # 38144
EPS = 1e-5

_cache = {}


def _prep_consts(inp):
    """Host-side preprocessing of weights into kernel-friendly layouts."""
    f = np.float32
    c = {}
    conv1_w = np.asarray(inp["conv1_w"], f)
    c["w1s"] = np.ascontiguousarray(
        np.concatenate([conv1_w[:, :, di, dj].T for (di, dj) in SHIFTS], axis=1)
    )  # (4, 64)
    c["b1"] = np.ascontiguousarray(np.asarray(inp["conv1_b"], f)[:, None])  # (16,1)
    conv2_w = np.asarray(inp["conv2_w"], f)
    c["w2s"] = np.ascontiguousarray(
        np.concatenate([conv2_w[:, :, di, dj].T for (di, dj) in SHIFTS], axis=1)
    )  # (16, 128)
    c["b2"] = np.ascontiguousarray(np.asarray(inp["conv2_b"], f)[:, None])  # (32,1)

    p = np.arange(N_PIX)
    c["coords"] = np.ascontiguousarray(
        np.stack([(p % 4) / 4.0, (p // 4) / 149.0]).astype(f)
    )  # (2, 596)

    # Q/K projection merged per head: cols h*128:h*128+64 = Q (stacked rows
    # 0:64), cols h*128+64:h*128+128 = K (stacked rows 64:128).
    qp_w = np.asarray(inp["qp_w"], f)
    kp_w = np.asarray(inp["kp_w"], f)
    kqw2 = np.zeros((34, 512), f)
    qkb2 = np.zeros((128, HEADS), f)
    for h in range(HEADS):
        kqw2[:, h * 128:h * 128 + 64] = qp_w[:, h * 64:(h + 1) * 64]
        kqw2[:, h * 128 + 64:h * 128 + 128] = kp_w[:, h * 64:(h + 1) * 64]
        qkb2[0:64, h] = np.asarray(inp["qp_b"], f)[h * 64:(h + 1) * 64]
        qkb2[64:128, h] = np.asarray(inp["kp_b"], f)[h * 64:(h + 1) * 64]
    c["kqw2"] = kqw2
    c["qkb2"] = qkb2

    c["vw"] = np.ascontiguousarray(np.asarray(inp["vp_w"], f))  # (34, 256)
    vbb2 = np.zeros((128, 512), f)   # per head: [V bias (64) | 0 (ones blk)]
    for h in range(HEADS):
        vbb2[:, h * 128:h * 128 + 64] = np.asarray(inp["vp_b"], f)[None,
                                                                   h * 64:(h + 1) * 64]
    c["vbb2"] = vbb2

    c["qklin"] = np.ascontiguousarray(
        np.concatenate([np.asarray(inp["qlin_w"], f),
                        np.asarray(inp["klin_w"], f)], axis=0)
    )  # (128, 596): rows 0:64 qlin (Q), 64:128 klin (K)

    qkbias = np.zeros((128, 10), f)
    qkl_b = np.asarray(inp["qlin_b"], f) + np.asarray(inp["klin_b"], f)
    for ci, (c0, c1) in enumerate(CH):
        qkbias[0:c1 - c0, ci] = qkl_b[c0:c1]
        qkbias[0:c1 - c0, 5 + ci] = qkl_b[c0:c1] + 1.0
    c["qkbias"] = qkbias

    c["alin"] = np.ascontiguousarray(np.asarray(inp["alin_w"], f))  # (596, 596)
    # fp8e4m3 DoubleRowSwInterleave weight pairs for alin rows 0:512 (x16 so
    # the ~0.05-scale entries sit in e4m3's normal range; undone in exp2's
    # scale).  Per matmul slice: cols [A[m], B[m]] pairs, m descending.
    import ml_dtypes
    alin16 = np.pad(np.asarray(inp["alin_w"], f) * ALSC, ((0, 0), (0, 44)))
    for j in range(2):
        A = alin16[256 * j:256 * j + 128]
        B = alin16[256 * j + 128:256 * j + 256]
        buf = np.zeros((128, 1280), f)
        for ci in range(5):
            c0 = 128 * ci
            blk = np.empty((128, 256), f)
            blk[:, 0::2] = A[:, c0:c0 + 128][:, ::-1]
            blk[:, 1::2] = B[:, c0:c0 + 128][:, ::-1]
            buf[:, 2 * c0:2 * c0 + 256] = blk
        c[f"alin_i8_{j}"] = np.ascontiguousarray(
            buf.astype(ml_dtypes.float8_e4m3))

    expb = np.zeros((128, 5), f)
    eb = np.asarray(inp["alin_b"], f) - np.asarray(inp["alin_w"], f).sum(axis=0)
    for ci, (c0, c1) in enumerate(CH):
        expb[0:c1 - c0, ci] = eb[c0:c1]
    c["expb"] = expb

    l1 = np.zeros((128, 128), f)
    lin1_w = np.asarray(inp["lin1_w"], f)
    l1[:, 0:64] = lin1_w[0:128]
    l1[:, 64:128] = lin1_w[128:256]
    c["lin1w"] = l1
    c["bl1"] = np.ascontiguousarray(np.asarray(inp["lin1_b"], f)[:, None])  # (64,1)
    c["lin2w"] = np.ascontiguousarray(np.asarray(inp["lin2_w"], f))  # (64,10)
    bl2 = np.zeros((10, 2), f)
    bl2[:, 0] = np.asarray(inp["lin2_b"], f)
    bl2[:, 1] = np.asarray(inp["lin2_b"], f) + 1.0
    c["bl2"] = bl2
    c["ones_r"] = np.ones((1, 128), f)
    c["ones_c"] = np.ones((128, 1), f)
    c["epsc"] = np.full((1, 1), EPS, f)
    c["id34"] = np.eye(34, dtype=f)
    # LN-stat helper constants: per tensor T in (Q, K, V) with weights W_T
    # (34, 256) and bias b_T: sum(T) = s^T W_T 1 + 596*sum(b),
    # ssq(T) = sum_k w_k^T G w_k + 2 s^T (W_T b_T) + 596*||b_T||^2.
    wsum3 = np.zeros((34, 3), f)
    wb3 = np.zeros((34, 3), f)
    c3k = np.zeros((1, 3), f)
    cs3k = np.zeros((1, 3), f)
    for i, (wn, bn) in enumerate((("qp_w", "qp_b"), ("kp_w", "kp_b"),
                                  ("vp_w", "vp_b"))):
        W = np.asarray(inp[wn], np.float64)
        b = np.asarray(inp[bn], np.float64)
        wsum3[:, i] = W.sum(axis=1).astype(f)
        wb3[:, i] = (2.0 * (W @ b)).astype(f)
        c3k[0, i] = np.float32(596.0 * float(b @ b) / LN_N)
        cs3k[0, i] = np.float32(596.0 * float(b.sum()) / LN_N)
    c["wsum3"] = wsum3
    c["wb3"] = wb3
    c["c3k"] = c3k
    c["cs3k"] = cs3k
    return c


CONST_SHAPES = {
    "w1s": (4, 64), "b1": (16, 1), "w2s": (16, 128), "b2": (32, 1),
    "coords": (2, N_PIX), "kqw2": (34, 512), "qkb2": (128, HEADS),
    "vw": (34, 256), "vbb2": (128, 512),
    "qklin": (128, N_PIX), "qkbias": (128, 10), "alin": (N_PIX, N_PIX),
    "expb": (128, 5), "lin1w": (128, 128), "bl1": (64, 1), "lin2w": (64, 10),
    "bl2": (10, 2), "ones_r": (1, 128), "ones_c": (128, 1), "epsc": (1, 1),
    "id34": (34, 34), "wsum3": (34, 3), "wb3": (34, 3), "c3k": (1, 3),
    "cs3k": (1, 3),
}
CONST_FP8 = {"alin_i8_0": (128, 1280), "alin_i8_1": (128, 1280)}


def build_nc(spb=SPB):
    """Build the Bass program (same program runs SPMD on each core)."""
    nc = bacc.Bacc("TRN2", target_bir_lowering=False, debug=False)

    x_dram = nc.dram_tensor("x", [spb, 4, 151, 6], F32, kind="ExternalInput").ap()
    out_dram = nc.dram_tensor("out", [spb, 10], F32, kind="ExternalOutput").ap()
    cdram = {
        k: nc.dram_tensor(k, list(v), F32, kind="ExternalInput").ap()
        for k, v in CONST_SHAPES.items()
    }
    for k, v in CONST_FP8.items():
        cdram[k] = nc.dram_tensor(k, list(v), FP8, kind="ExternalInput").ap()

    with tile.TileContext(nc) as tc, ExitStack() as ctx:
        pc = ctx.enter_context(tc.tile_pool(name="consts", bufs=1))
        # SBUF pools
        px = ctx.enter_context(tc.tile_pool(name="px", bufs=2))
        ph1 = ctx.enter_context(tc.tile_pool(name="ph1", bufs=2))
        pfeat = ctx.enter_context(tc.tile_pool(name="pfeat", bufs=2))
        pqk = ctx.enter_context(tc.tile_pool(name="pqk", bufs=8))
        pqkb = ctx.enter_context(tc.tile_pool(name="pqkb", bufs=8))
        pv = ctx.enter_context(tc.tile_pool(name="pv", bufs=12))
        pat = ctx.enter_context(tc.tile_pool(name="pat", bufs=10))
        pet = ctx.enter_context(tc.tile_pool(name="pet", bufs=3))
        pext = ctx.enter_context(tc.tile_pool(name="pext", bufs=7))
        psq = ctx.enter_context(tc.tile_pool(name="psq", bufs=2))
        pst = ctx.enter_context(tc.tile_pool(name="pst", bufs=3))
        peall = ctx.enter_context(tc.tile_pool(name="peall", bufs=4))
        pfix = ctx.enter_context(tc.tile_pool(name="pfix", bufs=1))
        # PSUM pools (8 banks total: 2+2+2+2), phase-separated so sample
        # s+1's front-end never waits on sample s's tail.
        PS = bass.MemorySpace.PSUM
        ps_front = ctx.enter_context(tc.tile_pool(name="ps_front", bufs=1, space=PS))
        ps_at = ctx.enter_context(tc.tile_pool(name="ps_at", bufs=3, space=PS))
        ps_a2 = ctx.enter_context(tc.tile_pool(name="ps_a2", bufs=3, space=PS))
        ps_e = ctx.enter_context(tc.tile_pool(name="ps_e", bufs=1, space=PS))

        # ---- prefetch sample 0's input before the const DMAs ----
        x_t0 = px.tile([4, 151, 6], F32, name="x_t", tag="x")
        nc.sync.dma_start(out=x_t0[:, :, :], in_=x_dram[0])

        # ---- load constants (fp32) ----
        csb = {}
        for k, shp in CONST_SHAPES.items():
            if k == "alin":
                continue
            t = pc.tile(list(shp), F32, name=f"c_{k}")
            nc.sync.dma_start(out=t[:, :], in_=cdram[k][:, :])
            csb[k] = t
        alin4_f32 = pc.tile([84, N_PIX], F32, name="c_alin4")
        nc.sync.dma_start(out=alin4_f32[:, :], in_=cdram["alin"][512:596, :])
        alin_i8 = []
        for j in range(2):
            t = pc.tile([128, 1280], FP8, name=f"alin_i8_{j}")
            nc.sync.dma_start(out=t[:, :], in_=cdram[f"alin_i8_{j}"][:, :])
            alin_i8.append(t)

        # ---- one-time bf16 conversions of matmul operands ----
        def to_bf(name, src, shp):
            t = pc.tile(list(shp), BF16, name=name)
            nc.vector.tensor_copy(t[:, :], src[:, :])
            return t

        w1s_bf = to_bf("w1s_bf", csb["w1s"], (4, 64))
        w2s_bf = to_bf("w2s_bf", csb["w2s"], (16, 128))
        coords_bf = to_bf("coords_bf", csb["coords"], (2, N_PIX))
        kqw2_bf = to_bf("kqw2_bf", csb["kqw2"], (34, 512))
        vw_bf = to_bf("vw_bf", csb["vw"], (34, 256))
        qklin_bf = to_bf("qklin_bf", csb["qklin"], (128, N_PIX))
        lin1w_bf = to_bf("lin1w_bf", csb["lin1w"], (128, 128))
        alin_bf4 = pc.tile([84, 640], BF16, name="alin_bf4")
        nc.vector.memset(alin_bf4[:, 596:640], 0.0)
        nc.vector.tensor_scalar_mul(alin_bf4[:, 0:N_PIX], alin4_f32[:, :],
                                    ALSC)
        id34_bf = to_bf("id34_bf", csb["id34"], (34, 34))
        ones_bf = pc.tile([128, 1], BF16, name="ones_bf")
        nc.vector.memset(ones_bf[:, :], 1.0)
        emax_all = pfix.tile([64, spb], F32, name="emax_all")
        emax_raw = pfix.tile([64, spb], F32, name="emax_raw")
        stats2_all = pfix.tile([1, 2 * spb], F32, name="stats2_all")

        # ================= pipelined per-sample stages =================
        W84 = 84 * HEADS
        vbb3c = csb["vbb2"].rearrange("p (h c) -> p h c", c=128)

        def front_a(s):
            """x load/cast + conv1 + conv2 + coords -> feats."""
            S = {"s": s}
            if s == 0:
                x_t = x_t0
            else:
                x_t = px.tile([4, 151, 6], F32, name="x_t", tag="x")
                nc.sync.dma_start(out=x_t[:, :, :], in_=x_dram[s])
            x_bf = px.tile([4, 151, 6], BF16, name="x_bf", tag="xbf")
            nc.gpsimd.tensor_copy(x_bf[:, :, :], x_t[:, :, :])

            h1 = ph1.tile([16, 750], BF16, name="h1", tag="h1")
            h1v = h1.rearrange("c (h w) -> c h w", w=5)
            for (r0, nr, dst0) in ((0, 102, 0), (102, 48, 510)):
                cps = ps_front.tile([16, nr * 5], F32, name="c1ps", tag="fr")
                for si, (di, dj) in enumerate(SHIFTS):
                    nc.tensor.matmul(
                        cps[:, :],
                        w1s_bf[:, si * 16:(si + 1) * 16],
                        x_bf[:, di + r0:di + r0 + nr, dj:dj + 5],
                        start=(si == 0), stop=(si == 3),
                    )
                nc.scalar.activation(h1[:, dst0:dst0 + nr * 5], cps[:, :],
                                     AF.Relu, bias=csb["b1"][:, 0:1])

            feats = pfeat.tile([34, N_PIX], BF16, name="feats", tag="feats")
            nc.gpsimd.tensor_copy(feats[32:34, :], coords_bf[:, :])
            for (r0, nr, dst0) in ((0, 128, 0), (128, 21, 512)):
                cps = ps_front.tile([32, nr * 4], F32, name="c2ps", tag="fr")
                for si, (di, dj) in enumerate(SHIFTS):
                    nc.tensor.matmul(
                        cps[:, :],
                        w2s_bf[:, si * 32:(si + 1) * 32],
                        h1v[:, di + r0:di + r0 + nr, dj:dj + 4],
                        start=(si == 0), stop=(si == 3),
                    )
                nc.scalar.activation(feats[0:32, dst0:dst0 + nr * 4], cps[:, :],
                                     AF.Relu, bias=csb["b2"][:, 0:1])
            S["feats"] = feats
            return S

        def front_b(S):
            """LN stats from s/G on the PE, then K/Q/V projections."""
            feats = S["feats"]
            # s = sum_f feats[:, f]; G = feats @ feats^T (via PE transposes)
            s_sb = pst.tile([34, 1], F32, name="s_sb", tag="s_sb")
            nc.vector.tensor_reduce(s_sb[:, :], feats[:, :],
                                    axis=mybir.AxisListType.X, op=ALU.add)
            g_ps = ps_front.tile([34, 34], F32, name="g_ps", tag="fr")
            for ci, (c0, c1) in enumerate(CH):
                csz = c1 - c0
                ft_ps = ps_a2.tile([128, 34], BF16, name="ft_ps", tag="a2")
                nc.tensor.transpose(ft_ps[0:csz, :], feats[:, c0:c1],
                                    id34_bf[:, :])
                ft_sb = pst.tile([128, 34], BF16, name="ft_sb", tag="ft")
                nc.vector.tensor_copy(ft_sb[0:csz, :], ft_ps[0:csz, :])
                nc.tensor.matmul(g_ps[:, :], ft_sb[0:csz, :],
                                 ft_sb[0:csz, :],
                                 start=(ci == 0), stop=(ci == 4))
            g_sb = pst.tile([34, 34], BF16, name="g_sb", tag="g_sb")
            nc.vector.tensor_copy(g_sb[:, :], g_ps[:, :])
            gw2_ps = ps_front.tile([34, 512], F32, name="gw2_ps", tag="fr")
            nc.tensor.matmul(gw2_ps[:, :], g_sb[:, :], kqw2_bf[:, :],
                             start=True, stop=True)
            d2 = psq.tile([34, 768], BF16, name="d2", tag="d2")
            nc.vector.tensor_tensor(d2[:, 0:512], csb["kqw2"][:, :],
                                    gw2_ps[:, :], op=ALU.mult)
            gwv_ps = ps_front.tile([34, 256], F32, name="gwv_ps", tag="fr")
            nc.tensor.matmul(gwv_ps[:, :], g_sb[:, :], vw_bf[:, :],
                             start=True, stop=True)
            nc.vector.tensor_tensor(d2[:, 512:768], csb["vw"][:, :],
                                    gwv_ps[:, :], op=ALU.mult)
            cs2_ps = ps_front.tile([1, 512], F32, name="cs2_ps", tag="fr")
            nc.tensor.matmul(cs2_ps[:, :], ones_bf[0:34, 0:1], d2[:, 0:512],
                             start=True, stop=True)
            csv_ps = ps_front.tile([1, 256], F32, name="csv_ps", tag="fr")
            nc.tensor.matmul(csv_ps[:, :], ones_bf[0:34, 0:1], d2[:, 512:768],
                             start=True, stop=True)
            # per-(h, qk) partial ssq, then fold heads
            r1 = pst.tile([1, 8], F32, name="r1", tag="r1")
            nc.vector.tensor_reduce(
                r1[:, :].rearrange("p (h t u) -> p h t u", t=2, u=1),
                cs2_ps[:, :].rearrange("p (h t d) -> p h t d", t=2, d=64),
                axis=mybir.AxisListType.X, op=ALU.add)
            ssqr = pst.tile([1, 3], F32, name="ssqr", tag="ssqr")
            nc.vector.tensor_reduce(
                ssqr[:, 0:2].rearrange("p (t u) -> p t u", u=1),
                r1[:, :].rearrange("p (h t) -> p t h", t=2),
                axis=mybir.AxisListType.X, op=ALU.add)
            nc.vector.tensor_reduce(ssqr[:, 2:3], csv_ps[:, :],
                                    axis=mybir.AxisListType.X, op=ALU.add)
            stats_ps = ps_front.tile([1, 6], F32, name="stats_ps", tag="fr")
            nc.tensor.matmul(stats_ps[0:1, 0:3], s_sb[:, :],
                             csb["wsum3"][:, :], start=True, stop=True)
            nc.tensor.matmul(stats_ps[0:1, 3:6], s_sb[:, :],
                             csb["wb3"][:, :], start=True, stop=True)
            mu3 = pst.tile([1, 3], F32, name="mu3", tag="mu3")
            nc.vector.scalar_tensor_tensor(mu3[:, :], stats_ps[0:1, 0:3],
                                           1.0 / LN_N, csb["cs3k"][0:1, :],
                                           op0=ALU.mult, op1=ALU.add)
            tsq = pst.tile([1, 3], F32, name="tsq", tag="tsq")
            nc.vector.tensor_tensor(tsq[:, :], ssqr[:, :], stats_ps[0:1, 3:6],
                                    op=ALU.add)
            msq3 = pst.tile([1, 3], F32, name="msq3", tag="msq3")
            nc.vector.scalar_tensor_tensor(msq3[:, :], tsq[:, :], 1.0 / LN_N,
                                           csb["c3k"][0:1, :],
                                           op0=ALU.mult, op1=ALU.add)
            S["mu3"] = mu3
            S["msq3"] = msq3

            # projections (plain copies; no accumulation needed)
            stacked = []
            for h in range(HEADS):
                st_t = pqk.tile([128, N_PIX], BF16, name="st_t", tag="qk")
                stacked.append(st_t)
                pps = ps_front.tile([128, 512], F32, name="pps", tag="fr")
                pps2 = ps_front.tile([128, 84], F32, name="pps2", tag="fr")
                nc.tensor.matmul(pps[:, :], kqw2_bf[:, h * 128:(h + 1) * 128],
                                 feats[:, 0:512], start=True, stop=True)
                nc.tensor.matmul(pps2[:, :], kqw2_bf[:, h * 128:(h + 1) * 128],
                                 feats[:, 512:596], start=True, stop=True)
                nc.vector.tensor_scalar_add(st_t[:, 0:512], pps[:, :],
                                            csb["qkb2"][:, h:h + 1])
                nc.vector.tensor_scalar_add(st_t[:, 512:596], pps2[:, :],
                                            csb["qkb2"][:, h:h + 1])

            vtiles = []
            for ci, (c0, c1) in enumerate(CH):
                csz = c1 - c0
                vps = ps_front.tile([128, 256], F32, name="vps", tag="fr")
                nc.tensor.matmul(vps[0:csz, :], feats[:, c0:c1],
                                 vw_bf[:, :], start=True, stop=True)
                vt = pv.tile([128, 512], BF16, name="vt", tag="v")
                vt3 = vt.rearrange("p (h c) -> p h c", c=128)
                vps3 = vps.rearrange("p (h c) -> p h c", c=64)
                nc.vector.memset(vt3[0:csz, :, 64:128], 1.0)
                nc.vector.scalar_tensor_tensor(
                    vt3[0:csz, :, 0:64], vps3[0:csz, :, :], 1.0,
                    vbb3c[0:csz, :, 0:64],
                    op0=ALU.mult, op1=ALU.add)
                vtiles.append(vt)
            S["stacked"] = stacked
            S["vtiles"] = vtiles
            return S

        def front_c(S):
            """LN scalar pipeline + LN apply (fp32 -> bf16)."""
            mu3, msq3 = S["mu3"], S["msq3"]
            nmu2 = pst.tile([1, 3], F32, name="nmu2", tag="nmu2")
            nc.vector.scalar_tensor_tensor(nmu2[:, :], mu3[:, :], -1.0,
                                           mu3[:, :],
                                           op0=ALU.mult, op1=ALU.mult)
            var3 = pst.tile([1, 3], F32, name="var3", tag="var3")
            nc.vector.tensor_tensor(var3[:, :], msq3[:, :], nmu2[:, :],
                                    op=ALU.add)
            std3 = pst.tile([1, 3], F32, name="std3", tag="std3")
            nc.scalar.activation(std3[:, :], var3[:, :], AF.Sqrt,
                                 bias=csb["epsc"][0:1, 0:1])
            rsnmr = pst.tile([1, 6], F32, name="rsnmr", tag="rsnmr")
            rsv = rsnmr.rearrange("p (a b) -> p a b", b=2)
            nc.vector.reciprocal(rsv[:, :, 0:1], std3[:, :])
            nc.vector.scalar_tensor_tensor(rsv[:, :, 1:2], mu3[:, :], -1.0,
                                           rsv[:, :, 0:1],
                                           op0=ALU.mult, op1=ALU.mult)
            bc_ps = ps_a2.tile([128, 6], F32, name="bc_ps", tag="a2")
            nc.tensor.matmul(bc_ps[:, :], csb["ones_r"][0:1, :], rsnmr[:, :],
                             start=True, stop=True)
            bc = pst.tile([128, 6], F32, name="bc", tag="bc")
            nc.vector.tensor_copy(bc[:, :], bc_ps[:, :])
            # bc cols: [rsQ, nmrQ, rsK, nmrK, rsV, nmrV]

            stacked_bf = []
            for h in range(HEADS):
                sb = pqkb.tile([128, N_PIX], BF16, name="st_bf", tag="qkb")
                stacked_bf.append(sb)
                nc.vector.tensor_scalar(sb[0:64, :], S["stacked"][h][0:64, :],
                                        bc[0:64, 0:1], bc[0:64, 1:2],
                                        op0=ALU.mult, op1=ALU.add)
                nc.vector.tensor_scalar(sb[64:128, :],
                                        S["stacked"][h][64:128, :],
                                        bc[0:64, 2:3], bc[0:64, 3:4],
                                        op0=ALU.mult, op1=ALU.add)
            for ci, (c0, c1) in enumerate(CH):
                csz = c1 - c0
                vt3 = S["vtiles"][ci].rearrange("p (h c) -> p h c", c=128)
                nc.vector.tensor_scalar(vt3[0:csz, :, 0:64],
                                        vt3[0:csz, :, 0:64],
                                        bc[0:csz, 4:5], bc[0:csz, 5:6],
                                        op0=ALU.mult, op1=ALU.add)
            S["stacked_bf"] = stacked_bf
            S["eall"] = [peall.tile([128, N_PIX], BF16, name=f"eall{i}",
                                    tag="eall") for i in range(2)]
            return S

        # ---- attention stages (pipeline carried across samples) ----
        def at_chunk(S, p, ci, dest):
            c0, c1 = CH[ci]
            csz = c1 - c0
            w = 512 if not p["merged"] else W84
            aps = ps_at.tile([128, 512], F32, name="aps", tag="at")
            if p["merged"]:
                for h in range(HEADS):
                    nc.tensor.matmul(aps[0:csz, h * 84:(h + 1) * 84],
                                     qklin_bf[:, c0:c1],
                                     S["stacked_bf"][h][:, 512:596],
                                     start=True, stop=True)
            else:
                nc.tensor.matmul(aps[0:csz, 0:512],
                                 qklin_bf[:, c0:c1],
                                 S["stacked_bf"][p["h"]][:, 0:512],
                                 start=True, stop=True)
            et = pet.tile([128, 512], F32, name="et", tag="et")
            nc.scalar.activation(et[0:csz, 0:w], aps[0:csz, 0:w],
                                 AF.Exp,
                                 bias=csb["qkbias"][0:csz, ci:ci + 1])
            nc.gpsimd.tensor_scalar_min(et[0:csz, 0:w],
                                        et[0:csz, 0:w], 1.0)
            nc.vector.scalar_tensor_tensor(
                dest[0:csz, 0:w], aps[0:csz, 0:w],
                csb["qkbias"][0:csz, 5 + ci:6 + ci],
                et[0:csz, 0:w], op0=ALU.add, op1=ALU.max)

        def e_c2(st, c2i):
            S, p, tiles = st["S"], st["p"], st["tiles"]
            c20, c21 = CH[c2i]
            c2sz = c21 - c20
            w = 512 if not p["merged"] else W84
            if c2i == 0:
                st["eps"] = ps_e.tile([128, 512], F32, name="eps_t", tag="e")
            eps_t = st["eps"]
            a2ps = ps_a2.tile([128, 512], F32, name="a2ps", tag="a2")
            for j in range(2):
                nc.tensor.matmul(
                    a2ps[0:128, 0:w],
                    alin_i8[j][:, 256 * c2i:256 * c2i + 256],
                    tiles[j][:, :, 0:w],
                    start=(j == 0), stop=False,
                    perf_mode=mybir.MatmulPerfMode.DoubleRowSwInterleave)
            nc.tensor.matmul(a2ps[0:128, 0:w],
                             alin_bf4[:, 128 * c2i:128 * c2i + 128],
                             tiles[2][0:84, 0:w],
                             start=False, stop=True)
            ext = pext.tile([128, 512], BF16, name="ext", tag="ext")
            nc.scalar.activation(ext[0:c2sz, 0:w],
                                 a2ps[0:c2sz, 0:w], AF.Exp,
                                 bias=csb["expb"][0:c2sz, c2i:c2i + 1],
                                 scale=1.0 / ALSC)
            if p["merged"]:
                # PSUM accumulation groups must not interleave within a
                # bank's 2KB zero region: buffer the ext tiles and run the
                # four per-head accumulations sequentially in e_tail.
                st.setdefault("exts", []).append(ext)
            else:
                nc.tensor.matmul(eps_t[:, 0:512],
                                 S["vtiles"][c2i][0:c2sz,
                                                  p["h"] * 128:
                                                  (p["h"] + 1) * 128],
                                 ext[0:c2sz, 0:512],
                                 start=(c2i == 0), stop=(c2i == 4))

        def e_tail(st):
            S, p, eps_t = st["S"], st["p"], st["eps"]
            w = 512 if not p["merged"] else W84
            eall = S["eall"]
            if p["merged"]:
                for h in range(HEADS):
                    for c2i, (c20, c21) in enumerate(CH):
                        c2sz = c21 - c20
                        nc.tensor.matmul(
                            eps_t[:, h * 84:(h + 1) * 84],
                            S["vtiles"][c2i][0:c2sz, h * 128:(h + 1) * 128],
                            st["exts"][c2i][0:c2sz, h * 84:(h + 1) * 84],
                            start=(c2i == 0), stop=(c2i == 4))
            recip64 = pst.tile([64, 512], F32, name="recip64", tag="recip")
            nc.vector.reciprocal(recip64[:, 0:w], eps_t[64:128, 0:w])
            if p["merged"]:
                for h in range(HEADS):
                    nc.vector.tensor_tensor(
                        eall[h // 2][(h % 2) * 64:(h % 2) * 64 + 64, 512:596],
                        eps_t[0:64, h * 84:(h + 1) * 84],
                        recip64[:, h * 84:(h + 1) * 84], op=ALU.mult)
            else:
                h = p["h"]
                nc.vector.tensor_tensor(
                    eall[h // 2][(h % 2) * 64:(h % 2) * 64 + 64, 0:512],
                    eps_t[0:64, 0:512], recip64[:, 0:512], op=ALU.mult)

        pending = [None]

        def do_pass(S, p):
            pair0 = pat.tile([128, 2, 512], FP8, name="atp0", tag="atile")
            pair1 = pat.tile([128, 2, 512], FP8, name="atp1", tag="atile")
            at4 = pat.tile([128, 512], BF16, name="at4", tag="a4")
            tiles = [pair0, pair1, at4]
            at_chunk(S, p, 0, pair0[:, 0, :])
            at_chunk(S, p, 1, pair0[:, 1, :])
            at_chunk(S, p, 2, pair1[:, 0, :])
            at_chunk(S, p, 3, pair1[:, 1, :])
            prev = pending[0]
            if prev is None:
                at_chunk(S, p, 4, at4[:, :])
            else:
                at_chunk(S, p, 4, at4[:, :])
                e_c2(prev, 0)
                e_c2(prev, 1)
                e_c2(prev, 2)
                e_c2(prev, 3)
                e_c2(prev, 4)
                e_tail(prev)
            pending[0] = {"S": S, "p": p, "tiles": tiles}

        def flush_pipe():
            prev = pending[0]
            for c2i in range(5):
                e_c2(prev, c2i)
            e_tail(prev)
            pending[0] = None

        def tail(S):
            """lin1 + LN2 raw stats (scalar pipeline batched at the end)."""
            s, eall = S["s"], S["eall"]
            e2 = psq.tile([64, N_PIX], F32, name="e2", tag="e2")
            ls2 = pst.tile([64, 2], F32, name="ls2", tag="ls2")
            lpart = pst.tile([64, 2], F32, name="lpart", tag="lpart")
            for (f0, f1) in FH:
                fsz = f1 - f0
                lps = ps_e.tile([64, 512], F32, name="lps", tag="e")
                for ck in range(2):
                    nc.tensor.matmul(lps[:, 0:fsz],
                                     lin1w_bf[:, ck * 64:(ck + 1) * 64],
                                     eall[ck][:, f0:f1],
                                     start=(ck == 0), stop=(ck == 1))
                nc.scalar.activation(e2[:, f0:f1], lps[:, 0:fsz], AF.Relu,
                                     bias=csb["bl1"][:, 0:1],
                                     accum_out=lpart[:, (0 if f0 == 0 else 1):
                                                     (1 if f0 == 0 else 2)])
            nc.vector.tensor_reduce(ls2[:, 0:1], lpart[:, :],
                                    axis=mybir.AxisListType.X, op=ALU.add)
            sqe = psq.tile([64, N_PIX], F32, name="sqe", tag="sqe")
            nc.scalar.activation(sqe[:, :], e2[:, :], AF.Square,
                                 accum_out=ls2[:, 1:2])
            nc.vector.tensor_reduce(emax_raw[:, s:s + 1], e2[:, :],
                                    axis=mybir.AxisListType.X, op=ALU.max)
            st2 = ps_at.tile([1, 2], F32, name="st2", tag="at")
            nc.tensor.matmul(st2[0:1, :], csb["ones_c"][0:64, 0:1], ls2[:, :],
                             start=True, stop=True)
            nc.vector.tensor_copy(stats2_all[:, 2 * s:2 * s + 2], st2[0:1, :])

        # ---- pipelined schedule: sample s+1's front-end is emitted between
        # sample s's attention passes; the at/e pass pipeline is carried
        # across the sample boundary.
        S = front_a(0)
        front_b(S)
        front_c(S)
        states = {0: S}
        for s in range(spb):
            S = states[s]
            plist = ([dict(h=h, merged=False) for h in range(HEADS)]
                     + [dict(h=None, merged=True)])
            do_pass(S, plist[0])
            if s > 0:
                tail(states.pop(s - 1))
            do_pass(S, plist[1])
            if s + 1 < spb:
                Sn = front_a(s + 1)
            do_pass(S, plist[2])
            if s + 1 < spb:
                front_b(Sn)
            do_pass(S, plist[3])
            if s + 1 < spb:
                front_c(Sn)
                states[s + 1] = Sn
            do_pass(S, plist[4])
        flush_pipe()
        tail(states.pop(spb - 1))

        # ---------------- batched LN2 scalar pipeline (all samples) --------
        m2a = pst.tile([1, 2 * spb], F32, name="m2a", tag="m2a")
        m2av = m2a.rearrange("p (a b) -> p a b", b=2)
        nc.vector.tensor_scalar_mul(m2a[:, :], stats2_all[:, :], 1.0 / LN2_N)
        nmu2a = pst.tile([1, spb], F32, name="nmu2a", tag="nmu2a")
        nc.vector.scalar_tensor_tensor(nmu2a[:, :],
                                       m2av[:, :, 0:1], -1.0, m2av[:, :, 0:1],
                                       op0=ALU.mult, op1=ALU.mult)
        var2a = pst.tile([1, spb], F32, name="var2a", tag="var2a")
        nc.vector.tensor_tensor(var2a[:, :], m2av[:, :, 1:2], nmu2a[:, :],
                                op=ALU.add)
        std2a = pst.tile([1, spb], F32, name="std2a", tag="std2a")
        nc.scalar.activation(std2a[:, :], var2a[:, :], AF.Sqrt,
                             bias=csb["epsc"][0:1, 0:1])
        rs2a = pst.tile([1, 2 * spb], F32, name="rs2a", tag="rs2a")
        rs2av = rs2a.rearrange("p (a b) -> p a b", b=2)
        nc.vector.reciprocal(rs2av[:, :, 0:1], std2a[:, :])
        nc.vector.scalar_tensor_tensor(rs2av[:, :, 1:2],
                                       m2av[:, :, 0:1], -1.0,
                                       rs2av[:, :, 0:1],
                                       op0=ALU.mult, op1=ALU.mult)
        bc2p = ps_at.tile([64, 2 * spb], F32, name="bc2p", tag="at")
        nc.tensor.matmul(bc2p[:, :], csb["ones_r"][0:1, 0:64], rs2a[:, :],
                         start=True, stop=True)
        bc2 = pst.tile([64, 2 * spb], F32, name="bc2", tag="bc2")
        nc.vector.tensor_copy(bc2[:, :], bc2p[:, :])
        for s in range(spb):
            nc.vector.tensor_scalar(emax_all[:, s:s + 1], emax_raw[:, s:s + 1],
                                    bc2[:, 2 * s:2 * s + 1],
                                    bc2[:, 2 * s + 1:2 * s + 2],
                                    op0=ALU.mult, op1=ALU.add)

        # ---------------- lin2 + final elu ----------------
        l2ps = ps_at.tile([10, spb], F32, name="l2ps", tag="at")
        nc.tensor.matmul(l2ps[:, :], csb["lin2w"][:, :], emax_all[:, :],
                         start=True, stop=True)
        fe = pst.tile([10, spb], F32, name="fe", tag="fe")
        nc.scalar.activation(fe[:, :], l2ps[:, :], AF.Exp,
                             bias=csb["bl2"][:, 0:1])
        nc.vector.tensor_scalar(fe[:, :], fe[:, :], 1.0, -1.0,
                                op0=ALU.min, op1=ALU.add)
        out_sb = pst.tile([10, spb], F32, name="out_sb", tag="out_sb")
        nc.vector.scalar_tensor_tensor(out_sb[:, :], l2ps[:, :],
                                       csb["bl2"][:, 0:1], fe[:, :],
                                       op0=ALU.add, op1=ALU.max)
        nc.sync.dma_start(out=out_dram.rearrange("s t -> t s"), in_=out_sb[:, :])

    return nc


def _reference_numpy(inp):
    """Pure-numpy fallback (only used if LN affine params are nontrivial)."""
    def ln(x, g=None, b=None):
        axes = tuple(range(1, x.ndim))
        mu = x.mean(axis=axes, keepdims=True)
        var = x.var(axis=axes, keepdims=True)
        y = (x - mu) / np.sqrt(var + EPS)
        return y * g + b if g is not None else y

    def elu(x):
        return np.where(x > 0, x, np.expm1(np.minimum(x, 0)))

    x = np.asarray(inp["x"], np.float64)
    N = x.shape[0]
    w1, b1 = np.asarray(inp["conv1_w"], np.float64), np.asarray(inp["conv1_b"], np.float64)
    h = np.zeros((N, 16, 150, 5))
    for di in range(2):
        for dj in range(2):
            h += np.einsum("oc,nchw->nohw", w1[:, :, di, dj],
                           x[:, :, di:di + 150, dj:dj + 5])
    h = np.maximum(h + b1[None, :, None, None], 0)
    w2, b2 = np.asarray(inp["conv2_w"], np.float64), np.asarray(inp["conv2_b"], np.float64)
    h2 = np.zeros((N, 32, 149, 4))
    for di in range(2):
        for dj in range(2):
            h2 += np.einsum("oc,nchw->nohw", w2[:, :, di, dj],
                            h[:, :, di:di + 149, dj:dj + 4])
    h2 = np.maximum(h2 + b2[None, :, None, None], 0)
    p = np.arange(N_PIX)
    xc, yc = (p % 4) / 4.0, (p // 4) / 149.0
    feats = np.concatenate(
        [h2.transpose(0, 2, 3, 1).reshape(N, N_PIX, 32),
         np.broadcast_to(np.stack([xc, yc], 1)[None], (N, N_PIX, 2))], axis=2)

    def proj(wn, bn, gn, bn2):
        P = (feats @ np.asarray(inp[wn], np.float64) + np.asarray(inp[bn], np.float64))
        P = P.reshape(N, N_PIX, HEADS, D).transpose(0, 2, 1, 3)
        return ln(P, np.asarray(inp[gn], np.float64), np.asarray(inp[bn2], np.float64))

    K = proj("kp_w", "kp_b", "knorm_g", "knorm_b")
    Q = proj("qp_w", "qp_b", "qnorm_g", "qnorm_b")
    V = proj("vp_w", "vp_b", "vnorm_g", "vnorm_b")
    A = elu(Q @ np.asarray(inp["qlin_w"], np.float64) + np.asarray(inp["qlin_b"], np.float64)
            + K @ np.asarray(inp["klin_w"], np.float64) + np.asarray(inp["klin_b"], np.float64))
    A = A @ np.asarray(inp["alin_w"], np.float64) + np.asarray(inp["alin_b"], np.float64)
    A = A - A.max(axis=-1, keepdims=True)
    A = np.exp(A)
    A = A / A.sum(axis=-1, keepdims=True)
    E = np.einsum("bhfc,bhcd->bhfd", A, V)
    E = E.transpose(0, 2, 1, 3).reshape(N, N_PIX, HEADS * D)
    E = np.maximum(E @ np.asarray(inp["lin1_w"], np.float64)
                   + np.asarray(inp["lin1_b"], np.float64), 0)
    E = ln(E)
    E = E.max(axis=1)
    out = E @ np.asarray(inp["lin2_w"], np.float64) + np.asarray(inp["lin2_b"], np.float64)
    return elu(out).astype(np.float32)


def kernel(**inputs):
    trivial = (np.all(np.asarray(inputs["knorm_g"]) == 1.0)
               and np.all(np.asarray(inputs["knorm_b"]) == 0.0)
               and np.all(np.asarray(inputs["qnorm_g"]) == 1.0)
               and np.all(np.asarray(inputs["qnorm_b"]) == 0.0)
               and np.all(np.asarray(inputs["vnorm_g"]) == 1.0)
               and np.all(np.asarray(inputs["vnorm_b"]) == 0.0))
    if not trivial:
        return _reference_numpy(inputs)

    x = np.ascontiguousarray(np.asarray(inputs["x"], np.float32))
    n = x.shape[0]
    assert n == N_CORES * SPB, f"expected batch {N_CORES * SPB}, got {n}"
    consts = _prep_consts(inputs)

    if "nc" not in _cache:
        nc = build_nc(SPB)
        nc.compile()
        _cache["nc"] = nc
    nc = _cache["nc"]

    in_maps = []
    for c in range(N_CORES):
        m = dict(consts)
        m["x"] = np.ascontiguousarray(x[c * SPB:(c + 1) * SPB])
        in_maps.append(m)

    import os
    trace = bool(int(os.environ.get("KERNEL_TRACE", "0")))
    res = run_bass_kernel_spmd(nc, in_maps, list(range(N_CORES)), trace=trace)
    kernel._last_results = res
    out = np.concatenate([np.asarray(r["out"]) for r in res.results], axis=0)
    return out.astype(np.float32)


kernel._last_results = None


# revision 34
# speedup vs baseline: 1.0944x; 1.0027x over previous
"""Fused Trainium2 kernel for nn_MultiHeadRelationalModule.

Data-parallel over 8 NeuronCores (8 samples each). The whole per-sample
pipeline (conv1 -> conv2 -> +coords -> K/Q/V proj -> LayerNorm ->
relational attention (4 heads, 596x596) -> softmax -> weighted sum ->
lin1 -> LN -> maxpool -> lin2 -> elu) runs on-chip; the big attention
maps never touch HBM.

v2: all large matmuls run in bf16 (4x faster per PE row than fp32 on
TRN2; fp32 needs 4 cycles/row, bf16 needs 1). PSUM accumulation stays
fp32. Q+K projections merged into one 128-partition matmul per head;
V projections merged across heads. Elementwise work balanced across
Act/DVE/Pool engines.

Key identities used:
  elu(x) + 1 == max(x + 1, min(exp(x), 1))        (exact)
  A' = elu(z)+1 fed to matmul with alin_w: subtract colsum(alin_w) in the
       following bias to undo the +1 (softmax bias becomes
       alin_b - alin_w.sum(0)).
  softmax over c2 with A2^T layout (c2 on partitions): exp on chip,
       denominator via an appended ones-column on V in the E matmul.
  LN(x) = (x - mu) * rsqrt(var + eps); affine params in this model are
       identity (ones/zeros), verified at runtime.
  max-pool commutes with the final LN (monotone affine map).
"""

import numpy as np
from contextlib import ExitStack

import concourse.bacc as bacc
import concourse.bass as bass
import concourse.mybir as mybir
import concourse.tile as tile
from concourse.bass_utils import run_bass_kernel_spmd

F32 = mybir.dt.float32
BF16 = mybir.dt.bfloat16
FP8 = mybir.dt.float8e4
ALSC = 16.0  # alin pre-scale into fp8e4m3 normal range; undone in exp2 scale
AF = mybir.ActivationFunctionType
ALU = mybir.AluOpType

N_CORES = 8
SPB = 8               # samples per core
N_PIX = 596
HEADS = 4
D = 64
CH = [(0, 128), (128, 256), (256, 384), (384, 512), (512, 596)]
FH = [(0, 512), (512, 596)]
SHIFTS = [(0, 0), (0, 1), (1, 0), (1, 1)]
LN_N = float(HEADS * N_PIX * D)       # 152576
LN2_N = float(N_PIX * D)              # 38144
EPS = 1e-5

_cache = {}


def _prep_consts(inp):
    """Host-side preprocessing of weights into kernel-friendly layouts."""
    f = np.float32
    c = {}
    conv1_w = np.asarray(inp["conv1_w"], f)
    c["w1s"] = np.ascontiguousarray(
        np.concatenate([conv1_w[:, :, di, dj].T for (di, dj) in SHIFTS], axis=1)
    )  # (4, 64)
    c["b1"] = np.ascontiguousarray(np.asarray(inp["conv1_b"], f)[:, None])  # (16,1)
    conv2_w = np.asarray(inp["conv2_w"], f)
    c["w2s"] = np.ascontiguousarray(
        np.concatenate([conv2_w[:, :, di, dj].T for (di, dj) in SHIFTS], axis=1)
    )  # (16, 128)
    c["b2"] = np.ascontiguousarray(np.asarray(inp["conv2_b"], f)[:, None])  # (32,1)

    p = np.arange(N_PIX)
    c["coords"] = np.ascontiguousarray(
        np.stack([(p % 4) / 4.0, (p // 4) / 149.0]).astype(f)
    )  # (2, 596)

    # Q/K projection merged per head: cols h*128:h*128+64 = Q (stacked rows
    # 0:64), cols h*128+64:h*128+128 = K (stacked rows 64:128).
    qp_w = np.asarray(inp["qp_w"], f)
    kp_w = np.asarray(inp["kp_w"], f)
    kqw2 = np.zeros((34, 512), f)
    qkb2 = np.zeros((128, HEADS), f)
    for h in range(HEADS):
        kqw2[:, h * 128:h * 128 + 64] = qp_w[:, h * 64:(h + 1) * 64]
        kqw2[:, h * 128 + 64:h * 128 + 128] = kp_w[:, h * 64:(h + 1) * 64]
        qkb2[0:64, h] = np.asarray(inp["qp_b"], f)[h * 64:(h + 1) * 64]
        qkb2[64:128, h] = np.asarray(inp["kp_b"], f)[h * 64:(h + 1) * 64]
    c["kqw2"] = kqw2
    c["qkb2"] = qkb2

    c["vw"] = np.ascontiguousarray(np.asarray(inp["vp_w"], f))  # (34, 256)
    vbb2 = np.zeros((128, 512), f)   # per head: [V bias (64) | 0 (ones blk)]
    for h in range(HEADS):
        vbb2[:, h * 128:h * 128 + 64] = np.asarray(inp["vp_b"], f)[None,
                                                                   h * 64:(h + 1) * 64]
    c["vbb2"] = vbb2

    c["qklin"] = np.ascontiguousarray(
        np.concatenate([np.asarray(inp["qlin_w"], f),
                        np.asarray(inp["klin_w"], f)], axis=0)
    )  # (128, 596): rows 0:64 qlin (Q), 64:128 klin (K)

    qkbias = np.zeros((128, 10), f)
    qkl_b = np.asarray(inp["qlin_b"], f) + np.asarray(inp["klin_b"], f)
    for ci, (c0, c1) in enumerate(CH):
        qkbias[0:c1 - c0, ci] = qkl_b[c0:c1]
        qkbias[0:c1 - c0, 5 + ci] = qkl_b[c0:c1] + 1.0
    c["qkbias"] = qkbias

    c["alin"] = np.ascontiguousarray(np.asarray(inp["alin_w"], f))  # (596, 596)
    # fp8e4m3 DoubleRowSwInterleave weight pairs for alin rows 0:512 (x16 so
    # the ~0.05-scale entries sit in e4m3's normal range; undone in exp2's
    # scale).  Per matmul slice: cols [A[m], B[m]] pairs, m descending.
    import ml_dtypes
    alin16 = np.pad(np.asarray(inp["alin_w"], f) * ALSC, ((0, 0), (0, 44)))
    for j in range(2):
        A = alin16[256 * j:256 * j + 128]
        B = alin16[256 * j + 128:256 * j + 256]
        buf = np.zeros((128, 1280), f)
        for ci in range(5):
            c0 = 128 * ci
            blk = np.empty((128, 256), f)
            blk[:, 0::2] = A[:, c0:c0 + 128][:, ::-1]
            blk[:, 1::2] = B[:, c0:c0 + 128][:, ::-1]
            buf[:, 2 * c0:2 * c0 + 256] = blk
        c[f"alin_i8_{j}"] = np.ascontiguousarray(
            buf.astype(ml_dtypes.float8_e4m3))

    expb = np.zeros((128, 5), f)
    eb = np.asarray(inp["alin_b"], f) - np.asarray(inp["alin_w"], f).sum(axis=0)
    for ci, (c0, c1) in enumerate(CH):
        expb[0:c1 - c0, ci] = eb[c0:c1]
    c["expb"] = expb

    l1 = np.zeros((128, 128), f)
    lin1_w = np.asarray(inp["lin1_w"], f)
    l1[:, 0:64] = lin1_w[0:128]
    l1[:, 64:128] = lin1_w[128:256]
    c["lin1w"] = l1
    c["bl1"] = np.ascontiguousarray(np.asarray(inp["lin1_b"], f)[:, None])  # (64,1)
    c["lin2w"] = np.ascontiguousarray(np.asarray(inp["lin2_w"], f))  # (64,10)
    bl2 = np.zeros((10, 2), f)
    bl2[:, 0] = np.asarray(inp["lin2_b"], f)
    bl2[:, 1] = np.asarray(inp["lin2_b"], f) + 1.0
    c["bl2"] = bl2
    c["ones_r"] = np.ones((1, 128), f)
    c["ones_c"] = np.ones((128, 1), f)
    c["epsc"] = np.full((1, 1), EPS, f)
    c["id34"] = np.eye(34, dtype=f)
    # LN-stat helper constants: per tensor T in (Q, K, V) with weights W_T
    # (34, 256) and bias b_T: sum(T) = s^T W_T 1 + 596*sum(b),
    # ssq(T) = sum_k w_k^T G w_k + 2 s^T (W_T b_T) + 596*||b_T||^2.
    wsum3 = np.zeros((34, 3), f)
    wb3 = np.zeros((34, 3), f)
    c3k = np.zeros((1, 3), f)
    cs3k = np.zeros((1, 3), f)
    for i, (wn, bn) in enumerate((("qp_w", "qp_b"), ("kp_w", "kp_b"),
                                  ("vp_w", "vp_b"))):
        W = np.asarray(inp[wn], np.float64)
        b = np.asarray(inp[bn], np.float64)
        wsum3[:, i] = W.sum(axis=1).astype(f)
        wb3[:, i] = (2.0 * (W @ b)).astype(f)
        c3k[0, i] = np.float32(596.0 * float(b @ b) / LN_N)
        cs3k[0, i] = np.float32(596.0 * float(b.sum()) / LN_N)
    c["wsum3"] = wsum3
    c["wb3"] = wb3
    c["c3k"] = c3k
    c["cs3k"] = cs3k
    return c


CONST_SHAPES = {
    "w1s": (4, 64), "b1": (16, 1), "w2s": (16, 128), "b2": (32, 1),
    "coords": (2, N_PIX), "kqw2": (34, 512), "qkb2": (128, HEADS),
    "vw": (34, 256), "vbb2": (128, 512),
    "qklin": (128, N_PIX), "qkbias": (128, 10), "alin": (N_PIX, N_PIX),
    "expb": (128, 5), "lin1w": (128, 128), "bl1": (64, 1), "lin2w": (64, 10),
    "bl2": (10, 2), "ones_r": (1, 128), "ones_c": (128, 1), "epsc": (1, 1),
    "id34": (34, 34), "wsum3": (34, 3), "wb3": (34, 3), "c3k": (1, 3),
    "cs3k": (1, 3),
}
CONST_FP8 = {"alin_i8_0": (128, 1280), "alin_i8_1": (128, 1280)}


def build_nc(spb=SPB):
    """Build the Bass program (same program runs SPMD on each core)."""
    nc = bacc.Bacc("TRN2", target_bir_lowering=False, debug=False)

    x_dram = nc.dram_tensor("x", [spb, 4, 151, 6], F32, kind="ExternalInput").ap()
    out_dram = nc.dram_tensor("out", [spb, 10], F32, kind="ExternalOutput").ap()
    cdram = {
        k: nc.dram_tensor(k, list(v), F32, kind="ExternalInput").ap()
        for k, v in CONST_SHAPES.items()
    }
    for k, v in CONST_FP8.items():
        cdram[k] = nc.dram_tensor(k, list(v), FP8, kind="ExternalInput").ap()

    with tile.TileContext(nc) as tc, ExitStack() as ctx:
        pc = ctx.enter_context(tc.tile_pool(name="consts", bufs=1))
        # SBUF pools
        px = ctx.enter_context(tc.tile_pool(name="px", bufs=2))
        ph1 = ctx.enter_context(tc.tile_pool(name="ph1", bufs=2))
        pfeat = ctx.enter_context(tc.tile_pool(name="pfeat", bufs=2))
        pqk = ctx.enter_context(tc.tile_pool(name="pqk", bufs=8))
        pqkb = ctx.enter_context(tc.tile_pool(name="pqkb", bufs=8))
        pv = ctx.enter_context(tc.tile_pool(name="pv", bufs=12))
        pat = ctx.enter_context(tc.tile_pool(name="pat", bufs=10))
        pet = ctx.enter_context(tc.tile_pool(name="pet", bufs=3))
        pext = ctx.enter_context(tc.tile_pool(name="pext", bufs=7))
        psq = ctx.enter_context(tc.tile_pool(name="psq", bufs=2))
        pst = ctx.enter_context(tc.tile_pool(name="pst", bufs=3))
        peall = ctx.enter_context(tc.tile_pool(name="peall", bufs=4))
        pfix = ctx.enter_context(tc.tile_pool(name="pfix", bufs=1))
        # PSUM pools (8 banks total: 2+2+2+2), phase-separated so sample
        # s+1's front-end never waits on sample s's tail.
        PS = bass.MemorySpace.PSUM
        ps_front = ctx.enter_context(tc.tile_pool(name="ps_front", bufs=1, space=PS))
        ps_at = ctx.enter_context(tc.tile_pool(name="ps_at", bufs=3, space=PS))
        ps_a2 = ctx.enter_context(tc.tile_pool(name="ps_a2", bufs=3, space=PS))
        ps_e = ctx.enter_context(tc.tile_pool(name="ps_e", bufs=1, space=PS))

        # ---- prefetch sample 0's input before the const DMAs ----
        x_t0 = px.tile([4, 151, 6], F32, name="x_t", tag="x")
        nc.sync.dma_start(out=x_t0[:, :, :], in_=x_dram[0])

        # ---- load constants (fp32) ----
        csb = {}
        for k, shp in CONST_SHAPES.items():
            if k == "alin":
                continue
            t = pc.tile(list(shp), F32, name=f"c_{k}")
            nc.sync.dma_start(out=t[:, :], in_=cdram[k][:, :])
            csb[k] = t
        alin4_f32 = pc.tile([84, N_PIX], F32, name="c_alin4")
        nc.sync.dma_start(out=alin4_f32[:, :], in_=cdram["alin"][512:596, :])
        alin_i8 = []
        for j in range(2):
            t = pc.tile([128, 1280], FP8, name=f"alin_i8_{j}")
            nc.sync.dma_start(out=t[:, :], in_=cdram[f"alin_i8_{j}"][:, :])
            alin_i8.append(t)

        # ---- one-time bf16 conversions of matmul operands ----
        def to_bf(name, src, shp):
            t = pc.tile(list(shp), BF16, name=name)
            nc.vector.tensor_copy(t[:, :], src[:, :])
            return t

        w1s_bf = to_bf("w1s_bf", csb["w1s"], (4, 64))
        w2s_bf = to_bf("w2s_bf", csb["w2s"], (16, 128))
        coords_bf = to_bf("coords_bf", csb["coords"], (2, N_PIX))
        kqw2_bf = to_bf("kqw2_bf", csb["kqw2"], (34, 512))
        vw_bf = to_bf("vw_bf", csb["vw"], (34, 256))
        qklin_bf = to_bf("qklin_bf", csb["qklin"], (128, N_PIX))
        lin1w_bf = to_bf("lin1w_bf", csb["lin1w"], (128, 128))
        alin_bf4 = pc.tile([84, 640], BF16, name="alin_bf4")
        nc.vector.memset(alin_bf4[:, 596:640], 0.0)
        nc.vector.tensor_scalar_mul(alin_bf4[:, 0:N_PIX], alin4_f32[:, :],
                                    ALSC)
        id34_bf = to_bf("id34_bf", csb["id34"], (34, 34))
        ones_bf = pc.tile([128, 1], BF16, name="ones_bf")
        nc.vector.memset(ones_bf[:, :], 1.0)
        emax_all = pfix.tile([64, spb], F32, name="emax_all")
        emax_raw = pfix.tile([64, spb], F32, name="emax_raw")
        stats2_all = pfix.tile([1, 2 * spb], F32, name="stats2_all")

        # ================= pipelined per-sample stages =================
        W84 = 84 * HEADS
        vbb3c = csb["vbb2"].rearrange("p (h c) -> p h c", c=128)

        def front_a(s):
            """x load/cast + conv1 + conv2 + coords -> feats."""
            S = {"s": s}
            if s == 0:
                x_t = x_t0
            else:
                x_t = px.tile([4, 151, 6], F32, name="x_t", tag="x")
                nc.sync.dma_start(out=x_t[:, :, :], in_=x_dram[s])
            x_bf = px.tile([4, 151, 6], BF16, name="x_bf", tag="xbf")
            nc.gpsimd.tensor_copy(x_bf[:, :, :], x_t[:, :, :])

            h1 = ph1.tile([16, 750], BF16, name="h1", tag="h1")
            h1v = h1.rearrange("c (h w) -> c h w", w=5)
            for (r0, nr, dst0) in ((0, 102, 0), (102, 48, 510)):
                cps = ps_front.tile([16, nr * 5], F32, name="c1ps", tag="fr")
                for si, (di, dj) in enumerate(SHIFTS):
                    nc.tensor.matmul(
                        cps[:, :],
                        w1s_bf[:, si * 16:(si + 1) * 16],
                        x_bf[:, di + r0:di + r0 + nr, dj:dj + 5],
                        start=(si == 0), stop=(si == 3),
                    )
                nc.scalar.activation(h1[:, dst0:dst0 + nr * 5], cps[:, :],
                                     AF.Relu, bias=csb["b1"][:, 0:1])

            feats = pfeat.tile([34, N_PIX], BF16, name="feats", tag="feats")
            nc.gpsimd.tensor_copy(feats[32:34, :], coords_bf[:, :])
            for (r0, nr, dst0) in ((0, 128, 0), (128, 21, 512)):
                cps = ps_front.tile([32, nr * 4], F32, name="c2ps", tag="fr")
                for si, (di, dj) in enumerate(SHIFTS):
                    nc.tensor.matmul(
                        cps[:, :],
                        w2s_bf[:, si * 32:(si + 1) * 32],
                        h1v[:, di + r0:di + r0 + nr, dj:dj + 4],
                        start=(si == 0), stop=(si == 3),
                    )
                nc.scalar.activation(feats[0:32, dst0:dst0 + nr * 4], cps[:, :],
                                     AF.Relu, bias=csb["b2"][:, 0:1])
            S["feats"] = feats
            return S

        def front_b(S):
            """LN stats from s/G on the PE, then K/Q/V projections."""
            feats = S["feats"]
            # s = sum_f feats[:, f]; G = feats @ feats^T (via PE transposes)
            s_sb = pst.tile([34, 1], F32, name="s_sb", tag="s_sb")
            nc.vector.tensor_reduce(s_sb[:, :], feats[:, :],
                                    axis=mybir.AxisListType.X, op=ALU.add)
            g_ps = ps_front.tile([34, 34], F32, name="g_ps", tag="fr")
            for ci, (c0, c1) in enumerate(CH):
                csz = c1 - c0
                ft_ps = ps_a2.tile([128, 34], BF16, name="ft_ps", tag="a2")
                nc.tensor.transpose(ft_ps[0:csz, :], feats[:, c0:c1],
                                    id34_bf[:, :])
                ft_sb = pst.tile([128, 34], BF16, name="ft_sb", tag="ft")
                nc.vector.tensor_copy(ft_sb[0:csz, :], ft_ps[0:csz, :])
                nc.tensor.matmul(g_ps[:, :], ft_sb[0:csz, :],
                                 ft_sb[0:csz, :],
                                 start=(ci == 0), stop=(ci == 4))
            g_sb = pst.tile([34, 34], BF16, name="g_sb", tag="g_sb")
            nc.vector.tensor_copy(g_sb[:, :], g_ps[:, :])
            gw2_ps = ps_front.tile([34, 512], F32, name="gw2_ps", tag="fr")
            nc.tensor.matmul(gw2_ps[:, :], g_sb[:, :], kqw2_bf[:, :],
                             start=True, stop=True)
            d2 = psq.tile([34, 768], BF16, name="d2", tag="d2")
            nc.vector.tensor_tensor(d2[:, 0:512], csb["kqw2"][:, :],
                                    gw2_ps[:, :], op=ALU.mult)
            gwv_ps = ps_front.tile([34, 256], F32, name="gwv_ps", tag="fr")
            nc.tensor.matmul(gwv_ps[:, :], g_sb[:, :], vw_bf[:, :],
                             start=True, stop=True)
            nc.vector.tensor_tensor(d2[:, 512:768], csb["vw"][:, :],
                                    gwv_ps[:, :], op=ALU.mult)
            cs2_ps = ps_front.tile([1, 512], F32, name="cs2_ps", tag="fr")
            nc.tensor.matmul(cs2_ps[:, :], ones_bf[0:34, 0:1], d2[:, 0:512],
                             start=True, stop=True)
            csv_ps = ps_front.tile([1, 256], F32, name="csv_ps", tag="fr")
            nc.tensor.matmul(csv_ps[:, :], ones_bf[0:34, 0:1], d2[:, 512:768],
                             start=True, stop=True)
            # per-(h, qk) partial ssq, then fold heads
            r1 = pst.tile([1, 8], F32, name="r1", tag="r1")
            nc.vector.tensor_reduce(
                r1[:, :].rearrange("p (h t u) -> p h t u", t=2, u=1),
                cs2_ps[:, :].rearrange("p (h t d) -> p h t d", t=2, d=64),
                axis=mybir.AxisListType.X, op=ALU.add)
            ssqr = pst.tile([1, 3], F32, name="ssqr", tag="ssqr")
            nc.vector.tensor_reduce(
                ssqr[:, 0:2].rearrange("p (t u) -> p t u", u=1),
                r1[:, :].rearrange("p (h t) -> p t h", t=2),
                axis=mybir.AxisListType.X, op=ALU.add)
            nc.vector.tensor_reduce(ssqr[:, 2:3], csv_ps[:, :],
                                    axis=mybir.AxisListType.X, op=ALU.add)
            stats_ps = ps_front.tile([1, 6], F32, name="stats_ps", tag="fr")
            nc.tensor.matmul(stats_ps[0:1, 0:3], s_sb[:, :],
                             csb["wsum3"][:, :], start=True, stop=True)
            nc.tensor.matmul(stats_ps[0:1, 3:6], s_sb[:, :],
                             csb["wb3"][:, :], start=True, stop=True)
            mu3 = pst.tile([1, 3], F32, name="mu3", tag="mu3")
            nc.vector.scalar_tensor_tensor(mu3[:, :], stats_ps[0:1, 0:3],
                                           1.0 / LN_N, csb["cs3k"][0:1, :],
                                           op0=ALU.mult, op1=ALU.add)
            tsq = pst.tile([1, 3], F32, name="tsq", tag="tsq")
            nc.vector.tensor_tensor(tsq[:, :], ssqr[:, :], stats_ps[0:1, 3:6],
                                    op=ALU.add)
            msq3 = pst.tile([1, 3], F32, name="msq3", tag="msq3")
            nc.vector.scalar_tensor_tensor(msq3[:, :], tsq[:, :], 1.0 / LN_N,
                                           csb["c3k"][0:1, :],
                                           op0=ALU.mult, op1=ALU.add)
            S["mu3"] = mu3
            S["msq3"] = msq3

            # projections (plain copies; no accumulation needed)
            stacked = []
            for h in range(HEADS):
                st_t = pqk.tile([128, N_PIX], BF16, name="st_t", tag="qk")
                stacked.append(st_t)
                pps = ps_front.tile([128, 512], F32, name="pps", tag="fr")
                pps2 = ps_front.tile([128, 84], F32, name="pps2", tag="fr")
                nc.tensor.matmul(pps[:, :], kqw2_bf[:, h * 128:(h + 1) * 128],
                                 feats[:, 0:512], start=True, stop=True)
                nc.tensor.matmul(pps2[:, :], kqw2_bf[:, h * 128:(h + 1) * 128],
                                 feats[:, 512:596], start=True, stop=True)
                nc.vector.tensor_scalar_add(st_t[:, 0:512], pps[:, :],
                                            csb["qkb2"][:, h:h + 1])
                nc.vector.tensor_scalar_add(st_t[:, 512:596], pps2[:, :],
                                            csb["qkb2"][:, h:h + 1])

            vtiles = []
            for ci, (c0, c1) in enumerate(CH):
                csz = c1 - c0
                vps = ps_front.tile([128, 256], F32, name="vps", tag="fr")
                nc.tensor.matmul(vps[0:csz, :], feats[:, c0:c1],
                                 vw_bf[:, :], start=True, stop=True)
                vt = pv.tile([128, 512], BF16, name="vt", tag="v")
                vt3 = vt.rearrange("p (h c) -> p h c", c=128)
                vps3 = vps.rearrange("p (h c) -> p h c", c=64)
                nc.vector.memset(vt3[0:csz, :, 64:128], 1.0)
                nc.vector.scalar_tensor_tensor(
                    vt3[0:csz, :, 0:64], vps3[0:csz, :, :], 1.0,
                    vbb3c[0:csz, :, 0:64],
                    op0=ALU.mult, op1=ALU.add)
                vtiles.append(vt)
            S["stacked"] = stacked
            S["vtiles"] = vtiles
            return S

        def front_c(S):
            """LN scalar pipeline + LN apply (fp32 -> bf16)."""
            mu3, msq3 = S["mu3"], S["msq3"]
            nmu2 = pst.tile([1, 3], F32, name="nmu2", tag="nmu2")
            nc.vector.scalar_tensor_tensor(nmu2[:, :], mu3[:, :], -1.0,
                                           mu3[:, :],
                                           op0=ALU.mult, op1=ALU.mult)
            var3 = pst.tile([1, 3], F32, name="var3", tag="var3")
            nc.vector.tensor_tensor(var3[:, :], msq3[:, :], nmu2[:, :],
                                    op=ALU.add)
            std3 = pst.tile([1, 3], F32, name="std3", tag="std3")
            nc.scalar.activation(std3[:, :], var3[:, :], AF.Sqrt,
                                 bias=csb["epsc"][0:1, 0:1])
            rsnmr = pst.tile([1, 6], F32, name="rsnmr", tag="rsnmr")
            rsv = rsnmr.rearrange("p (a b) -> p a b", b=2)
            nc.vector.reciprocal(rsv[:, :, 0:1], std3[:, :])
            nc.vector.scalar_tensor_tensor(rsv[:, :, 1:2], mu3[:, :], -1.0,
                                           rsv[:, :, 0:1],
                                           op0=ALU.mult, op1=ALU.mult)
            bc_ps = ps_a2.tile([128, 6], F32, name="bc_ps", tag="a2")
            nc.tensor.matmul(bc_ps[:, :], csb["ones_r"][0:1, :], rsnmr[:, :],
                             start=True, stop=True)
            bc = pst.tile([128, 6], F32, name="bc", tag="bc")
            nc.vector.tensor_copy(bc[:, :], bc_ps[:, :])
            # bc cols: [rsQ, nmrQ, rsK, nmrK, rsV, nmrV]

            stacked_bf = []
            for h in range(HEADS):
                sb = pqkb.tile([128, N_PIX], BF16, name="st_bf", tag="qkb")
                stacked_bf.append(sb)
                nc.vector.tensor_scalar(sb[0:64, :], S["stacked"][h][0:64, :],
                                        bc[0:64, 0:1], bc[0:64, 1:2],
                                        op0=ALU.mult, op1=ALU.add)
                nc.vector.tensor_scalar(sb[64:128, :],
                                        S["stacked"][h][64:128, :],
                                        bc[0:64, 2:3], bc[0:64, 3:4],
                                        op0=ALU.mult, op1=ALU.add)
            for ci, (c0, c1) in enumerate(CH):
                csz = c1 - c0
                vt3 = S["vtiles"][ci].rearrange("p (h c) -> p h c", c=128)
                nc.vector.tensor_scalar(vt3[0:csz, :, 0:64],
                                        vt3[0:csz, :, 0:64],
                                        bc[0:csz, 4:5], bc[0:csz, 5:6],
                                        op0=ALU.mult, op1=ALU.add)
            S["stacked_bf"] = stacked_bf
            S["eall"] = [peall.tile([128, N_PIX], BF16, name=f"eall{i}",
                                    tag="eall") for i in range(2)]
            return S

        # ---- attention stages (pipeline carried across samples) ----
        def at_chunk(S, p, ci, dest):
            c0, c1 = CH[ci]
            csz = c1 - c0
            w = 512 if not p["merged"] else W84
            aps = ps_at.tile([128, 512], F32, name="aps", tag="at")
            if p["merged"]:
                for h in range(HEADS):
                    nc.tensor.matmul(aps[0:csz, h * 84:(h + 1) * 84],
                                     qklin_bf[:, c0:c1],
                                     S["stacked_bf"][h][:, 512:596],
                                     start=True, stop=True)
            else:
                nc.tensor.matmul(aps[0:csz, 0:512],
                                 qklin_bf[:, c0:c1],
                                 S["stacked_bf"][p["h"]][:, 0:512],
                                 start=True, stop=True)
            et = pet.tile([128, 512], F32, name="et", tag="et")
            nc.scalar.activation(et[0:csz, 0:w], aps[0:csz, 0:w],
                                 AF.Exp,
                                 bias=csb["qkbias"][0:csz, ci:ci + 1])
            nc.gpsimd.tensor_scalar_min(et[0:csz, 0:w],
                                        et[0:csz, 0:w], 1.0)
            nc.vector.scalar_tensor_tensor(
                dest[0:csz, 0:w], aps[0:csz, 0:w],
                csb["qkbias"][0:csz, 5 + ci:6 + ci],
                et[0:csz, 0:w], op0=ALU.add, op1=ALU.max)

        def e_c2(st, c2i):
            S, p, tiles = st["S"], st["p"], st["tiles"]
            c20, c21 = CH[c2i]
            c2sz = c21 - c20
            w = 512 if not p["merged"] else W84
            if c2i == 0:
                st["eps"] = ps_e.tile([128, 512], F32, name="eps_t", tag="e")
            eps_t = st["eps"]
            a2ps = ps_a2.tile([128, 512], F32, name="a2ps", tag="a2")
            for j in range(2):
                nc.tensor.matmul(
                    a2ps[0:128, 0:w],
                    alin_i8[j][:, 256 * c2i:256 * c2i + 256],
                    tiles[j][:, :, 0:w],
                    start=(j == 0), stop=False,
                    perf_mode=mybir.MatmulPerfMode.DoubleRowSwInterleave)
            nc.tensor.matmul(a2ps[0:128, 0:w],
                             alin_bf4[:, 128 * c2i:128 * c2i + 128],
                             tiles[2][0:84, 0:w],
                             start=False, stop=True)
            ext = pext.tile([128, 512], BF16, name="ext", tag="ext")
            nc.scalar.activation(ext[0:c2sz, 0:w],
                                 a2ps[0:c2sz, 0:w], AF.Exp,
                                 bias=csb["expb"][0:c2sz, c2i:c2i + 1],
                                 scale=1.0 / ALSC)
            if p["merged"]:
                # PSUM accumulation groups must not interleave within a
                # bank's 2KB zero region: buffer the ext tiles and run the
                # four per-head accumulations sequentially in e_tail.
                st.setdefault("exts", []).append(ext)
            else:
                nc.tensor.matmul(eps_t[:, 0:512],
                                 S["vtiles"][c2i][0:c2sz,
                                                  p["h"] * 128:
                                                  (p["h"] + 1) * 128],
                                 ext[0:c2sz, 0:512],
                                 start=(c2i == 0), stop=(c2i == 4))

        def e_tail(st):
            S, p, eps_t = st["S"], st["p"], st["eps"]
            w = 512 if not p["merged"] else W84
            eall = S["eall"]
            if p["merged"]:
                for h in range(HEADS):
                    for c2i, (c20, c21) in enumerate(CH):
                        c2sz = c21 - c20
                        nc.tensor.matmul(
                            eps_t[:, h * 84:(h + 1) * 84],
                            S["vtiles"][c2i][0:c2sz, h * 128:(h + 1) * 128],
                            st["exts"][c2i][0:c2sz, h * 84:(h + 1) * 84],
                            start=(c2i == 0), stop=(c2i == 4))
            recip64 = pst.tile([64, 512], F32, name="recip64", tag="recip")
            nc.vector.reciprocal(recip64[:, 0:w], eps_t[64:128, 0:w])
            if p["merged"]:
                for h in range(HEADS):
                    nc.vector.tensor_tensor(
                        eall[h // 2][(h % 2) * 64:(h % 2) * 64 + 64, 512:596],
                        eps_t[0:64, h * 84:(h + 1) * 84],
                        recip64[:, h * 84:(h + 1) * 84], op=ALU.mult)
            else:
                h = p["h"]
                nc.vector.tensor_tensor(
                    eall[h // 2][(h % 2) * 64:(h % 2) * 64 + 64, 0:512],
                    eps_t[0:64, 0:512], recip64[:, 0:512], op=ALU.mult)

        pending = [None]

        def do_pass(S, p):
            pair0 = pat.tile([128, 2, 512], FP8, name="atp0", tag="atile")
            pair1 = pat.tile([128, 2, 512], FP8, name="atp1", tag="atile")
            at4 = pat.tile([128, 512], BF16, name="at4", tag="a4")
            tiles = [pair0, pair1, at4]
            prev = pending[0]
            at_chunk(S, p, 0, pair0[:, 0, :])
            at_chunk(S, p, 1, pair0[:, 1, :])
            if prev is None:
                at_chunk(S, p, 2, pair1[:, 0, :])
                at_chunk(S, p, 3, pair1[:, 1, :])
                at_chunk(S, p, 4, at4[:, :])
            else:
                e_c2(prev, 0)
                at_chunk(S, p, 2, pair1[:, 0, :])
                e_c2(prev, 1)
                at_chunk(S, p, 3, pair1[:, 1, :])
                e_c2(prev, 2)
                at_chunk(S, p, 4, at4[:, :])
                e_c2(prev, 3)
                e_c2(prev, 4)
                e_tail(prev)
            pending[0] = {"S": S, "p": p, "tiles": tiles}

        def flush_pipe():
            prev = pending[0]
            for c2i in range(5):
                e_c2(prev, c2i)
            e_tail(prev)
            pending[0] = None

        def tail(S):
            """lin1 + LN2 raw stats (scalar pipeline batched at the end)."""
            s, eall = S["s"], S["eall"]
            e2 = psq.tile([64, N_PIX], F32, name="e2", tag="e2")
            ls2 = pst.tile([64, 2], F32, name="ls2", tag="ls2")
            lpart = pst.tile([64, 2], F32, name="lpart", tag="lpart")
            for (f0, f1) in FH:
                fsz = f1 - f0
                lps = ps_e.tile([64, 512], F32, name="lps", tag="e")
                for ck in range(2):
                    nc.tensor.matmul(lps[:, 0:fsz],
                                     lin1w_bf[:, ck * 64:(ck + 1) * 64],
                                     eall[ck][:, f0:f1],
                                     start=(ck == 0), stop=(ck == 1))
                nc.scalar.activation(e2[:, f0:f1], lps[:, 0:fsz], AF.Relu,
                                     bias=csb["bl1"][:, 0:1],
                                     accum_out=lpart[:, (0 if f0 == 0 else 1):
                                                     (1 if f0 == 0 else 2)])
            nc.vector.tensor_reduce(ls2[:, 0:1], lpart[:, :],
                                    axis=mybir.AxisListType.X, op=ALU.add)
            sqe = psq.tile([64, N_PIX], F32, name="sqe", tag="sqe")
            nc.scalar.activation(sqe[:, :], e2[:, :], AF.Square,
                                 accum_out=ls2[:, 1:2])
            nc.vector.tensor_reduce(emax_raw[:, s:s + 1], e2[:, :],
                                    axis=mybir.AxisListType.X, op=ALU.max)
            st2 = ps_at.tile([1, 2], F32, name="st2", tag="at")
            nc.tensor.matmul(st2[0:1, :], csb["ones_c"][0:64, 0:1], ls2[:, :],
                             start=True, stop=True)
            nc.vector.tensor_copy(stats2_all[:, 2 * s:2 * s + 2], st2[0:1, :])

        # ---- pipelined schedule: sample s+1's front-end is emitted between
        # sample s's attention passes; the at/e pass pipeline is carried
        # across the sample boundary.
        S = front_a(0)
        front_b(S)
        front_c(S)
        states = {0: S}
        for s in range(spb):
            S = states[s]
            plist = ([dict(h=h, merged=False) for h in range(HEADS)]
                     + [dict(h=None, merged=True)])
            do_pass(S, plist[0])
            if s > 0:
                tail(states.pop(s - 1))
            do_pass(S, plist[1])
            if s + 1 < spb:
                Sn = front_a(s + 1)
            do_pass(S, plist[2])
            if s + 1 < spb:
                front_b(Sn)
            do_pass(S, plist[3])
            if s + 1 < spb:
                front_c(Sn)
                states[s + 1] = Sn
            do_pass(S, plist[4])
        flush_pipe()
        tail(states.pop(spb - 1))

        # ---------------- batched LN2 scalar pipeline (all samples) --------
        m2a = pst.tile([1, 2 * spb], F32, name="m2a", tag="m2a")
        m2av = m2a.rearrange("p (a b) -> p a b", b=2)
        nc.vector.tensor_scalar_mul(m2a[:, :], stats2_all[:, :], 1.0 / LN2_N)
        nmu2a = pst.tile([1, spb], F32, name="nmu2a", tag="nmu2a")
        nc.vector.scalar_tensor_tensor(nmu2a[:, :],
                                       m2av[:, :, 0:1], -1.0, m2av[:, :, 0:1],
                                       op0=ALU.mult, op1=ALU.mult)
        var2a = pst.tile([1, spb], F32, name="var2a", tag="var2a")
        nc.vector.tensor_tensor(var2a[:, :], m2av[:, :, 1:2], nmu2a[:, :],
                                op=ALU.add)
        std2a = pst.tile([1, spb], F32, name="std2a", tag="std2a")
        nc.scalar.activation(std2a[:, :], var2a[:, :], AF.Sqrt,
                             bias=csb["epsc"][0:1, 0:1])
        rs2a = pst.tile([1, 2 * spb], F32, name="rs2a", tag="rs2a")
        rs2av = rs2a.rearrange("p (a b) -> p a b", b=2)
        nc.vector.reciprocal(rs2av[:, :, 0:1], std2a[:, :])
        nc.vector.scalar_tensor_tensor(rs2av[:, :, 1:2],
                                       m2av[:, :, 0:1], -1.0,
                                       rs2av[:, :, 0:1],
                                       op0=ALU.mult, op1=ALU.mult)
        bc2p = ps_at.tile([64, 2 * spb], F32, name="bc2p", tag="at")
        nc.tensor.matmul(bc2p[:, :], csb["ones_r"][0:1, 0:64], rs2a[:, :],
                         start=True, stop=True)
        bc2 = pst.tile([64, 2 * spb], F32, name="bc2", tag="bc2")
        nc.vector.tensor_copy(bc2[:, :], bc2p[:, :])
        for s in range(spb):
            nc.vector.tensor_scalar(emax_all[:, s:s + 1], emax_raw[:, s:s + 1],
                                    bc2[:, 2 * s:2 * s + 1],
                                    bc2[:, 2 * s + 1:2 * s + 2],
                                    op0=ALU.mult, op1=ALU.add)

        # ---------------- lin2 + final elu ----------------
        l2ps = ps_at.tile([10, spb], F32, name="l2ps", tag="at")
        nc.tensor.matmul(l2ps[:, :], csb["lin2w"][:, :], emax_all[:, :],
                         start=True, stop=True)
        fe = pst.tile([10, spb], F32, name="fe", tag="fe")
        nc.scalar.activation(fe[:, :], l2ps[:, :], AF.Exp,
                             bias=csb["bl2"][:, 0:1])
        nc.vector.tensor_scalar(fe[:, :], fe[:, :], 1.0, -1.0,
                                op0=ALU.min, op1=ALU.add)
        out_sb = pst.tile([10, spb], F32, name="out_sb", tag="out_sb")
        nc.vector.scalar_tensor_tensor(out_sb[:, :], l2ps[:, :],
                                       csb["bl2"][:, 0:1], fe[:, :],
                                       op0=ALU.add, op1=ALU.max)
        nc.sync.dma_start(out=out_dram.rearrange("s t -> t s"), in_=out_sb[:, :])

    return nc


def _reference_numpy(inp):
    """Pure-numpy fallback (only used if LN affine params are nontrivial)."""
    def ln(x, g=None, b=None):
        axes = tuple(range(1, x.ndim))
        mu = x.mean(axis=axes, keepdims=True)
        var = x.var(axis=axes, keepdims=True)
        y = (x - mu) / np.sqrt(var + EPS)
        return y * g + b if g is not None else y

    def elu(x):
        return np.where(x > 0, x, np.expm1(np.minimum(x, 0)))

    x = np.asarray(inp["x"], np.float64)
    N = x.shape[0]
    w1, b1 = np.asarray(inp["conv1_w"], np.float64), np.asarray(inp["conv1_b"], np.float64)
    h = np.zeros((N, 16, 150, 5))
    for di in range(2):
        for dj in range(2):
            h += np.einsum("oc,nchw->nohw", w1[:, :, di, dj],
                           x[:, :, di:di + 150, dj:dj + 5])
    h = np.maximum(h + b1[None, :, None, None], 0)
    w2, b2 = np.asarray(inp["conv2_w"], np.float64), np.asarray(inp["conv2_b"], np.float64)
    h2 = np.zeros((N, 32, 149, 4))
    for di in range(2):
        for dj in range(2):
            h2 += np.einsum("oc,nchw->nohw", w2[:, :, di, dj],
                            h[:, :, di:di + 149, dj:dj + 4])
    h2 = np.maximum(h2 + b2[None, :, None, None], 0)
    p = np.arange(N_PIX)
    xc, yc = (p % 4) / 4.0, (p // 4) / 149.0
    feats = np.concatenate(
        [h2.transpose(0, 2, 3, 1).reshape(N, N_PIX, 32),
         np.broadcast_to(np.stack([xc, yc], 1)[None], (N, N_PIX, 2))], axis=2)

    def proj(wn, bn, gn, bn2):
        P = (feats @ np.asarray(inp[wn], np.float64) + np.asarray(inp[bn], np.float64))
        P = P.reshape(N, N_PIX, HEADS, D).transpose(0, 2, 1, 3)
        return ln(P, np.asarray(inp[gn], np.float64), np.asarray(inp[bn2], np.float64))

    K = proj("kp_w", "kp_b", "knorm_g", "knorm_b")
    Q = proj("qp_w", "qp_b", "qnorm_g", "qnorm_b")
    V = proj("vp_w", "vp_b", "vnorm_g", "vnorm_b")
    A = elu(Q @ np.asarray(inp["qlin_w"], np.float64) + np.asarray(inp["qlin_b"], np.float64)
            + K @ np.asarray(inp["klin_w"], np.float64) + np.asarray(inp["klin_b"], np.float64))
    A = A @ np.asarray(inp["alin_w"], np.float64) + np.asarray(inp["alin_b"], np.float64)
    A = A - A.max(axis=-1, keepdims=True)
    A = np.exp(A)
    A = A / A.sum(axis=-1, keepdims=True)
    E = np.einsum("bhfc,bhcd->bhfd", A, V)
    E = E.transpose(0, 2, 1, 3).reshape(N, N_PIX, HEADS * D)
    E = np.maximum(E @ np.asarray(inp["lin1_w"], np.float64)
                   + np.asarray(inp["lin1_b"], np.float64), 0)
    E = ln(E)
    E = E.max(axis=1)
    out = E @ np.asarray(inp["lin2_w"], np.float64) + np.asarray(inp["lin2_b"], np.float64)
    return elu(out).astype(np.float32)


def kernel(**inputs):
    trivial = (np.all(np.asarray(inputs["knorm_g"]) == 1.0)
               and np.all(np.asarray(inputs["knorm_b"]) == 0.0)
               and np.all(np.asarray(inputs["qnorm_g"]) == 1.0)
               and np.all(np.asarray(inputs["qnorm_b"]) == 0.0)
               and np.all(np.asarray(inputs["vnorm_g"]) == 1.0)
               and np.all(np.asarray(inputs["vnorm_b"]) == 0.0))
    if not trivial:
        return _reference_numpy(inputs)

    x = np.ascontiguousarray(np.asarray(inputs["x"], np.float32))
    n = x.shape[0]
    assert n == N_CORES * SPB, f"expected batch {N_CORES * SPB}, got {n}"
    consts = _prep_consts(inputs)

    if "nc" not in _cache:
        nc = build_nc(SPB)
        nc.compile()
        _cache["nc"] = nc
    nc = _cache["nc"]

    in_maps = []
    for c in range(N_CORES):
        m = dict(consts)
        m["x"] = np.ascontiguousarray(x[c * SPB:(c + 1) * SPB])
        in_maps.append(m)

    import os
    trace = bool(int(os.environ.get("KERNEL_TRACE", "0")))
    res = run_bass_kernel_spmd(nc, in_maps, list(range(N_CORES)), trace=trace)
    kernel._last_results = res
    out = np.concatenate([np.asarray(r["out"]) for r in res.results], axis=0)
    return out.astype(np.float32)


kernel._last_results = None
